# revision 31
# baseline (speedup 1.0000x reference)
"""CrystalGNN (SchNet-style) Trainium2 Bass kernel — self-contained.

Sharding: nodes/graphs block-partitioned across 8 NeuronCores (graph-aligned
slabs); edges partitioned by owner(dst) and grouped by 128-node dst tile;
small weights replicated. Per conv: h = x@W1+b1 (fp16, slab-local) ->
AllGather -> per-tile edge pipeline (dma_gather h[src], RBF+block-diag Web
matmul for f, m = h*f, one-hot scatter matmul into PSUM) -> x update +
softplus. Mean-pool via one-hot matmul + small MLP heads on-device.
"""
"""Host-side sharding/preprocessing + numpy device-model for the CrystalGNN kernel.

Everything is parameterized by the problem dims so the same code paths can be
exercised at a small scale in CoreSim and at full scale on hardware.
"""
import numpy as np

F = 64          # atom feats
NRBF = 10
NCONV = 3
H = 64
CORES = 8


def ceil_div(a, b):
    return (a + b - 1) // b


def round_up(a, b):
    return ceil_div(a, b) * b


class Prep:
    """Per-problem host preprocessing. All outputs are numpy arrays keyed for
    the bass kernel's DRAM tensors (one dict per core)."""

    def __init__(self, x_ids, edge_index, edge_attr, batch, weights, n_graphs,
                 locut=None):
        N = x_ids.shape[0]
        E = edge_index.shape[1]
        G = n_graphs
        assert G % CORES == 0
        gpc = G // CORES  # graphs per core
        self.N, self.E, self.G, self.gpc = N, E, G, gpc

        batch = np.asarray(batch).astype(np.int64)
        x_ids = np.asarray(x_ids).astype(np.int64)
        src = np.asarray(edge_index[0]).astype(np.int64)
        dst = np.asarray(edge_index[1]).astype(np.int64)
        d = np.asarray(edge_attr).astype(np.float32)

        # graph -> node range (batch is sorted)
        gstart = np.searchsorted(batch, np.arange(G), side="left")
        gend = np.searchsorted(batch, np.arange(G), side="right")
        # core k owns graphs [k*gpc, (k+1)*gpc) -> nodes [cstart[k], cend[k])
        cstart = gstart[np.arange(CORES) * gpc]
        cend = np.append(cstart[1:], N)
        own = cend - cstart
        max_own = int(own.max())
        # slab size: per-core node capacity, multiple of 128
        SLAB = round_up(max_own, 128)
        NT = SLAB // 128  # node tiles per core
        self.SLAB, self.NT = SLAB, NT
        self.cstart, self.cend = cstart, cend

        # slab row of each global node
        owner = np.searchsorted(cstart, np.arange(N), side="right") - 1
        srow = SLAB * owner + (np.arange(N) - cstart[owner])
        self.owner, self.srow = owner, srow

        # int16 split point for gather indices (slab rows)
        self.LOCUT = min(32768, CORES * SLAB) if locut is None else locut

        # ---- edge partition: edge belongs to owner[dst], tile = local dst block
        e_owner = owner[dst]
        e_tile = (dst - cstart[e_owner]) // 128
        e_dstloc = (dst - cstart[e_owner]) % 128           # local id within tile
        e_srow = srow[src]
        e_lo = e_srow < self.LOCUT

        # per (core, tile): count lo/hi edges
        # order edges by (core, tile, hi?, arbitrary)
        key = ((e_owner * NT + e_tile) * 2 + (~e_lo).astype(np.int64))
        order = np.argsort(key, kind="stable")
        s_core = e_owner[order]
        s_tile = e_tile[order]
        s_lo = e_lo[order]
        s_d = d[order]
        s_dstloc = e_dstloc[order]
        s_srow = e_srow[order]

        # counts
        n_lo = np.zeros((CORES, NT), np.int64)
        n_hi = np.zeros((CORES, NT), np.int64)
        np.add.at(n_lo, (e_owner[e_lo], e_tile[e_lo]), 1)
        np.add.at(n_hi, (e_owner[~e_lo], e_tile[~e_lo]), 1)
        LO_BLKS = int(ceil_div(n_lo.max(), 128))
        HI_BLKS = int(ceil_div(n_hi.max(), 128))
        BK = LO_BLKS + HI_BLKS
        self.LO_BLKS, self.HI_BLKS, self.BK = LO_BLKS, HI_BLKS, BK
        self.n_lo, self.n_hi = n_lo, n_hi

        LO_SLOTS = LO_BLKS * 128
        HI_SLOTS = HI_BLKS * 128
        SLOTS = BK * 128

        # ---- per-core packed arrays
        # slot s of (core,tile): s in [0, LO_SLOTS) lo edges; [LO_SLOTS, SLOTS) hi
        d_arr = np.zeros((CORES, NT, 128, BK), np.float32)
        dst_arr = np.full((CORES, NT, 128, BK), -1.0, np.float16)
        idxlo = np.zeros((CORES, NT, LO_SLOTS), np.int16)
        idxhi = np.zeros((CORES, NT, HI_SLOTS), np.int16)

        # fill using the sorted stream
        # positions within each (core,tile,lo/hi) group
        grp_key = (s_core * NT + s_tile) * 2 + (~s_lo).astype(np.int64)
        # index within group
        uniq, first_idx = np.unique(grp_key, return_index=True)
        pos_in_grp = np.arange(len(grp_key)) - np.repeat(
            first_idx, np.diff(np.append(first_idx, len(grp_key)))
        )
        slot = np.where(s_lo, pos_in_grp, LO_SLOTS + pos_in_grp)
        p = slot % 128
        b = slot // 128
        d_arr[s_core, s_tile, p, b] = s_d
        dst_arr[s_core, s_tile, p, b] = s_dstloc.astype(np.float16)
        lo_m = s_lo
        idxlo[s_core[lo_m], s_tile[lo_m], pos_in_grp[lo_m]] = s_srow[lo_m].astype(
            np.int16
        )
        hi_m = ~s_lo
        idxhi[s_core[hi_m], s_tile[hi_m], pos_in_grp[hi_m]] = (
            s_srow[hi_m] - self.LOCUT
        ).astype(np.int16)

        # pad gather idx: everything stays -1 after the real edges (trailing skip).
        # counts per tile (rounded: the dma consumes them via num_idxs_reg)
        cnts = np.stack([n_lo, n_hi], axis=-1).astype(np.int32)  # [CORES, NT, 2]

        # wrap idx arrays to the [16, n/16] layout: position i -> [i % 16, i // 16]
        def wrap16(a):  # [..., S] -> [..., 16, S//16]
            S = a.shape[-1]
            return np.ascontiguousarray(
                a.reshape(*a.shape[:-1], S // 16, 16).swapaxes(-1, -2)
            )

        GRP = 2 if NT % 2 == 0 else 1
        self.GRP, self.NGRP = GRP, NT // GRP
        self.d_arr = d_arr
        self.dst_arr = dst_arr
        self.idxlo_w = wrap16(idxlo)
        self.idxhi_w = wrap16(idxhi)
        self.idxlo_g = wrap16(idxlo.reshape(CORES, self.NGRP, GRP * LO_SLOTS))
        self.idxhi_g = wrap16(idxhi.reshape(CORES, self.NGRP, GRP * HI_SLOTS))
        self.cnts = cnts

        # ---- node-side per-core tables
        # x_ids slab: [CORES, SLAB] (pad -> 0)
        xids_slab = np.zeros((CORES, SLAB), np.int64)
        gid_slab = np.full((CORES, SLAB), -1.0, np.float16)  # local graph id
        for k in range(CORES):
            n = own[k]
            xids_slab[k, :n] = x_ids[cstart[k]:cend[k]]
            gid_slab[k, :n] = (batch[cstart[k]:cend[k]] - k * gpc).astype(np.float16)
        self.xids_w = wrap16(xids_slab.astype(np.int16))  # emb table < 32768 rows
        self.gid_slab = gid_slab.reshape(CORES, NT, 128)

        # inverse counts per graph (local)
        cnt_g = np.zeros((CORES, gpc), np.float32)
        for k in range(CORES):
            ids, c = np.unique(
                (batch[cstart[k]:cend[k]] - k * gpc), return_counts=True
            )
            cnt_g[k, ids] = c
        self.inv_cnt = (1.0 / np.maximum(cnt_g, 1.0)).astype(np.float32)  # [CORES,gpc]

        # ---- weights (augmented)
        w = weights
        self.W1b = np.concatenate(
            [w["blk_W1"], w["blk_b1"][:, None, :]], axis=1
        ).astype(np.float32)  # [NCONV, F+1, F]
        self.W2b = np.concatenate(
            [w["blk_W2"], w["blk_b2"][:, None, :]], axis=1
        ).astype(np.float32)
        self.Web = np.concatenate(
            [w["blk_We"], w["blk_be"][:, None, :]], axis=1
        ).astype(np.float16)  # [NCONV, NRBF+1, F]
        self.emb = np.asarray(w["emb"]).astype(np.float32)
        for nm in ("Ws", "bs", "Wbg1", "bbg1", "Wbg2", "bbg2",
                   "Weh1", "beh1", "Weh2", "beh2"):
            setattr(self, nm, np.asarray(w[nm]).astype(np.float32))

        # RBF offsets
        offs = np.linspace(0.0, 6.0, NRBF).astype(np.float32)
        self.offs = offs
        self.coeff = np.float32(-0.5 / (offs[1] - offs[0]) ** 2)



"""Bass/Tile kernel builder for the CrystalGNN (SchNet-style) message-passing net.

Data layout (per core, SPMD identical program):
  - nodes sharded by graph: core k owns graphs [k*gpc,(k+1)*gpc) -> a slab of
    SLAB node rows (NT = SLAB/128 tiles of 128 nodes).
  - x state [128, NT*F] f32 lives in SBUF for the whole kernel.
  - h table (x @ W1 + b1, fp16, padded to 128 cols) is written per-slab to DRAM
    and AllGather'd so every core can dma_gather rows of any node.
  - edges partitioned by owner(dst), grouped by dst tile; per tile a fixed
    budget of BK*128 edge slots (LO_BLKS lo-src + HI_BLKS hi-src blocks,
    src slab-row < / >= LOCUT for int16 gather indices).
  - per tile: gather h[src]; RBF e from distances; f = e_aug @ Web (PE);
    m = h*f (DVE, fp16); one-hot S from dst ids (DVE is_equal);
    aggT[64,128] += m.T @ S (PE, PSUM f32); x += aggT.T@W2+b2; softplus.
  - pool: pooledT[F,gpc] += x_tile.T @ onehot(graph); * 1/cnt; 3-layer MLP.
"""
import numpy as np
from contextlib import ExitStack

import concourse.bass as bass
import concourse.bacc as bacc
import concourse.mybir as mybir
from concourse import tile

F = 64
NRBF = 10
NCONV = 3
CORES = 8
AF = mybir.ActivationFunctionType
OP = mybir.AluOpType
DT = mybir.dt


# ln(1+t) ~= t*(P0 + P1 t + P2 t^2 + P3 t^3 + P4 t^4), t in [0,1]
# (max abs err 8.1e-5; exact 0 at t=0). Lets softplus avoid the Ln
# activation so a single act-func table serves the whole program.
LN1P_C = (0.99988793, -0.49636828, 0.30467236, -0.15602843, 0.04106451)


def build_bass(prep, unroll=10, debug=False, stop_after=None, py_loops=False,
               repeat=1, ag_strided=True, mixed_tt=True, chunk_ag=False):
    """Returns (nc, input_names) — the SPMD program for all cores."""
    NT, BK = prep.NT, prep.BK
    LO_BLKS, HI_BLKS = prep.LO_BLKS, prep.HI_BLKS
    LO_SLOTS, HI_SLOTS = LO_BLKS * 128, HI_BLKS * 128
    SLAB, gpc, LOCUT = prep.SLAB, prep.gpc, prep.LOCUT
    GRP, NGRP = prep.GRP, prep.NGRP
    CS = CORES * SLAB
    coeff = float(prep.coeff)
    R = NRBF + 1
    WL = GRP * LO_SLOTS // 16
    WH = GRP * HI_SLOTS // 16

    nc = bacc.Bacc("TRN2", target_bir_lowering=False, debug=False,
                   num_devices=CORES)

    # ---------------- DRAM inputs ----------------
    def din(name, shape, dt):
        return nc.dram_tensor(name, list(shape), dt, kind="ExternalInput").ap()

    d_dram = din("d_edges", (NT * 128, BK), DT.float32)
    dst_dram = din("dst_edges", (NT * 128, BK), DT.float16)
    idxlo_dram = din("idxlo", (NGRP * 128, GRP * LO_SLOTS // 16), DT.int16)
    idxhi_dram = din("idxhi", (NGRP * 128, GRP * HI_SLOTS // 16), DT.int16)
    gid_dram = din("gid", (NT * 128, 1), DT.float16)
    xids_dram = din("xids", (128, SLAB // 16), DT.int16)
    emb_dram = din("emb95", (95, F), DT.float32)
    w1b_dram = din("w1b", (F + 1, NCONV * F), DT.float32)
    w2b_dram = din("w2b", (F + 1, NCONV * F), DT.float32)
    # block-diagonal Web for grouped f-matmuls: groups of FG=8 chunks (and a
    # tail group of BK%8 chunks). webbd8[(c,k),(c',j)] = Web[k,j] * (c==c')
    FG = 8
    TFG = BK % FG if BK % FG else FG  # tail group size
    webbd8_dram = din("webbd8", (FG * (NRBF + 1), NCONV * FG * F), DT.float16)
    webbdt_dram = din("webbdt", (TFG * (NRBF + 1), NCONV * TFG * F),
                      DT.float16)
    offs_dram = din("offs", (128, NRBF), DT.float32)
    iota_dram = din("iota128", (128, 128), DT.float16)
    iotag_dram = din("iotag", (128, gpc), DT.float32)
    ident16_dram = din("ident16", (128, 128), DT.float16)
    ident32_dram = din("ident32", (128, 128), DT.float32)
    invc_dram = din("invc", (F, gpc), DT.float32)
    ws_dram = din("ws", (F, 2 * F), DT.float32)
    bs_dram = din("bs", (2 * F, 1), DT.float32)
    wbg1_dram = din("wbg1", (2 * F, F), DT.float32)
    bbg1_dram = din("bbg1", (F, 1), DT.float32)
    wbg2_dram = din("wbg2", (F, 1), DT.float32)
    bbg2_dram = din("bbg2", (1, 1), DT.float32)
    weh1_dram = din("weh1", (2 * F, F), DT.float32)
    beh1_dram = din("beh1", (F, 1), DT.float32)
    weh2_dram = din("weh2", (F, 1), DT.float32)
    beh2_dram = din("beh2", (1, 1), DT.float32)

    dbg = {}
    if debug:
        dbg["x0"] = nc.dram_tensor("dbg_x0", [128, NT * F], DT.float32,
                                   kind="ExternalOutput").ap()
        dbg["hall0"] = nc.dram_tensor("dbg_hall0", [CS, 128], DT.float16,
                                      kind="ExternalOutput").ap()
        for i in range(NCONV):
            dbg[f"x{i+1}"] = nc.dram_tensor(f"dbg_x{i+1}", [128, NT * F],
                                            DT.float32,
                                            kind="ExternalOutput").ap()
        dbg["poolT"] = nc.dram_tensor("dbg_poolT", [F, gpc], DT.float32,
                                      kind="ExternalOutput").ap()
        dbg["cT"] = nc.dram_tensor("dbg_cT", [2 * F, gpc], DT.float32,
                                   kind="ExternalOutput").ap()
    h_all_t = nc.dram_tensor("h_all", [CS, 128], DT.float16,
                             addr_space="Shared")
    h_all2_t = nc.dram_tensor("h_all2", [CS, 128], DT.float16,
                              addr_space="Shared")
    h_bufs = (h_all_t, h_all2_t)
    # contiguous 64-col staging for the collective (half the AG bytes);
    # expanded locally into the 256B-row gather tables above
    h_a64_t = nc.dram_tensor("h_a64", [CS, F], DT.float16,
                             addr_space="Shared")
    obg_dram = nc.dram_tensor("obg", [gpc, 1], DT.float32,
                              kind="ExternalOutput").ap()
    oeh_dram = nc.dram_tensor("oeh", [gpc, 1], DT.float32,
                              kind="ExternalOutput").ap()

    with tile.TileContext(nc) as tc, ExitStack() as stk:
        cpool = stk.enter_context(tc.tile_pool(name="const", bufs=1))
        dpool = stk.enter_context(tc.tile_pool(name="dram", bufs=1,
                                               space="DRAM"))
        wk = stk.enter_context(tc.tile_pool(name="wk", bufs=4))
        wk2 = stk.enter_context(tc.tile_pool(name="wk2", bufs=2))
        conv_stk = ExitStack()
        gp = conv_stk.enter_context(tc.tile_pool(name="gp", bufs=3))
        pp = conv_stk.enter_context(tc.tile_pool(name="pp", bufs=2,
                                                 space="PSUM"))
        ppf = conv_stk.enter_context(tc.tile_pool(name="ppf", bufs=2,
                                                  space="PSUM"))
        ppx = conv_stk.enter_context(tc.tile_pool(name="ppx", bufs=1,
                                                  space="PSUM"))

        h_own64 = dpool.tile([SLAB, F], DT.float16)

        # ---------------- persistent SBUF ----------------
        def load_const(name, ap_dram, shape, dt):
            t = cpool.tile(list(shape), dt, tag=name)
            nc.sync.dma_start(out=t[:], in_=ap_dram)
            return t

        w1b_sb = load_const("w1b", w1b_dram, (F + 1, NCONV * F), DT.float32)
        w2b_sb = load_const("w2b", w2b_dram, (F + 1, NCONV * F), DT.float32)
        webbd8_sb = load_const("webbd8", webbd8_dram,
                               (FG * (NRBF + 1), NCONV * FG * F), DT.float16)
        webbdt_sb = load_const("webbdt", webbdt_dram,
                               (TFG * (NRBF + 1), NCONV * TFG * F), DT.float16)
        offs_sb = load_const("offs", offs_dram, (128, NRBF), DT.float32)
        iota_sb = load_const("iota", iota_dram, (128, 128), DT.float16)
        iotag_sb = load_const("iotag", iotag_dram, (128, gpc), DT.float32)
        id16_sb = load_const("id16", ident16_dram, (128, 128), DT.float16)
        id32_sb = load_const("id32", ident32_dram, (128, 128), DT.float32)
        invc_sb = load_const("invc", invc_dram, (F, gpc), DT.float32)
        ws_sb = load_const("ws", ws_dram, (F, 2 * F), DT.float32)
        bs_sb = load_const("bs", bs_dram, (2 * F, 1), DT.float32)
        wbg1_sb = load_const("wbg1", wbg1_dram, (2 * F, F), DT.float32)
        bbg1_sb = load_const("bbg1", bbg1_dram, (F, 1), DT.float32)
        wbg2_sb = load_const("wbg2", wbg2_dram, (F, 1), DT.float32)
        bbg2_sb = load_const("bbg2", bbg2_dram, (1, 1), DT.float32)
        weh1_sb = load_const("weh1", weh1_dram, (2 * F, F), DT.float32)
        beh1_sb = load_const("beh1", beh1_dram, (F, 1), DT.float32)
        weh2_sb = load_const("weh2", weh2_dram, (F, 1), DT.float32)
        beh2_sb = load_const("beh2", beh2_dram, (1, 1), DT.float32)
        xids_sb = load_const("xids", xids_dram, (128, SLAB // 16), DT.int16)

        x_sb = cpool.tile([128, NT * F], DT.float32, tag="x")

        # persistent static edge-side state (loaded/computed once, reused
        # across all convs)
        dst16_sb = cpool.tile([128, NT * BK], DT.float16, tag="dst16")
        nc.sync.dma_start(
            out=dst16_sb[:].rearrange("p (t b) -> p t b", b=BK),
            in_=dst_dram.rearrange("(t p) b -> p t b", p=128),
        )
        ixlo_sb = cpool.tile([128, NGRP * WL], DT.int16, tag="ixlo")
        nc.sync.dma_start(
            out=ixlo_sb[:].rearrange("p (g w) -> p g w", w=WL),
            in_=idxlo_dram.rearrange("(g p) w -> p g w", p=128),
        )
        ixhi_sb = cpool.tile([128, NGRP * WH], DT.int16, tag="ixhi")
        nc.sync.dma_start(
            out=ixhi_sb[:].rearrange("p (g w) -> p g w", w=WH),
            in_=idxhi_dram.rearrange("(g p) w -> p g w", p=128),
        )
        e16_sb = cpool.tile([128, NT * BK * R], DT.float16, tag="e16")
        hbuf = cpool.tile([128, NT * F], DT.float16, tag="hbuf")

        # ---------------- x0 = emb[x_ids] ----------------
        nc.gpsimd.dma_gather(
            x_sb[:].rearrange("p (b e) -> p b e", e=F),
            emb_dram,
            xids_sb[:],
            SLAB,
            SLAB,
            F,
            single_packet=False,
        )

        # ---------------- helpers ----------------
        def h_chain(iv, i):
            """hbuf[:, tile] = fp16(x_tile @ W1[i] + b1[i])."""
            xcp = wk2.tile([128, F], DT.float32, tag="xcp")
            nc.scalar.copy(xcp[:], x_sb[:, bass.ts(iv, F)])
            xT_ps = ppx.tile([F, 128], DT.float32, tag="xps")
            nc.tensor.transpose(xT_ps[:], xcp[:], id32_sb[:])
            xT_sb = wk2.tile([F + 1, 128], DT.float32, tag="xT")
            nc.scalar.copy(xT_sb[0:F, :], xT_ps[:])
            nc.vector.memset(xT_sb[F:F + 1, :], 1.0)
            h_ps = ppx.tile([128, F], DT.float32, tag="xps2")
            nc.tensor.matmul(h_ps[:], xT_sb[:], w1b_sb[:, i * F:(i + 1) * F],
                             start=True, stop=True)
            nc.scalar.copy(hbuf[:, bass.ts(iv, F)], h_ps[:])

        # h-chunk boundaries (in tiles) for the overlapped AllGather: each
        # chunk is flushed + allgathered as soon as its h tiles are done, so
        # the collective overlaps the remaining edge compute of the conv.
        AGC = [0, NT]  # single-shot AG (collectives need contiguous APs)

        def ag_chunk(dst_t, t0, t1):
            """Flush hbuf, AllGather the contiguous 64-col staging (half
            the bytes of the padded table), then locally expand into the
            256B-row gather table dst_t (pad cols stay garbage, never
            read)."""
            assert (t0, t1) == (0, NT)
            nc.sync.dma_start(
                out=h_own64[:].rearrange("(t p) c -> p t c", p=128),
                in_=hbuf[:].rearrange("p (t c) -> p t c", c=F),
            )
            nc.gpsimd.collective_compute(
                "AllGather",
                OP.bypass,
                replica_groups=[list(range(CORES))],
                ins=[h_own64[:].opt()],
                outs=[h_a64_t[:].opt()],
            )
            nc.sync.dma_start(
                out=dst_t[:][:, 0:F].rearrange("(r p) c -> p r c", p=128),
                in_=h_a64_t[:].rearrange("(r p) c -> p r c", p=128),
            )

        def gather_group(gv, hs_lo, hs_hi, h_src):
            nc.gpsimd.dma_gather(
                hs_lo[:].rearrange("p (b e) -> p b e", e=128),
                h_src[:], ixlo_sb[:, bass.ts(gv, WL)],
                GRP * LO_SLOTS, GRP * LO_SLOTS, 128, single_packet=False,
            )
            nc.gpsimd.dma_gather(
                hs_hi[:].rearrange("p (b e) -> p b e", e=128),
                h_src[:][LOCUT:CS, :], ixhi_sb[:, bass.ts(gv, WH)],
                GRP * HI_SLOTS, GRP * HI_SLOTS, 128, single_packet=False,
            )

        def edge_phase(iv, i, tg, hs_lo, hs_hi):
            """Returns aggT psum tile [F, 128] accumulated over the tile."""
            hsl3 = hs_lo[:].rearrange("p (b e) -> p b e", e=128)
            hsh3 = hs_hi[:].rearrange("p (b e) -> p b e", e=128)
            tile_e16 = e16_sb[:, bass.ts(iv, BK * R)]
            # local static-offset copy of this tile's dst ids (cheap; lets
            # the broadcast APs below use raw strides)
            dst_sb = wk.tile([128, BK], DT.float16, tag="dst")
            nc.gpsimd.tensor_copy(dst_sb[:], dst16_sb[:, bass.ts(iv, BK)])

            aggT_ps = pp.tile([F, 128], DT.float32, tag="aggT")

            # f / m / S / scatter in groups of FG chunks; per group one
            # transpose of e16 cols -> eT [fn*11, 128], one block-diag matmul
            for g0 in range(0, BK, FG):
                fn = min(FG, BK - g0)
                eT_ps = ppf.tile([FG * R, 128], DT.float16, tag="eTp")
                nc.tensor.transpose(
                    eT_ps[0:fn * R, :],
                    tile_e16[:, g0 * R:(g0 + fn) * R],
                    id16_sb[:],
                )
                eT_sb = wk.tile([FG * R, 128], DT.float16, tag="eTs")
                nc.scalar.copy(eT_sb[0:fn * R, :], eT_ps[0:fn * R, :])
                f_ps = ppf.tile([128, FG * F], DT.float32, tag="fps")
                bd = webbd8_sb if fn == FG else webbdt_sb
                nc.tensor.matmul(
                    f_ps[:, 0:fn * F],
                    eT_sb[0:fn * R, :],
                    bd[:, i * fn * F:(i + 1) * fn * F],
                    start=True, stop=True,
                )
                if mixed_tt:
                    f3 = f_ps[:].rearrange("p (b e) -> p b e", e=F)
                else:
                    f16 = wk.tile([128, FG * F], DT.float16, tag="f16")
                    nc.scalar.copy(f16[0:128, 0:fn * F], f_ps[:, 0:fn * F])
                    f3 = f16[:].rearrange("p (b e) -> p b e", e=F)
                m_sb = wk.tile([128, FG * F], DT.float16, tag="m")
                segs = []
                c0, c1 = g0, g0 + fn
                if c0 < LO_BLKS:
                    segs.append((hsl3, tg * LO_BLKS + c0, c0,
                                 min(c1, LO_BLKS) - c0))
                if c1 > LO_BLKS:
                    cc0 = max(c0, LO_BLKS)
                    segs.append((hsh3, tg * HI_BLKS + (cc0 - LO_BLKS),
                                 cc0, c1 - cc0))
                for (src3, b0, coff, n) in segs:
                    nc.vector.tensor_tensor(
                        m_sb[:].rearrange("p (b e) -> p b e", e=F)[
                            :, coff - g0:coff - g0 + n, :],
                        src3[:, b0:b0 + n, 0:F],
                        f3[:, coff - g0:coff - g0 + n, :],
                        OP.mult,
                    )
                S_sb = wk.tile([128, FG * 128], DT.float16, tag="S")
                dst_b = bass.AP(
                    dst_sb.tensor,
                    dst_sb[:, g0:g0 + fn].offset,
                    [dst_sb[:].ap[0], [1, fn], [0, 128]],
                )
                iota_b = bass.AP(
                    iota_sb.tensor, iota_sb[:].offset,
                    [iota_sb[:].ap[0], [0, fn], [1, 128]],
                )
                nc.vector.tensor_tensor(
                    S_sb[:].rearrange("p (b e) -> p b e", e=128)[:, 0:fn, :],
                    dst_b, iota_b, OP.is_equal,
                )
                for c in range(fn):
                    cg = g0 + c
                    nc.tensor.matmul(
                        aggT_ps[:],
                        m_sb[:, c * F:(c + 1) * F],
                        S_sb[:, c * 128:(c + 1) * 128],
                        start=(cg == 0), stop=(cg == BK - 1),
                    )
            return aggT_ps

        def x_accum(iv, i, tg, aggT_ps, xs_g):
            """xs_g[:, tg*F:] = x + agg @ W2 + b2 (pre-softplus)."""
            aggT_sb = wk2.tile([F + 1, 128], DT.float32, tag="aggTs")
            nc.scalar.copy(aggT_sb[0:F, :], aggT_ps[:])
            nc.vector.memset(aggT_sb[F:F + 1, :], 1.0)
            xup_ps = ppx.tile([128, F], DT.float32, tag="xps2")
            nc.tensor.matmul(xup_ps[:], aggT_sb[:],
                             w2b_sb[:, i * F:(i + 1) * F],
                             start=True, stop=True)
            nc.vector.tensor_tensor(xs_g[:, tg * F:(tg + 1) * F], xup_ps[:],
                                    x_sb[:, bass.ts(iv, F)], OP.add)

        def softplus_group(gv, xs_g):
            """x_sb[group] = relu(xs) + ln1p(exp(-|xs|)), poly ln1p on Pool.

            Uses only Abs/Exp activations so one act-func table set serves
            the whole program (no per-tile table reloads).
            """
            n = GRP * F
            ab = wk2.tile([128, n], DT.float32, tag="ab")
            nc.scalar.activation(ab[:], xs_g[:], AF.Abs)
            nc.scalar.activation(ab[:], ab[:], AF.Exp, scale=-1.0)
            acc = wk2.tile([128, n], DT.float32, tag="acc")
            nc.vector.tensor_scalar(acc[:], ab[:], LN1P_C[4], LN1P_C[3],
                                    OP.mult, OP.add)
            for ck in (LN1P_C[2], LN1P_C[1], LN1P_C[0]):
                nc.vector.tensor_tensor(acc[:], acc[:], ab[:], OP.mult)
                nc.vector.tensor_scalar_add(acc[:], acc[:], ck)
            nc.vector.tensor_tensor(acc[:], acc[:], ab[:], OP.mult)
            nc.vector.scalar_tensor_tensor(
                x_sb[:, bass.ts(gv, GRP * F)], xs_g[:], 0.0, acc[:],
                OP.max, OP.add)

        # ---------------- prologue: e16 (RBF) + h0 ----------------
        nc.vector.memset(e16_sb[:], 1.0)  # aug ones cols; exp fills the rest

        def body_pro(iv):
            d_sb = wk.tile([128, BK], DT.float32, tag="d")
            nc.sync.dma_start(
                out=d_sb[:],
                in_=d_dram.rearrange("(t p) b -> t p b", p=128)[
                    bass.ds(iv, 1)][0],
            )
            e32 = wk.tile([128, BK * NRBF], DT.float32, tag="e32")
            d_b = bass.AP(d_sb.tensor, d_sb[:].offset,
                          [d_sb[:].ap[0], d_sb[:].ap[1], [0, NRBF]])
            offs_b = bass.AP(offs_sb.tensor, offs_sb[:].offset,
                             [offs_sb[:].ap[0], [0, BK], offs_sb[:].ap[1]])
            e32_3 = e32[:].rearrange("p (b r) -> p b r", r=NRBF)
            nc.vector.tensor_tensor(e32_3, d_b, offs_b, OP.subtract)
            nc.vector.tensor_tensor(e32[:], e32[:], e32[:], OP.mult)
            e16_t = e16_sb[:, bass.ts(iv, BK * R)].rearrange(
                "p (b r) -> p b r", r=R)
            nc.scalar.activation(e16_t[:, :, 0:NRBF], e32_3, AF.Exp,
                                 scale=coeff)
            h_chain(iv, 0)

        # prologue tile loop with chunked h0 allgathers into h_bufs[0]
        ag_i = 0
        for t in range(NT):
            body_pro(t)
            while ag_i + 1 < len(AGC) and t + 1 == AGC[ag_i + 1]:
                ag_chunk(h_bufs[0], AGC[ag_i], AGC[ag_i + 1])
                ag_i += 1
        stopped = stop_after in ("h0", "ag0")
        if debug:
            nc.sync.dma_start(out=dbg["x0"], in_=x_sb[:])
            nc.sync.dma_start(out=dbg["hall0"], in_=h_all_t[:])

        conv_c = 0  # running conv index for h_all ping-pong
        for rep in range(repeat):
            if stopped:
                break
            for i in range(NCONV):
                if stopped or (stop_after is not None
                               and stop_after.startswith("conv")
                               and i > int(stop_after[4:])):
                    stopped = True
                    break
                last = (rep == repeat - 1) and (i == NCONV - 1)
                rbuf = h_bufs[conv_c % 2]
                wbuf = h_bufs[(conv_c + 1) % 2]

                def body_conv_group(gv, i=i, last=last, rbuf=rbuf):
                    hs_lo = gp.tile([128, GRP * LO_SLOTS], DT.float16,
                                    tag="hslo")
                    hs_hi = gp.tile([128, GRP * HI_SLOTS], DT.float16,
                                    tag="hshi")
                    gather_group(gv, hs_lo, hs_hi, rbuf)
                    xs_g = gp.tile([128, GRP * F], DT.float32, tag="xsg")
                    for tg in range(GRP):
                        iv = gv * GRP + tg
                        aggT_ps = edge_phase(iv, i, tg, hs_lo, hs_hi)
                        x_accum(iv, i, tg, aggT_ps, xs_g)
                    softplus_group(gv, xs_g)
                    if not last:
                        for tg in range(GRP):
                            h_chain(gv * GRP + tg, (i + 1) % NCONV)

                ag_i = 0
                for g in range(NGRP):
                    body_conv_group(g)
                    if not last:
                        # issue each h chunk's allgather as soon as its
                        # tiles' h_chain is done (overlaps edge compute)
                        while (ag_i + 1 < len(AGC)
                               and (g + 1) * GRP >= AGC[ag_i + 1]):
                            ag_chunk(wbuf, AGC[ag_i], AGC[ag_i + 1])
                            ag_i += 1
                if stop_after == f"conv{i}" and not last:
                    stopped = True
                    break
                if debug and rep == repeat - 1:
                    nc.sync.dma_start(out=dbg[f"x{i+1}"], in_=x_sb[:])
                conv_c += 1

        # ---------------- pooling ----------------
        if stopped:
            zz = wk.tile([1, gpc], DT.float32, tag="zz")
            nc.vector.memset(zz[:], 0.0)
            nc.sync.dma_start(out=obg_dram.rearrange("g one -> one g")[0:1, :],
                              in_=zz[:])
            nc.sync.dma_start(out=oeh_dram.rearrange("g one -> one g")[0:1, :],
                              in_=zz[:])
            conv_stk.close()
            do_rest = False
        else:
            conv_stk.close()
            do_rest = True
        if do_rest:
            ppm = stk.enter_context(tc.tile_pool(name="ppm", bufs=1,
                                                 space="PSUM"))
            poolT_ps = ppm.tile([F, gpc], DT.float32, tag="poolT")

            def body_pool(iv):
                gid_sb = wk.tile([128, 1], DT.float16, tag="gid")
                nc.sync.dma_start(
                    out=gid_sb[:],
                    in_=gid_dram.rearrange("(t p) b -> t p b", p=128)[
                        bass.ds(iv, 1)][0],
                )
                gid32 = wk.tile([128, 1], DT.float32, tag="gid32")
                nc.scalar.copy(gid32[:], gid_sb[:])
                Sp = wk.tile([128, gpc], DT.float32, tag="Sp")
                gid_b = bass.AP(gid32.tensor, gid32[:].offset,
                                [gid32[:].ap[0], [0, gpc]])
                nc.vector.tensor_tensor(Sp[:], gid_b, iotag_sb[:], OP.is_equal)
                nc.tensor.matmul(poolT_ps[:], x_sb[:, iv * F:(iv + 1) * F],
                                 Sp[:], start=(iv == 0), stop=(iv == NT - 1))

            for t in range(NT):
                body_pool(t)

            # mean + MLP (feature-major: cT = relu(Ws.T @ pooled + bs))
            poolT_sb = wk.tile([F, gpc], DT.float32, tag="poolTs")
            nc.vector.tensor_tensor(poolT_sb[:], poolT_ps[:], invc_sb[:], OP.mult)
            cT_ps = ppm.tile([2 * F, gpc], DT.float32, tag="cT")
            nc.tensor.matmul(cT_ps[:], ws_sb[:], poolT_sb[:], start=True,
                             stop=True)
            cT_sb = wk.tile([2 * F, gpc], DT.float32, tag="cTs")
            nc.scalar.activation(cT_sb[:], cT_ps[:], AF.Relu, bias=bs_sb[:])
            if debug:
                nc.sync.dma_start(out=dbg["poolT"], in_=poolT_sb[:])
                nc.sync.dma_start(out=dbg["cT"], in_=cT_sb[:])

            for (w1s, b1s, w2s, b2s, out_dram, tg) in (
                (wbg1_sb, bbg1_sb, wbg2_sb, bbg2_sb, obg_dram, "bg"),
                (weh1_sb, beh1_sb, weh2_sb, beh2_sb, oeh_dram, "eh"),
            ):
                t1_ps = ppm.tile([F, gpc], DT.float32, tag="t1")
                nc.tensor.matmul(t1_ps[:], w1s[:], cT_sb[:], start=True, stop=True)
                t1_sb = wk.tile([F, gpc], DT.float32, tag="t1s" + tg)
                nc.scalar.activation(t1_sb[:], t1_ps[:], AF.Relu, bias=b1s[:])
                o_ps = ppm.tile([1, gpc], DT.float32, tag="o")
                nc.tensor.matmul(o_ps[:], w2s[:], t1_sb[:], start=True, stop=True)
                o_sb = wk.tile([1, gpc], DT.float32, tag="os" + tg)
                nc.scalar.activation(o_sb[:], o_ps[:], AF.Identity, bias=b2s[:])
                nc.sync.dma_start(out=out_dram.rearrange("g one -> one g")[0:1, :],
                                  in_=o_sb[:])

    nc.compile()
    return nc


def make_in_maps(prep):
    """Per-core input dicts for run_bass_kernel_spmd."""
    NT, BK, SLAB, gpc = prep.NT, prep.BK, prep.SLAB, prep.gpc
    LO_SLOTS, HI_SLOTS = prep.LO_BLKS * 128, prep.HI_BLKS * 128
    maps = []
    iota128 = np.tile(np.arange(128, dtype=np.float16)[None, :], (128, 1))
    iotag = np.tile(np.arange(gpc, dtype=np.float32)[None, :], (128, 1))
    id16 = np.eye(128, dtype=np.float16)
    id32 = np.eye(128, dtype=np.float32)
    offs = np.tile(prep.offs[None, :], (128, 1)).astype(np.float32)

    def rep16(a):  # [16, S] -> [128, S] replicated
        return np.tile(a, (8, 1))

    R = NRBF + 1
    FG = 8
    TFG = BK % FG if BK % FG else FG

    def make_bd(fn):
        bd = np.zeros((fn * R, NCONV * fn * F), np.float16)
        for i in range(NCONV):
            for c in range(fn):
                bd[c * R:(c + 1) * R, i * fn * F + c * F:i * fn * F +
                   (c + 1) * F] = prep.Web[i]
        return bd

    webbd8 = make_bd(FG)
    webbdt = make_bd(TFG)

    for k in range(CORES):
        m = dict(
            d_edges=prep.d_arr[k].reshape(NT * 128, BK),
            dst_edges=prep.dst_arr[k].reshape(NT * 128, BK),
            idxlo=np.ascontiguousarray(
                np.tile(prep.idxlo_g[k], (1, 8, 1)).reshape(
                    prep.NGRP * 128, prep.GRP * LO_SLOTS // 16)),
            idxhi=np.ascontiguousarray(
                np.tile(prep.idxhi_g[k], (1, 8, 1)).reshape(
                    prep.NGRP * 128, prep.GRP * HI_SLOTS // 16)),
            gid=prep.gid_slab[k].reshape(NT * 128, 1).astype(np.float16),
            xids=rep16(prep.xids_w[k]),
            emb95=prep.emb,
            w1b=np.ascontiguousarray(
                prep.W1b.transpose(1, 0, 2).reshape(F + 1, NCONV * F)),
            w2b=np.ascontiguousarray(
                prep.W2b.transpose(1, 0, 2).reshape(F + 1, NCONV * F)),
            webbd8=webbd8,
            webbdt=webbdt,
            offs=offs,
            iota128=iota128,
            iotag=iotag,
            ident16=id16,
            ident32=id32,
            invc=np.tile(prep.inv_cnt[k][None, :], (F, 1)).astype(np.float32),
            ws=prep.Ws,
            bs=prep.bs.reshape(2 * F, 1),
            wbg1=prep.Wbg1,
            bbg1=prep.bbg1.reshape(F, 1),
            wbg2=prep.Wbg2,
            bbg2=prep.bbg2.reshape(1, 1),
            weh1=prep.Weh1,
            beh1=prep.beh1.reshape(F, 1),
            weh2=prep.Weh2,
            beh2=prep.beh2.reshape(1, 1),
        )
        maps.append({k2: np.ascontiguousarray(v) for k2, v in m.items()})
    return maps


def kernel(**inputs):
    import numpy as np
    from concourse.bass_utils import run_bass_kernel_spmd

    wkeys = ("emb blk_W1 blk_b1 blk_We blk_be blk_W2 blk_b2 Ws bs Wbg1 bbg1 "
             "Wbg2 bbg2 Weh1 beh1 Weh2 beh2").split()
    weights = {k: np.asarray(inputs[k]) for k in wkeys}
    p = Prep(np.asarray(inputs["x_ids"]), np.asarray(inputs["edge_index"]),
             np.asarray(inputs["edge_attr"]), np.asarray(inputs["batch"]),
             weights, n_graphs=512)
    nc = build_bass(p, unroll=25)
    maps = make_in_maps(p)
    res = run_bass_kernel_spmd(nc, maps, list(range(CORES)))
    bg = np.concatenate([np.asarray(res.results[k]["obg"], dtype=np.float32)
                         for k in range(CORES)])
    eh = np.concatenate([np.asarray(res.results[k]["oeh"], dtype=np.float32)
                         for k in range(CORES)])
    return bg, eh



# revision 32
# speedup vs baseline: 1.2134x; 1.2134x over previous
"""CrystalGNN (SchNet-style) Trainium2 Bass kernel — self-contained.

Sharding: nodes/graphs block-partitioned across 8 NeuronCores (graph-aligned
slabs); edges partitioned by owner(dst) and grouped by 128-node dst tile;
small weights replicated. Per conv: h = x@W1+b1 (fp16, slab-local) ->
AllGather -> per-tile edge pipeline (dma_gather h[src], RBF+block-diag Web
matmul for f, m = h*f, one-hot scatter matmul into PSUM) -> x update +
softplus. Mean-pool via one-hot matmul + small MLP heads on-device.
"""
"""Host-side sharding/preprocessing + numpy device-model for the CrystalGNN kernel.

Everything is parameterized by the problem dims so the same code paths can be
exercised at a small scale in CoreSim and at full scale on hardware.
"""
import numpy as np

F = 64          # atom feats
NRBF = 10
NCONV = 3
H = 64
CORES = 8


def ceil_div(a, b):
    return (a + b - 1) // b


def round_up(a, b):
    return ceil_div(a, b) * b


class Prep:
    """Per-problem host preprocessing. All outputs are numpy arrays keyed for
    the bass kernel's DRAM tensors (one dict per core)."""

    def __init__(self, x_ids, edge_index, edge_attr, batch, weights, n_graphs,
                 locut=None):
        N = x_ids.shape[0]
        E = edge_index.shape[1]
        G = n_graphs
        assert G % CORES == 0
        gpc = G // CORES  # graphs per core
        self.N, self.E, self.G, self.gpc = N, E, G, gpc

        batch = np.asarray(batch).astype(np.int64)
        x_ids = np.asarray(x_ids).astype(np.int64)
        src = np.asarray(edge_index[0]).astype(np.int64)
        dst = np.asarray(edge_index[1]).astype(np.int64)
        d = np.asarray(edge_attr).astype(np.float32)

        # graph -> node range (batch is sorted)
        gstart = np.searchsorted(batch, np.arange(G), side="left")
        gend = np.searchsorted(batch, np.arange(G), side="right")
        # core k owns graphs [k*gpc, (k+1)*gpc) -> nodes [cstart[k], cend[k])
        cstart = gstart[np.arange(CORES) * gpc]
        cend = np.append(cstart[1:], N)
        own = cend - cstart
        max_own = int(own.max())
        # slab size: per-core node capacity, multiple of 128
        SLAB = round_up(max_own, 128)
        NT = SLAB // 128  # node tiles per core
        self.SLAB, self.NT = SLAB, NT
        self.cstart, self.cend = cstart, cend

        # slab row of each global node
        owner = np.searchsorted(cstart, np.arange(N), side="right") - 1
        srow = SLAB * owner + (np.arange(N) - cstart[owner])
        self.owner, self.srow = owner, srow

        # int16 split point for gather indices (slab rows)
        self.LOCUT = min(32768, CORES * SLAB) if locut is None else locut

        # ---- edge partition: edge belongs to owner[dst], tile = local dst block
        e_owner = owner[dst]
        e_tile = (dst - cstart[e_owner]) // 128
        e_dstloc = (dst - cstart[e_owner]) % 128           # local id within tile
        e_srow = srow[src]
        e_lo = e_srow < self.LOCUT

        # per (core, tile): count lo/hi edges
        # order edges by (core, tile, hi?, arbitrary)
        key = ((e_owner * NT + e_tile) * 2 + (~e_lo).astype(np.int64))
        order = np.argsort(key, kind="stable")
        s_core = e_owner[order]
        s_tile = e_tile[order]
        s_lo = e_lo[order]
        s_d = d[order]
        s_dstloc = e_dstloc[order]
        s_srow = e_srow[order]

        # counts
        n_lo = np.zeros((CORES, NT), np.int64)
        n_hi = np.zeros((CORES, NT), np.int64)
        np.add.at(n_lo, (e_owner[e_lo], e_tile[e_lo]), 1)
        np.add.at(n_hi, (e_owner[~e_lo], e_tile[~e_lo]), 1)
        LO_BLKS = int(ceil_div(n_lo.max(), 128))
        HI_BLKS = int(ceil_div(n_hi.max(), 128))
        BK = LO_BLKS + HI_BLKS
        self.LO_BLKS, self.HI_BLKS, self.BK = LO_BLKS, HI_BLKS, BK
        self.n_lo, self.n_hi = n_lo, n_hi

        LO_SLOTS = LO_BLKS * 128
        HI_SLOTS = HI_BLKS * 128
        SLOTS = BK * 128

        # ---- per-core packed arrays
        # slot s of (core,tile): s in [0, LO_SLOTS) lo edges; [LO_SLOTS, SLOTS) hi
        d_arr = np.zeros((CORES, NT, 128, BK), np.float32)
        dst_arr = np.full((CORES, NT, 128, BK), -1.0, np.float16)
        idxlo = np.zeros((CORES, NT, LO_SLOTS), np.int16)
        idxhi = np.zeros((CORES, NT, HI_SLOTS), np.int16)

        # fill using the sorted stream
        # positions within each (core,tile,lo/hi) group
        grp_key = (s_core * NT + s_tile) * 2 + (~s_lo).astype(np.int64)
        # index within group
        uniq, first_idx = np.unique(grp_key, return_index=True)
        pos_in_grp = np.arange(len(grp_key)) - np.repeat(
            first_idx, np.diff(np.append(first_idx, len(grp_key)))
        )
        slot = np.where(s_lo, pos_in_grp, LO_SLOTS + pos_in_grp)
        p = slot % 128
        b = slot // 128
        d_arr[s_core, s_tile, p, b] = s_d
        dst_arr[s_core, s_tile, p, b] = s_dstloc.astype(np.float16)
        lo_m = s_lo
        idxlo[s_core[lo_m], s_tile[lo_m], pos_in_grp[lo_m]] = s_srow[lo_m].astype(
            np.int16
        )
        hi_m = ~s_lo
        idxhi[s_core[hi_m], s_tile[hi_m], pos_in_grp[hi_m]] = (
            s_srow[hi_m] - self.LOCUT
        ).astype(np.int16)

        # pad gather idx: everything stays -1 after the real edges (trailing skip).
        # counts per tile (rounded: the dma consumes them via num_idxs_reg)
        cnts = np.stack([n_lo, n_hi], axis=-1).astype(np.int32)  # [CORES, NT, 2]

        # wrap idx arrays to the [16, n/16] layout: position i -> [i % 16, i // 16]
        def wrap16(a):  # [..., S] -> [..., 16, S//16]
            S = a.shape[-1]
            return np.ascontiguousarray(
                a.reshape(*a.shape[:-1], S // 16, 16).swapaxes(-1, -2)
            )

        GRP = 2 if NT % 2 == 0 else 1
        self.GRP, self.NGRP = GRP, NT // GRP
        self.d_arr = d_arr
        self.dst_arr = dst_arr
        self.idxlo_w = wrap16(idxlo)
        self.idxhi_w = wrap16(idxhi)
        self.idxlo_g = wrap16(idxlo.reshape(CORES, self.NGRP, GRP * LO_SLOTS))
        self.idxhi_g = wrap16(idxhi.reshape(CORES, self.NGRP, GRP * HI_SLOTS))
        self.cnts = cnts

        # ---- node-side per-core tables
        # x_ids slab: [CORES, SLAB] (pad -> 0)
        xids_slab = np.zeros((CORES, SLAB), np.int64)
        gid_slab = np.full((CORES, SLAB), -1.0, np.float16)  # local graph id
        for k in range(CORES):
            n = own[k]
            xids_slab[k, :n] = x_ids[cstart[k]:cend[k]]
            gid_slab[k, :n] = (batch[cstart[k]:cend[k]] - k * gpc).astype(np.float16)
        self.xids_w = wrap16(xids_slab.astype(np.int16))  # emb table < 32768 rows
        self.gid_slab = gid_slab.reshape(CORES, NT, 128)

        # inverse counts per graph (local)
        cnt_g = np.zeros((CORES, gpc), np.float32)
        for k in range(CORES):
            ids, c = np.unique(
                (batch[cstart[k]:cend[k]] - k * gpc), return_counts=True
            )
            cnt_g[k, ids] = c
        self.inv_cnt = (1.0 / np.maximum(cnt_g, 1.0)).astype(np.float32)  # [CORES,gpc]

        # ---- weights (augmented)
        w = weights
        self.W1b = np.concatenate(
            [w["blk_W1"], w["blk_b1"][:, None, :]], axis=1
        ).astype(np.float32)  # [NCONV, F+1, F]
        self.W2b = np.concatenate(
            [w["blk_W2"], w["blk_b2"][:, None, :]], axis=1
        ).astype(np.float32)
        self.Web = np.concatenate(
            [w["blk_We"], w["blk_be"][:, None, :]], axis=1
        ).astype(np.float16)  # [NCONV, NRBF+1, F]
        self.emb = np.asarray(w["emb"]).astype(np.float32)
        for nm in ("Ws", "bs", "Wbg1", "bbg1", "Wbg2", "bbg2",
                   "Weh1", "beh1", "Weh2", "beh2"):
            setattr(self, nm, np.asarray(w[nm]).astype(np.float32))

        # RBF offsets
        offs = np.linspace(0.0, 6.0, NRBF).astype(np.float32)
        self.offs = offs
        self.coeff = np.float32(-0.5 / (offs[1] - offs[0]) ** 2)



"""Bass/Tile kernel builder for the CrystalGNN (SchNet-style) message-passing net.

Data layout (per core, SPMD identical program):
  - nodes sharded by graph: core k owns graphs [k*gpc,(k+1)*gpc) -> a slab of
    SLAB node rows (NT = SLAB/128 tiles of 128 nodes).
  - x state [128, NT*F] f32 lives in SBUF for the whole kernel.
  - h table (x @ W1 + b1, fp16, padded to 128 cols) is written per-slab to DRAM
    and AllGather'd so every core can dma_gather rows of any node.
  - edges partitioned by owner(dst), grouped by dst tile; per tile a fixed
    budget of BK*128 edge slots (LO_BLKS lo-src + HI_BLKS hi-src blocks,
    src slab-row < / >= LOCUT for int16 gather indices).
  - per tile: gather h[src]; RBF e from distances; f = e_aug @ Web (PE);
    m = h*f (DVE, fp16); one-hot S from dst ids (DVE is_equal);
    aggT[64,128] += m.T @ S (PE, PSUM f32); x += aggT.T@W2+b2; softplus.
  - pool: pooledT[F,gpc] += x_tile.T @ onehot(graph); * 1/cnt; 3-layer MLP.
"""
import numpy as np
from contextlib import ExitStack

import concourse.bass as bass
import concourse.bacc as bacc
import concourse.mybir as mybir
from concourse import tile

F = 64
NRBF = 10
NCONV = 3
CORES = 8
AF = mybir.ActivationFunctionType
OP = mybir.AluOpType
DT = mybir.dt


# ln(1+t) ~= t*(P0 + P1 t + P2 t^2 + P3 t^3 + P4 t^4), t in [0,1]
# (max abs err 8.1e-5; exact 0 at t=0). Lets softplus avoid the Ln
# activation so a single act-func table serves the whole program.
LN1P_C = (0.99988793, -0.49636828, 0.30467236, -0.15602843, 0.04106451)


def build_bass(prep, unroll=10, debug=False, stop_after=None, py_loops=False,
               repeat=1, ag_strided=True, mixed_tt=True, chunk_ag=False):
    """Returns (nc, input_names) — the SPMD program for all cores."""
    NT, BK = prep.NT, prep.BK
    LO_BLKS, HI_BLKS = prep.LO_BLKS, prep.HI_BLKS
    LO_SLOTS, HI_SLOTS = LO_BLKS * 128, HI_BLKS * 128
    SLAB, gpc, LOCUT = prep.SLAB, prep.gpc, prep.LOCUT
    GRP, NGRP = prep.GRP, prep.NGRP
    CS = CORES * SLAB
    coeff = float(prep.coeff)
    R = NRBF + 1
    WL = GRP * LO_SLOTS // 16
    WH = GRP * HI_SLOTS // 16

    nc = bacc.Bacc("TRN2", target_bir_lowering=False, debug=False,
                   num_devices=CORES)

    # ---------------- DRAM inputs ----------------
    def din(name, shape, dt):
        return nc.dram_tensor(name, list(shape), dt, kind="ExternalInput").ap()

    d_dram = din("d_edges", (NT * 128, BK), DT.float32)
    dst_dram = din("dst_edges", (NT * 128, BK), DT.float16)
    idxlo_dram = din("idxlo", (NGRP * 128, GRP * LO_SLOTS // 16), DT.int16)
    idxhi_dram = din("idxhi", (NGRP * 128, GRP * HI_SLOTS // 16), DT.int16)
    gid_dram = din("gid", (NT * 128, 1), DT.float16)
    xids_dram = din("xids", (128, SLAB // 16), DT.int16)
    emb_dram = din("emb95", (95, F), DT.float32)
    w1b_dram = din("w1b", (F + 1, NCONV * F), DT.float32)
    w2b_dram = din("w2b", (F + 1, NCONV * F), DT.float32)
    # block-diagonal Web for grouped f-matmuls: groups of FG=8 chunks (and a
    # tail group of BK%8 chunks). webbd8[(c,k),(c',j)] = Web[k,j] * (c==c')
    FG = 8
    TFG = BK % FG if BK % FG else FG  # tail group size
    webbd8_dram = din("webbd8", (FG * (NRBF + 1), NCONV * FG * F), DT.float16)
    webbdt_dram = din("webbdt", (TFG * (NRBF + 1), NCONV * TFG * F),
                      DT.float16)
    offs_dram = din("offs", (128, NRBF), DT.float32)
    iota_dram = din("iota128", (128, 128), DT.float16)
    iotag_dram = din("iotag", (128, gpc), DT.float32)
    ident16_dram = din("ident16", (128, 128), DT.float16)
    ident32_dram = din("ident32", (128, 128), DT.float32)
    invc_dram = din("invc", (F, gpc), DT.float32)
    ws_dram = din("ws", (F, 2 * F), DT.float32)
    bs_dram = din("bs", (2 * F, 1), DT.float32)
    wbg1_dram = din("wbg1", (2 * F, F), DT.float32)
    bbg1_dram = din("bbg1", (F, 1), DT.float32)
    wbg2_dram = din("wbg2", (F, 1), DT.float32)
    bbg2_dram = din("bbg2", (1, 1), DT.float32)
    weh1_dram = din("weh1", (2 * F, F), DT.float32)
    beh1_dram = din("beh1", (F, 1), DT.float32)
    weh2_dram = din("weh2", (F, 1), DT.float32)
    beh2_dram = din("beh2", (1, 1), DT.float32)

    dbg = {}
    if debug:
        dbg["x0"] = nc.dram_tensor("dbg_x0", [128, NT * F], DT.float32,
                                   kind="ExternalOutput").ap()
        dbg["hall0"] = nc.dram_tensor("dbg_hall0", [CS, 128], DT.float16,
                                      kind="ExternalOutput").ap()
        for i in range(NCONV):
            dbg[f"x{i+1}"] = nc.dram_tensor(f"dbg_x{i+1}", [128, NT * F],
                                            DT.float32,
                                            kind="ExternalOutput").ap()
        dbg["poolT"] = nc.dram_tensor("dbg_poolT", [F, gpc], DT.float32,
                                      kind="ExternalOutput").ap()
        dbg["cT"] = nc.dram_tensor("dbg_cT", [2 * F, gpc], DT.float32,
                                   kind="ExternalOutput").ap()
    h_all_t = nc.dram_tensor("h_all", [CS, 128], DT.float16,
                             addr_space="Shared")
    h_all2_t = nc.dram_tensor("h_all2", [CS, 128], DT.float16,
                              addr_space="Shared")
    h_bufs = (h_all_t, h_all2_t)
    obg_dram = nc.dram_tensor("obg", [gpc, 1], DT.float32,
                              kind="ExternalOutput").ap()
    oeh_dram = nc.dram_tensor("oeh", [gpc, 1], DT.float32,
                              kind="ExternalOutput").ap()

    with tile.TileContext(nc) as tc, ExitStack() as stk:
        cpool = stk.enter_context(tc.tile_pool(name="const", bufs=1))
        dpool = stk.enter_context(tc.tile_pool(name="dram", bufs=1,
                                               space="DRAM"))
        wk = stk.enter_context(tc.tile_pool(name="wk", bufs=4))
        wk2 = stk.enter_context(tc.tile_pool(name="wk2", bufs=2))
        conv_stk = ExitStack()
        gp = conv_stk.enter_context(tc.tile_pool(name="gp", bufs=3))
        pp = conv_stk.enter_context(tc.tile_pool(name="pp", bufs=2,
                                                 space="PSUM"))
        ppf = conv_stk.enter_context(tc.tile_pool(name="ppf", bufs=2,
                                                  space="PSUM"))
        ppx = conv_stk.enter_context(tc.tile_pool(name="ppx", bufs=1,
                                                  space="PSUM"))

        h_own = dpool.tile([SLAB, 128], DT.float16)

        # ---------------- persistent SBUF ----------------
        def load_const(name, ap_dram, shape, dt):
            t = cpool.tile(list(shape), dt, tag=name)
            nc.sync.dma_start(out=t[:], in_=ap_dram)
            return t

        w1b_sb = load_const("w1b", w1b_dram, (F + 1, NCONV * F), DT.float32)
        w2b_sb = load_const("w2b", w2b_dram, (F + 1, NCONV * F), DT.float32)
        webbd8_sb = load_const("webbd8", webbd8_dram,
                               (FG * (NRBF + 1), NCONV * FG * F), DT.float16)
        webbdt_sb = load_const("webbdt", webbdt_dram,
                               (TFG * (NRBF + 1), NCONV * TFG * F), DT.float16)
        offs_sb = load_const("offs", offs_dram, (128, NRBF), DT.float32)
        iota_sb = load_const("iota", iota_dram, (128, 128), DT.float16)
        iotag_sb = load_const("iotag", iotag_dram, (128, gpc), DT.float32)
        id16_sb = load_const("id16", ident16_dram, (128, 128), DT.float16)
        id32_sb = load_const("id32", ident32_dram, (128, 128), DT.float32)
        invc_sb = load_const("invc", invc_dram, (F, gpc), DT.float32)
        ws_sb = load_const("ws", ws_dram, (F, 2 * F), DT.float32)
        bs_sb = load_const("bs", bs_dram, (2 * F, 1), DT.float32)
        wbg1_sb = load_const("wbg1", wbg1_dram, (2 * F, F), DT.float32)
        bbg1_sb = load_const("bbg1", bbg1_dram, (F, 1), DT.float32)
        wbg2_sb = load_const("wbg2", wbg2_dram, (F, 1), DT.float32)
        bbg2_sb = load_const("bbg2", bbg2_dram, (1, 1), DT.float32)
        weh1_sb = load_const("weh1", weh1_dram, (2 * F, F), DT.float32)
        beh1_sb = load_const("beh1", beh1_dram, (F, 1), DT.float32)
        weh2_sb = load_const("weh2", weh2_dram, (F, 1), DT.float32)
        beh2_sb = load_const("beh2", beh2_dram, (1, 1), DT.float32)
        xids_sb = load_const("xids", xids_dram, (128, SLAB // 16), DT.int16)

        x_sb = cpool.tile([128, NT * F], DT.float32, tag="x")

        # persistent static edge-side state (loaded/computed once, reused
        # across all convs)
        dst16_sb = cpool.tile([128, NT * BK], DT.float16, tag="dst16")
        nc.sync.dma_start(
            out=dst16_sb[:].rearrange("p (t b) -> p t b", b=BK),
            in_=dst_dram.rearrange("(t p) b -> p t b", p=128),
        )
        ixlo_sb = cpool.tile([128, NGRP * WL], DT.int16, tag="ixlo")
        nc.sync.dma_start(
            out=ixlo_sb[:].rearrange("p (g w) -> p g w", w=WL),
            in_=idxlo_dram.rearrange("(g p) w -> p g w", p=128),
        )
        ixhi_sb = cpool.tile([128, NGRP * WH], DT.int16, tag="ixhi")
        nc.sync.dma_start(
            out=ixhi_sb[:].rearrange("p (g w) -> p g w", w=WH),
            in_=idxhi_dram.rearrange("(g p) w -> p g w", p=128),
        )
        e16_sb = cpool.tile([128, NT * BK * R], DT.float16, tag="e16")
        hbuf = cpool.tile([128, NT * F], DT.float16, tag="hbuf")

        # ---------------- x0 = emb[x_ids] ----------------
        nc.gpsimd.dma_gather(
            x_sb[:].rearrange("p (b e) -> p b e", e=F),
            emb_dram,
            xids_sb[:],
            SLAB,
            SLAB,
            F,
            single_packet=False,
        )

        # ---------------- helpers ----------------
        def h_chain(iv, i):
            """hbuf[:, tile] = fp16(x_tile @ W1[i] + b1[i])."""
            xcp = wk2.tile([128, F], DT.float32, tag="xcp")
            nc.scalar.copy(xcp[:], x_sb[:, bass.ts(iv, F)])
            xT_ps = ppx.tile([F, 128], DT.float32, tag="xps")
            nc.tensor.transpose(xT_ps[:], xcp[:], id32_sb[:])
            xT_sb = wk2.tile([F + 1, 128], DT.float32, tag="xT")
            nc.scalar.copy(xT_sb[0:F, :], xT_ps[:])
            nc.vector.memset(xT_sb[F:F + 1, :], 1.0)
            h_ps = ppx.tile([128, F], DT.float32, tag="xps2")
            nc.tensor.matmul(h_ps[:], xT_sb[:], w1b_sb[:, i * F:(i + 1) * F],
                             start=True, stop=True)
            nc.scalar.copy(hbuf[:, bass.ts(iv, F)], h_ps[:])

        # h-chunk boundaries (in tiles) for the overlapped AllGather: each
        # chunk is flushed + allgathered as soon as its h tiles are done, so
        # the collective overlaps the remaining edge compute of the conv.
        AGC = sorted({round(NT * k / 4) for k in range(5)})
        if not chunk_ag:
            AGC = [0, NT]

        def flush_chunk(t0, t1):
            """DMA: hbuf tiles [t0,t1) (SBUF) -> h_own rows [:, 0:64]."""
            nc.sync.dma_start(
                out=h_own[:][t0 * 128:t1 * 128, 0:F].rearrange(
                    "(t p) c -> p t c", p=128),
                in_=hbuf[:, t0 * F:t1 * F].rearrange("p (t c) -> p t c", c=F),
            )

        def ag_chunk(dst_t, t0, t1):
            flush_chunk(t0, t1)
            nc.gpsimd.collective_compute(
                "AllGather",
                OP.bypass,
                replica_groups=[list(range(CORES))],
                ins=[h_own[:][t0 * 128:t1 * 128, :].opt()],
                outs=[dst_t[:].rearrange("(c s) f -> c s f", s=SLAB)[
                    :, t0 * 128:t1 * 128, :].opt()],
            )

        def gather_group(gv, hs_lo, hs_hi, h_src):
            nc.gpsimd.dma_gather(
                hs_lo[:].rearrange("p (b e) -> p b e", e=128),
                h_src[:], ixlo_sb[:, bass.ts(gv, WL)],
                GRP * LO_SLOTS, GRP * LO_SLOTS, 128, single_packet=False,
            )
            nc.gpsimd.dma_gather(
                hs_hi[:].rearrange("p (b e) -> p b e", e=128),
                h_src[:][LOCUT:CS, :], ixhi_sb[:, bass.ts(gv, WH)],
                GRP * HI_SLOTS, GRP * HI_SLOTS, 128, single_packet=False,
            )

        def edge_phase(iv, i, tg, hs_lo, hs_hi):
            """Returns aggT psum tile [F, 128] accumulated over the tile."""
            hsl3 = hs_lo[:].rearrange("p (b e) -> p b e", e=128)
            hsh3 = hs_hi[:].rearrange("p (b e) -> p b e", e=128)
            tile_e16 = e16_sb[:, bass.ts(iv, BK * R)]
            # local static-offset copy of this tile's dst ids (cheap; lets
            # the broadcast APs below use raw strides)
            dst_sb = wk.tile([128, BK], DT.float16, tag="dst")
            nc.gpsimd.tensor_copy(dst_sb[:], dst16_sb[:, bass.ts(iv, BK)])

            aggT_ps = pp.tile([F, 128], DT.float32, tag="aggT")

            # f / m / S / scatter in groups of FG chunks; per group one
            # transpose of e16 cols -> eT [fn*11, 128], one block-diag matmul
            for g0 in range(0, BK, FG):
                fn = min(FG, BK - g0)
                eT_ps = ppf.tile([FG * R, 128], DT.float16, tag="eTp")
                nc.tensor.transpose(
                    eT_ps[0:fn * R, :],
                    tile_e16[:, g0 * R:(g0 + fn) * R],
                    id16_sb[:],
                )
                eT_sb = wk.tile([FG * R, 128], DT.float16, tag="eTs")
                nc.scalar.copy(eT_sb[0:fn * R, :], eT_ps[0:fn * R, :])
                f_ps = ppf.tile([128, FG * F], DT.float32, tag="fps")
                bd = webbd8_sb if fn == FG else webbdt_sb
                nc.tensor.matmul(
                    f_ps[:, 0:fn * F],
                    eT_sb[0:fn * R, :],
                    bd[:, i * fn * F:(i + 1) * fn * F],
                    start=True, stop=True,
                )
                if mixed_tt:
                    f3 = f_ps[:].rearrange("p (b e) -> p b e", e=F)
                else:
                    f16 = wk.tile([128, FG * F], DT.float16, tag="f16")
                    nc.scalar.copy(f16[0:128, 0:fn * F], f_ps[:, 0:fn * F])
                    f3 = f16[:].rearrange("p (b e) -> p b e", e=F)
                m_sb = wk.tile([128, FG * F], DT.float16, tag="m")
                segs = []
                c0, c1 = g0, g0 + fn
                if c0 < LO_BLKS:
                    segs.append((hsl3, tg * LO_BLKS + c0, c0,
                                 min(c1, LO_BLKS) - c0))
                if c1 > LO_BLKS:
                    cc0 = max(c0, LO_BLKS)
                    segs.append((hsh3, tg * HI_BLKS + (cc0 - LO_BLKS),
                                 cc0, c1 - cc0))
                for (src3, b0, coff, n) in segs:
                    nc.vector.tensor_tensor(
                        m_sb[:].rearrange("p (b e) -> p b e", e=F)[
                            :, coff - g0:coff - g0 + n, :],
                        src3[:, b0:b0 + n, 0:F],
                        f3[:, coff - g0:coff - g0 + n, :],
                        OP.mult,
                    )
                S_sb = wk.tile([128, FG * 128], DT.float16, tag="S")
                dst_b = bass.AP(
                    dst_sb.tensor,
                    dst_sb[:, g0:g0 + fn].offset,
                    [dst_sb[:].ap[0], [1, fn], [0, 128]],
                )
                iota_b = bass.AP(
                    iota_sb.tensor, iota_sb[:].offset,
                    [iota_sb[:].ap[0], [0, fn], [1, 128]],
                )
                nc.vector.tensor_tensor(
                    S_sb[:].rearrange("p (b e) -> p b e", e=128)[:, 0:fn, :],
                    dst_b, iota_b, OP.is_equal,
                )
                for c in range(fn):
                    cg = g0 + c
                    nc.tensor.matmul(
                        aggT_ps[:],
                        m_sb[:, c * F:(c + 1) * F],
                        S_sb[:, c * 128:(c + 1) * 128],
                        start=(cg == 0), stop=(cg == BK - 1),
                    )
            return aggT_ps

        def x_accum(iv, i, tg, aggT_ps, xs_g):
            """xs_g[:, tg*F:] = x + agg @ W2 + b2 (pre-softplus)."""
            aggT_sb = wk2.tile([F + 1, 128], DT.float32, tag="aggTs")
            nc.scalar.copy(aggT_sb[0:F, :], aggT_ps[:])
            nc.vector.memset(aggT_sb[F:F + 1, :], 1.0)
            xup_ps = ppx.tile([128, F], DT.float32, tag="xps2")
            nc.tensor.matmul(xup_ps[:], aggT_sb[:],
                             w2b_sb[:, i * F:(i + 1) * F],
                             start=True, stop=True)
            nc.vector.tensor_tensor(xs_g[:, tg * F:(tg + 1) * F], xup_ps[:],
                                    x_sb[:, bass.ts(iv, F)], OP.add)

        def softplus_group(gv, xs_g):
            """x_sb[group] = relu(xs) + ln1p(exp(-|xs|)), poly ln1p on Pool.

            Uses only Abs/Exp activations so one act-func table set serves
            the whole program (no per-tile table reloads).
            """
            n = GRP * F
            ab = wk2.tile([128, n], DT.float32, tag="ab")
            nc.scalar.activation(ab[:], xs_g[:], AF.Abs)
            nc.scalar.activation(ab[:], ab[:], AF.Exp, scale=-1.0)
            acc = wk2.tile([128, n], DT.float32, tag="acc")
            nc.vector.tensor_scalar(acc[:], ab[:], LN1P_C[4], LN1P_C[3],
                                    OP.mult, OP.add)
            for ck in (LN1P_C[2], LN1P_C[1], LN1P_C[0]):
                nc.vector.tensor_tensor(acc[:], acc[:], ab[:], OP.mult)
                nc.vector.tensor_scalar_add(acc[:], acc[:], ck)
            nc.vector.tensor_tensor(acc[:], acc[:], ab[:], OP.mult)
            nc.vector.scalar_tensor_tensor(
                x_sb[:, bass.ts(gv, GRP * F)], xs_g[:], 0.0, acc[:],
                OP.max, OP.add)

        # ---------------- prologue: e16 (RBF) + h0 ----------------
        nc.vector.memset(e16_sb[:], 1.0)  # aug ones cols; exp fills the rest

        def body_pro(iv):
            d_sb = wk.tile([128, BK], DT.float32, tag="d")
            nc.sync.dma_start(
                out=d_sb[:],
                in_=d_dram.rearrange("(t p) b -> t p b", p=128)[
                    bass.ds(iv, 1)][0],
            )
            e32 = wk.tile([128, BK * NRBF], DT.float32, tag="e32")
            d_b = bass.AP(d_sb.tensor, d_sb[:].offset,
                          [d_sb[:].ap[0], d_sb[:].ap[1], [0, NRBF]])
            offs_b = bass.AP(offs_sb.tensor, offs_sb[:].offset,
                             [offs_sb[:].ap[0], [0, BK], offs_sb[:].ap[1]])
            e32_3 = e32[:].rearrange("p (b r) -> p b r", r=NRBF)
            nc.vector.tensor_tensor(e32_3, d_b, offs_b, OP.subtract)
            nc.vector.tensor_tensor(e32[:], e32[:], e32[:], OP.mult)
            e16_t = e16_sb[:, bass.ts(iv, BK * R)].rearrange(
                "p (b r) -> p b r", r=R)
            nc.scalar.activation(e16_t[:, :, 0:NRBF], e32_3, AF.Exp,
                                 scale=coeff)
            h_chain(iv, 0)

        # prologue tile loop with chunked h0 allgathers into h_bufs[0]
        ag_i = 0
        for t in range(NT):
            body_pro(t)
            while ag_i + 1 < len(AGC) and t + 1 == AGC[ag_i + 1]:
                ag_chunk(h_bufs[0], AGC[ag_i], AGC[ag_i + 1])
                ag_i += 1
        stopped = stop_after in ("h0", "ag0")
        if debug:
            nc.sync.dma_start(out=dbg["x0"], in_=x_sb[:])
            nc.sync.dma_start(out=dbg["hall0"], in_=h_all_t[:])

        conv_c = 0  # running conv index for h_all ping-pong
        for rep in range(repeat):
            if stopped:
                break
            for i in range(NCONV):
                if stopped or (stop_after is not None
                               and stop_after.startswith("conv")
                               and i > int(stop_after[4:])):
                    stopped = True
                    break
                last = (rep == repeat - 1) and (i == NCONV - 1)
                rbuf = h_bufs[conv_c % 2]
                wbuf = h_bufs[(conv_c + 1) % 2]

                def body_conv_group(gv, i=i, last=last, rbuf=rbuf):
                    hs_lo = gp.tile([128, GRP * LO_SLOTS], DT.float16,
                                    tag="hslo")
                    hs_hi = gp.tile([128, GRP * HI_SLOTS], DT.float16,
                                    tag="hshi")
                    gather_group(gv, hs_lo, hs_hi, rbuf)
                    xs_g = gp.tile([128, GRP * F], DT.float32, tag="xsg")
                    for tg in range(GRP):
                        iv = gv * GRP + tg
                        aggT_ps = edge_phase(iv, i, tg, hs_lo, hs_hi)
                        x_accum(iv, i, tg, aggT_ps, xs_g)
                    softplus_group(gv, xs_g)
                    if not last:
                        for tg in range(GRP):
                            h_chain(gv * GRP + tg, (i + 1) % NCONV)

                ag_i = 0
                for g in range(NGRP):
                    body_conv_group(g)
                    if not last:
                        # issue each h chunk's allgather as soon as its
                        # tiles' h_chain is done (overlaps edge compute)
                        while (ag_i + 1 < len(AGC)
                               and (g + 1) * GRP >= AGC[ag_i + 1]):
                            ag_chunk(wbuf, AGC[ag_i], AGC[ag_i + 1])
                            ag_i += 1
                if stop_after == f"conv{i}" and not last:
                    stopped = True
                    break
                if debug and rep == repeat - 1:
                    nc.sync.dma_start(out=dbg[f"x{i+1}"], in_=x_sb[:])
                conv_c += 1

        # ---------------- pooling ----------------
        if stopped:
            zz = wk.tile([1, gpc], DT.float32, tag="zz")
            nc.vector.memset(zz[:], 0.0)
            nc.sync.dma_start(out=obg_dram.rearrange("g one -> one g")[0:1, :],
                              in_=zz[:])
            nc.sync.dma_start(out=oeh_dram.rearrange("g one -> one g")[0:1, :],
                              in_=zz[:])
            conv_stk.close()
            do_rest = False
        else:
            conv_stk.close()
            do_rest = True
        if do_rest:
            ppm = stk.enter_context(tc.tile_pool(name="ppm", bufs=1,
                                                 space="PSUM"))
            poolT_ps = ppm.tile([F, gpc], DT.float32, tag="poolT")

            def body_pool(iv):
                gid_sb = wk.tile([128, 1], DT.float16, tag="gid")
                nc.sync.dma_start(
                    out=gid_sb[:],
                    in_=gid_dram.rearrange("(t p) b -> t p b", p=128)[
                        bass.ds(iv, 1)][0],
                )
                gid32 = wk.tile([128, 1], DT.float32, tag="gid32")
                nc.scalar.copy(gid32[:], gid_sb[:])
                Sp = wk.tile([128, gpc], DT.float32, tag="Sp")
                gid_b = bass.AP(gid32.tensor, gid32[:].offset,
                                [gid32[:].ap[0], [0, gpc]])
                nc.vector.tensor_tensor(Sp[:], gid_b, iotag_sb[:], OP.is_equal)
                nc.tensor.matmul(poolT_ps[:], x_sb[:, iv * F:(iv + 1) * F],
                                 Sp[:], start=(iv == 0), stop=(iv == NT - 1))

            for t in range(NT):
                body_pool(t)

            # mean + MLP (feature-major: cT = relu(Ws.T @ pooled + bs))
            poolT_sb = wk.tile([F, gpc], DT.float32, tag="poolTs")
            nc.vector.tensor_tensor(poolT_sb[:], poolT_ps[:], invc_sb[:], OP.mult)
            cT_ps = ppm.tile([2 * F, gpc], DT.float32, tag="cT")
            nc.tensor.matmul(cT_ps[:], ws_sb[:], poolT_sb[:], start=True,
                             stop=True)
            cT_sb = wk.tile([2 * F, gpc], DT.float32, tag="cTs")
            nc.scalar.activation(cT_sb[:], cT_ps[:], AF.Relu, bias=bs_sb[:])
            if debug:
                nc.sync.dma_start(out=dbg["poolT"], in_=poolT_sb[:])
                nc.sync.dma_start(out=dbg["cT"], in_=cT_sb[:])

            for (w1s, b1s, w2s, b2s, out_dram, tg) in (
                (wbg1_sb, bbg1_sb, wbg2_sb, bbg2_sb, obg_dram, "bg"),
                (weh1_sb, beh1_sb, weh2_sb, beh2_sb, oeh_dram, "eh"),
            ):
                t1_ps = ppm.tile([F, gpc], DT.float32, tag="t1")
                nc.tensor.matmul(t1_ps[:], w1s[:], cT_sb[:], start=True, stop=True)
                t1_sb = wk.tile([F, gpc], DT.float32, tag="t1s" + tg)
                nc.scalar.activation(t1_sb[:], t1_ps[:], AF.Relu, bias=b1s[:])
                o_ps = ppm.tile([1, gpc], DT.float32, tag="o")
                nc.tensor.matmul(o_ps[:], w2s[:], t1_sb[:], start=True, stop=True)
                o_sb = wk.tile([1, gpc], DT.float32, tag="os" + tg)
                nc.scalar.activation(o_sb[:], o_ps[:], AF.Identity, bias=b2s[:])
                nc.sync.dma_start(out=out_dram.rearrange("g one -> one g")[0:1, :],
                                  in_=o_sb[:])

    nc.compile()
    return nc


def make_in_maps(prep):
    """Per-core input dicts for run_bass_kernel_spmd."""
    NT, BK, SLAB, gpc = prep.NT, prep.BK, prep.SLAB, prep.gpc
    LO_SLOTS, HI_SLOTS = prep.LO_BLKS * 128, prep.HI_BLKS * 128
    maps = []
    iota128 = np.tile(np.arange(128, dtype=np.float16)[None, :], (128, 1))
    iotag = np.tile(np.arange(gpc, dtype=np.float32)[None, :], (128, 1))
    id16 = np.eye(128, dtype=np.float16)
    id32 = np.eye(128, dtype=np.float32)
    offs = np.tile(prep.offs[None, :], (128, 1)).astype(np.float32)

    def rep16(a):  # [16, S] -> [128, S] replicated
        return np.tile(a, (8, 1))

    R = NRBF + 1
    FG = 8
    TFG = BK % FG if BK % FG else FG

    def make_bd(fn):
        bd = np.zeros((fn * R, NCONV * fn * F), np.float16)
        for i in range(NCONV):
            for c in range(fn):
                bd[c * R:(c + 1) * R, i * fn * F + c * F:i * fn * F +
                   (c + 1) * F] = prep.Web[i]
        return bd

    webbd8 = make_bd(FG)
    webbdt = make_bd(TFG)

    for k in range(CORES):
        m = dict(
            d_edges=prep.d_arr[k].reshape(NT * 128, BK),
            dst_edges=prep.dst_arr[k].reshape(NT * 128, BK),
            idxlo=np.ascontiguousarray(
                np.tile(prep.idxlo_g[k], (1, 8, 1)).reshape(
                    prep.NGRP * 128, prep.GRP * LO_SLOTS // 16)),
            idxhi=np.ascontiguousarray(
                np.tile(prep.idxhi_g[k], (1, 8, 1)).reshape(
                    prep.NGRP * 128, prep.GRP * HI_SLOTS // 16)),
            gid=prep.gid_slab[k].reshape(NT * 128, 1).astype(np.float16),
            xids=rep16(prep.xids_w[k]),
            emb95=prep.emb,
            w1b=np.ascontiguousarray(
                prep.W1b.transpose(1, 0, 2).reshape(F + 1, NCONV * F)),
            w2b=np.ascontiguousarray(
                prep.W2b.transpose(1, 0, 2).reshape(F + 1, NCONV * F)),
            webbd8=webbd8,
            webbdt=webbdt,
            offs=offs,
            iota128=iota128,
            iotag=iotag,
            ident16=id16,
            ident32=id32,
            invc=np.tile(prep.inv_cnt[k][None, :], (F, 1)).astype(np.float32),
            ws=prep.Ws,
            bs=prep.bs.reshape(2 * F, 1),
            wbg1=prep.Wbg1,
            bbg1=prep.bbg1.reshape(F, 1),
            wbg2=prep.Wbg2,
            bbg2=prep.bbg2.reshape(1, 1),
            weh1=prep.Weh1,
            beh1=prep.beh1.reshape(F, 1),
            weh2=prep.Weh2,
            beh2=prep.beh2.reshape(1, 1),
        )
        maps.append({k2: np.ascontiguousarray(v) for k2, v in m.items()})
    return maps


def kernel(**inputs):
    import numpy as np
    from concourse.bass_utils import run_bass_kernel_spmd

    wkeys = ("emb blk_W1 blk_b1 blk_We blk_be blk_W2 blk_b2 Ws bs Wbg1 bbg1 "
             "Wbg2 bbg2 Weh1 beh1 Weh2 beh2").split()
    weights = {k: np.asarray(inputs[k]) for k in wkeys}
    p = Prep(np.asarray(inputs["x_ids"]), np.asarray(inputs["edge_index"]),
             np.asarray(inputs["edge_attr"]), np.asarray(inputs["batch"]),
             weights, n_graphs=512)
    nc = build_bass(p, unroll=25)
    maps = make_in_maps(p)
    res = run_bass_kernel_spmd(nc, maps, list(range(CORES)))
    bg = np.concatenate([np.asarray(res.results[k]["obg"], dtype=np.float32)
                         for k in range(CORES)])
    eh = np.concatenate([np.asarray(res.results[k]["oeh"], dtype=np.float32)
                         for k in range(CORES)])
    return bg, eh



# revision 37
# speedup vs baseline: 1.2869x; 1.0605x over previous
"""CrystalGNN (SchNet-style) Trainium2 Bass kernel — self-contained.

Sharding: nodes/graphs block-partitioned across 8 NeuronCores (graph-aligned
slabs); edges partitioned by owner(dst) and grouped by 128-node dst tile;
small weights replicated. Per conv: h = x@W1+b1 (fp16, slab-local) ->
AllGather -> per-tile edge pipeline (dma_gather h[src], RBF+block-diag Web
matmul for f, m = h*f, one-hot scatter matmul into PSUM) -> x update +
softplus. Mean-pool via one-hot matmul + small MLP heads on-device.
"""
"""Host-side sharding/preprocessing + numpy device-model for the CrystalGNN kernel.

Everything is parameterized by the problem dims so the same code paths can be
exercised at a small scale in CoreSim and at full scale on hardware.
"""
import numpy as np

F = 64          # atom feats
NRBF = 10
NCONV = 3
H = 64
CORES = 8


def ceil_div(a, b):
    return (a + b - 1) // b


def round_up(a, b):
    return ceil_div(a, b) * b


class Prep:
    """Per-problem host preprocessing. All outputs are numpy arrays keyed for
    the bass kernel's DRAM tensors (one dict per core)."""

    def __init__(self, x_ids, edge_index, edge_attr, batch, weights, n_graphs,
                 locut=None):
        N = x_ids.shape[0]
        E = edge_index.shape[1]
        G = n_graphs
        assert G % CORES == 0
        gpc = G // CORES  # graphs per core
        self.N, self.E, self.G, self.gpc = N, E, G, gpc

        batch = np.asarray(batch).astype(np.int64)
        x_ids = np.asarray(x_ids).astype(np.int64)
        src = np.asarray(edge_index[0]).astype(np.int64)
        dst = np.asarray(edge_index[1]).astype(np.int64)
        d = np.asarray(edge_attr).astype(np.float32)

        # graph -> node range (batch is sorted)
        gstart = np.searchsorted(batch, np.arange(G), side="left")
        gend = np.searchsorted(batch, np.arange(G), side="right")
        # core k owns graphs [k*gpc, (k+1)*gpc) -> nodes [cstart[k], cend[k])
        cstart = gstart[np.arange(CORES) * gpc]
        cend = np.append(cstart[1:], N)
        own = cend - cstart
        max_own = int(own.max())
        # slab size: per-core node capacity, multiple of 128
        SLAB = round_up(max_own, 128)
        NT = SLAB // 128  # node tiles per core
        self.SLAB, self.NT = SLAB, NT
        self.cstart, self.cend = cstart, cend

        # slab row of each global node
        owner = np.searchsorted(cstart, np.arange(N), side="right") - 1

        # int16 split point for gather indices (slab rows)
        self.LOCUT = min(32768, CORES * SLAB) if locut is None else locut

        # ---- balance dst-degree across each core's 128-node tiles: greedy
        # LPT assignment of nodes to tiles minimizing per-tile
        # ceil(lo/128)+ceil(hi/128), which sets BK (the padded edge-slot
        # budget every edge-phase cost scales with).
        srow_old = SLAB * owner + (np.arange(N) - cstart[owner])
        est_lo = srow_old[src] < self.LOCUT
        deg_lo = np.zeros(N, np.int64)
        deg_hi = np.zeros(N, np.int64)
        np.add.at(deg_lo, dst[est_lo], 1)
        np.add.at(deg_hi, dst[~est_lo], 1)
        new_local = np.empty(N, np.int64)
        for k in range(CORES):
            nodes = np.arange(cstart[k], cend[k])
            # balance hi tightly (its block budget has usable slack; lo's
            # does not), keep lo under its existing block budget
            order_n = nodes[np.argsort(
                -(deg_hi[nodes] * 10000 + deg_lo[nodes]), kind="stable")]
            tl = np.zeros(NT, np.int64)
            th = np.zeros(NT, np.int64)
            tc = np.zeros(NT, np.int64)
            lo_cap = 128 * (-(-(deg_lo[nodes].sum()) // (128 * NT)) + 1)
            for n in order_n:
                dl, dh = deg_lo[n], deg_hi[n]
                cost = (th + dh) * 10**6 + (tl + dl)
                cost[(tc >= 128) | (tl + dl > lo_cap)] = 2**62
                t = int(np.argmin(cost))
                new_local[n] = t * 128 + tc[t]
                tl[t] += dl
                th[t] += dh
                tc[t] += 1
        srow = SLAB * owner + new_local
        self.owner, self.srow = owner, srow

        # ---- edge partition: edge belongs to owner[dst], tile = local dst block
        e_owner = owner[dst]
        e_tile = new_local[dst] // 128
        e_dstloc = new_local[dst] % 128                    # local id within tile
        e_srow = srow[src]
        e_lo = e_srow < self.LOCUT

        # per (core, tile): count lo/hi edges
        # order edges by (core, tile, hi?, arbitrary)
        key = ((e_owner * NT + e_tile) * 2 + (~e_lo).astype(np.int64))
        order = np.argsort(key, kind="stable")
        s_core = e_owner[order]
        s_tile = e_tile[order]
        s_lo = e_lo[order]
        s_d = d[order]
        s_dstloc = e_dstloc[order]
        s_srow = e_srow[order]

        # counts
        n_lo = np.zeros((CORES, NT), np.int64)
        n_hi = np.zeros((CORES, NT), np.int64)
        np.add.at(n_lo, (e_owner[e_lo], e_tile[e_lo]), 1)
        np.add.at(n_hi, (e_owner[~e_lo], e_tile[~e_lo]), 1)
        LO_BLKS = int(ceil_div(n_lo.max(), 128))
        HI_BLKS = int(ceil_div(n_hi.max(), 128))
        BK = LO_BLKS + HI_BLKS
        self.LO_BLKS, self.HI_BLKS, self.BK = LO_BLKS, HI_BLKS, BK
        self.n_lo, self.n_hi = n_lo, n_hi

        LO_SLOTS = LO_BLKS * 128
        HI_SLOTS = HI_BLKS * 128
        SLOTS = BK * 128

        # ---- per-core packed arrays
        # slot s of (core,tile): s in [0, LO_SLOTS) lo edges; [LO_SLOTS, SLOTS) hi
        d_arr = np.zeros((CORES, NT, 128, BK), np.float32)
        dst_arr = np.full((CORES, NT, 128, BK), -1.0, np.float16)
        idxlo = np.zeros((CORES, NT, LO_SLOTS), np.int16)
        idxhi = np.zeros((CORES, NT, HI_SLOTS), np.int16)

        # fill using the sorted stream
        # positions within each (core,tile,lo/hi) group
        grp_key = (s_core * NT + s_tile) * 2 + (~s_lo).astype(np.int64)
        # index within group
        uniq, first_idx = np.unique(grp_key, return_index=True)
        pos_in_grp = np.arange(len(grp_key)) - np.repeat(
            first_idx, np.diff(np.append(first_idx, len(grp_key)))
        )
        slot = np.where(s_lo, pos_in_grp, LO_SLOTS + pos_in_grp)
        p = slot % 128
        b = slot // 128
        d_arr[s_core, s_tile, p, b] = s_d
        dst_arr[s_core, s_tile, p, b] = s_dstloc.astype(np.float16)
        lo_m = s_lo
        idxlo[s_core[lo_m], s_tile[lo_m], pos_in_grp[lo_m]] = s_srow[lo_m].astype(
            np.int16
        )
        hi_m = ~s_lo
        idxhi[s_core[hi_m], s_tile[hi_m], pos_in_grp[hi_m]] = (
            s_srow[hi_m] - self.LOCUT
        ).astype(np.int16)

        # pad gather idx: everything stays -1 after the real edges (trailing skip).
        # counts per tile (rounded: the dma consumes them via num_idxs_reg)
        cnts = np.stack([n_lo, n_hi], axis=-1).astype(np.int32)  # [CORES, NT, 2]

        # wrap idx arrays to the [16, n/16] layout: position i -> [i % 16, i // 16]
        def wrap16(a):  # [..., S] -> [..., 16, S//16]
            S = a.shape[-1]
            return np.ascontiguousarray(
                a.reshape(*a.shape[:-1], S // 16, 16).swapaxes(-1, -2)
            )

        GRP = 2 if NT % 2 == 0 else 1
        self.GRP, self.NGRP = GRP, NT // GRP
        self.d_arr = d_arr
        self.dst_arr = dst_arr
        self.idxlo_w = wrap16(idxlo)
        self.idxhi_w = wrap16(idxhi)
        self.idxlo_g = wrap16(idxlo.reshape(CORES, self.NGRP, GRP * LO_SLOTS))
        self.idxhi_g = wrap16(idxhi.reshape(CORES, self.NGRP, GRP * HI_SLOTS))
        self.cnts = cnts

        # ---- node-side per-core tables
        # x_ids slab: [CORES, SLAB] (pad -> 0)
        xids_slab = np.zeros((CORES, SLAB), np.int64)
        gid_slab = np.full((CORES, SLAB), -1.0, np.float16)  # local graph id
        for k in range(CORES):
            nodes = np.arange(cstart[k], cend[k])
            xids_slab[k, new_local[nodes]] = x_ids[nodes]
            gid_slab[k, new_local[nodes]] = (
                batch[nodes] - k * gpc).astype(np.float16)
        self.xids_w = wrap16(xids_slab.astype(np.int16))  # emb table < 32768 rows
        self.gid_slab = gid_slab.reshape(CORES, NT, 128)

        # inverse counts per graph (local)
        cnt_g = np.zeros((CORES, gpc), np.float32)
        for k in range(CORES):
            ids, c = np.unique(
                (batch[cstart[k]:cend[k]] - k * gpc), return_counts=True
            )
            cnt_g[k, ids] = c
        self.inv_cnt = (1.0 / np.maximum(cnt_g, 1.0)).astype(np.float32)  # [CORES,gpc]

        # ---- weights (augmented)
        w = weights
        self.W1b = np.concatenate(
            [w["blk_W1"], w["blk_b1"][:, None, :]], axis=1
        ).astype(np.float32)  # [NCONV, F+1, F]
        self.W2b = np.concatenate(
            [w["blk_W2"], w["blk_b2"][:, None, :]], axis=1
        ).astype(np.float32)
        self.Web = np.concatenate(
            [w["blk_We"], w["blk_be"][:, None, :]], axis=1
        ).astype(np.float16)  # [NCONV, NRBF+1, F]
        self.emb = np.asarray(w["emb"]).astype(np.float32)
        for nm in ("Ws", "bs", "Wbg1", "bbg1", "Wbg2", "bbg2",
                   "Weh1", "beh1", "Weh2", "beh2"):
            setattr(self, nm, np.asarray(w[nm]).astype(np.float32))

        # RBF offsets
        offs = np.linspace(0.0, 6.0, NRBF).astype(np.float32)
        self.offs = offs
        self.coeff = np.float32(-0.5 / (offs[1] - offs[0]) ** 2)



"""Bass/Tile kernel builder for the CrystalGNN (SchNet-style) message-passing net.

Data layout (per core, SPMD identical program):
  - nodes sharded by graph: core k owns graphs [k*gpc,(k+1)*gpc) -> a slab of
    SLAB node rows (NT = SLAB/128 tiles of 128 nodes).
  - x state [128, NT*F] f32 lives in SBUF for the whole kernel.
  - h table (x @ W1 + b1, fp16, padded to 128 cols) is written per-slab to DRAM
    and AllGather'd so every core can dma_gather rows of any node.
  - edges partitioned by owner(dst), grouped by dst tile; per tile a fixed
    budget of BK*128 edge slots (LO_BLKS lo-src + HI_BLKS hi-src blocks,
    src slab-row < / >= LOCUT for int16 gather indices).
  - per tile: gather h[src]; RBF e from distances; f = e_aug @ Web (PE);
    m = h*f (DVE, fp16); one-hot S from dst ids (DVE is_equal);
    aggT[64,128] += m.T @ S (PE, PSUM f32); x += aggT.T@W2+b2; softplus.
  - pool: pooledT[F,gpc] += x_tile.T @ onehot(graph); * 1/cnt; 3-layer MLP.
"""
import numpy as np
from contextlib import ExitStack

import concourse.bass as bass
import concourse.bacc as bacc
import concourse.mybir as mybir
from concourse import tile

F = 64
NRBF = 10
NCONV = 3
CORES = 8
AF = mybir.ActivationFunctionType
OP = mybir.AluOpType
DT = mybir.dt


# ln(1+t) ~= t*(P0 + P1 t + P2 t^2 + P3 t^3 + P4 t^4), t in [0,1]
# (max abs err 8.1e-5; exact 0 at t=0). Lets softplus avoid the Ln
# activation so a single act-func table serves the whole program.
LN1P_C = (0.99988793, -0.49636828, 0.30467236, -0.15602843, 0.04106451)


def build_bass(prep, unroll=10, debug=False, stop_after=None, py_loops=False,
               repeat=1, ag_strided=True, mixed_tt=True, chunk_ag=False,
               skip_conv_ag=False):
    """Returns (nc, input_names) — the SPMD program for all cores."""
    NT, BK = prep.NT, prep.BK
    LO_BLKS, HI_BLKS = prep.LO_BLKS, prep.HI_BLKS
    LO_SLOTS, HI_SLOTS = LO_BLKS * 128, HI_BLKS * 128
    SLAB, gpc, LOCUT = prep.SLAB, prep.gpc, prep.LOCUT
    GRP, NGRP = prep.GRP, prep.NGRP
    CS = CORES * SLAB
    coeff = float(prep.coeff)
    R = NRBF + 1
    WL = GRP * LO_SLOTS // 16
    WH = GRP * HI_SLOTS // 16

    nc = bacc.Bacc("TRN2", target_bir_lowering=False, debug=False,
                   num_devices=CORES)

    # ---------------- DRAM inputs ----------------
    def din(name, shape, dt):
        return nc.dram_tensor(name, list(shape), dt, kind="ExternalInput").ap()

    d_dram = din("d_edges", (NT * 128, BK), DT.float32)
    dst_dram = din("dst_edges", (NT * 128, BK), DT.float16)
    idxlo_dram = din("idxlo", (NGRP * 128, GRP * LO_SLOTS // 16), DT.int16)
    idxhi_dram = din("idxhi", (NGRP * 128, GRP * HI_SLOTS // 16), DT.int16)
    gid_dram = din("gid", (NT * 128, 1), DT.float16)
    xids_dram = din("xids", (128, SLAB // 16), DT.int16)
    emb_dram = din("emb95", (95, F), DT.float32)
    w1b_dram = din("w1b", (F + 1, NCONV * F), DT.float32)
    w2b_dram = din("w2b", (F + 1, NCONV * F), DT.float32)
    # block-diagonal Web for grouped f-matmuls: groups of FG=8 chunks (and a
    # tail group of BK%8 chunks). webbd8[(c,k),(c',j)] = Web[k,j] * (c==c')
    FG = 8
    TFG = BK % FG if BK % FG else FG  # tail group size
    webbd8_dram = din("webbd8", (FG * (NRBF + 1), NCONV * FG * F), DT.float16)
    webbdt_dram = din("webbdt", (TFG * (NRBF + 1), NCONV * TFG * F),
                      DT.float16)
    offs_dram = din("offs", (128, NRBF), DT.float32)
    iota_dram = din("iota128", (128, 128), DT.float16)
    iotag_dram = din("iotag", (128, gpc), DT.float32)
    ident16_dram = din("ident16", (128, 128), DT.float16)
    ident32_dram = din("ident32", (128, 128), DT.float32)
    invc_dram = din("invc", (F, gpc), DT.float32)
    ws_dram = din("ws", (F, 2 * F), DT.float32)
    bs_dram = din("bs", (2 * F, 1), DT.float32)
    wbg1_dram = din("wbg1", (2 * F, F), DT.float32)
    bbg1_dram = din("bbg1", (F, 1), DT.float32)
    wbg2_dram = din("wbg2", (F, 1), DT.float32)
    bbg2_dram = din("bbg2", (1, 1), DT.float32)
    weh1_dram = din("weh1", (2 * F, F), DT.float32)
    beh1_dram = din("beh1", (F, 1), DT.float32)
    weh2_dram = din("weh2", (F, 1), DT.float32)
    beh2_dram = din("beh2", (1, 1), DT.float32)

    dbg = {}
    if debug:
        dbg["x0"] = nc.dram_tensor("dbg_x0", [128, NT * F], DT.float32,
                                   kind="ExternalOutput").ap()
        dbg["hall0"] = nc.dram_tensor("dbg_hall0", [CS, 128], DT.float16,
                                      kind="ExternalOutput").ap()
        for i in range(NCONV):
            dbg[f"x{i+1}"] = nc.dram_tensor(f"dbg_x{i+1}", [128, NT * F],
                                            DT.float32,
                                            kind="ExternalOutput").ap()
        dbg["poolT"] = nc.dram_tensor("dbg_poolT", [F, gpc], DT.float32,
                                      kind="ExternalOutput").ap()
        dbg["cT"] = nc.dram_tensor("dbg_cT", [2 * F, gpc], DT.float32,
                                   kind="ExternalOutput").ap()
    h_all_t = nc.dram_tensor("h_all", [CS, 128], DT.float16,
                             addr_space="Shared")
    h_all2_t = nc.dram_tensor("h_all2", [CS, 128], DT.float16,
                              addr_space="Shared")
    h_bufs = (h_all_t, h_all2_t)
    obg_dram = nc.dram_tensor("obg", [gpc, 1], DT.float32,
                              kind="ExternalOutput").ap()
    oeh_dram = nc.dram_tensor("oeh", [gpc, 1], DT.float32,
                              kind="ExternalOutput").ap()

    with tile.TileContext(nc) as tc, ExitStack() as stk:
        cpool = stk.enter_context(tc.tile_pool(name="const", bufs=1))
        dpool = stk.enter_context(tc.tile_pool(name="dram", bufs=1,
                                               space="DRAM"))
        wk = stk.enter_context(tc.tile_pool(name="wk", bufs=4))
        wk2 = stk.enter_context(tc.tile_pool(name="wk2", bufs=2))
        conv_stk = ExitStack()
        gp = conv_stk.enter_context(tc.tile_pool(name="gp", bufs=3))
        pp = conv_stk.enter_context(tc.tile_pool(name="pp", bufs=2,
                                                 space="PSUM"))
        ppf = conv_stk.enter_context(tc.tile_pool(name="ppf", bufs=2,
                                                  space="PSUM"))
        ppx = conv_stk.enter_context(tc.tile_pool(name="ppx", bufs=1,
                                                  space="PSUM"))

        h_own = dpool.tile([SLAB, 128], DT.float16)

        # ---------------- persistent SBUF ----------------
        def load_const(name, ap_dram, shape, dt):
            t = cpool.tile(list(shape), dt, tag=name)
            nc.sync.dma_start(out=t[:], in_=ap_dram)
            return t

        w1b_sb = load_const("w1b", w1b_dram, (F + 1, NCONV * F), DT.float32)
        w2b_sb = load_const("w2b", w2b_dram, (F + 1, NCONV * F), DT.float32)
        webbd8_sb = load_const("webbd8", webbd8_dram,
                               (FG * (NRBF + 1), NCONV * FG * F), DT.float16)
        webbdt_sb = load_const("webbdt", webbdt_dram,
                               (TFG * (NRBF + 1), NCONV * TFG * F), DT.float16)
        offs_sb = load_const("offs", offs_dram, (128, NRBF), DT.float32)
        iota_sb = load_const("iota", iota_dram, (128, 128), DT.float16)
        iotag_sb = load_const("iotag", iotag_dram, (128, gpc), DT.float32)
        id16_sb = load_const("id16", ident16_dram, (128, 128), DT.float16)
        id32_sb = load_const("id32", ident32_dram, (128, 128), DT.float32)
        invc_sb = load_const("invc", invc_dram, (F, gpc), DT.float32)
        ws_sb = load_const("ws", ws_dram, (F, 2 * F), DT.float32)
        bs_sb = load_const("bs", bs_dram, (2 * F, 1), DT.float32)
        wbg1_sb = load_const("wbg1", wbg1_dram, (2 * F, F), DT.float32)
        bbg1_sb = load_const("bbg1", bbg1_dram, (F, 1), DT.float32)
        wbg2_sb = load_const("wbg2", wbg2_dram, (F, 1), DT.float32)
        bbg2_sb = load_const("bbg2", bbg2_dram, (1, 1), DT.float32)
        weh1_sb = load_const("weh1", weh1_dram, (2 * F, F), DT.float32)
        beh1_sb = load_const("beh1", beh1_dram, (F, 1), DT.float32)
        weh2_sb = load_const("weh2", weh2_dram, (F, 1), DT.float32)
        beh2_sb = load_const("beh2", beh2_dram, (1, 1), DT.float32)
        xids_sb = load_const("xids", xids_dram, (128, SLAB // 16), DT.int16)

        x_sb = cpool.tile([128, NT * F], DT.float32, tag="x")

        # persistent static edge-side state (loaded/computed once, reused
        # across all convs)
        dst16_sb = cpool.tile([128, NT * BK], DT.float16, tag="dst16")
        nc.sync.dma_start(
            out=dst16_sb[:].rearrange("p (t b) -> p t b", b=BK),
            in_=dst_dram.rearrange("(t p) b -> p t b", p=128),
        )
        ixlo_sb = cpool.tile([128, NGRP * WL], DT.int16, tag="ixlo")
        nc.sync.dma_start(
            out=ixlo_sb[:].rearrange("p (g w) -> p g w", w=WL),
            in_=idxlo_dram.rearrange("(g p) w -> p g w", p=128),
        )
        ixhi_sb = cpool.tile([128, NGRP * WH], DT.int16, tag="ixhi")
        nc.sync.dma_start(
            out=ixhi_sb[:].rearrange("p (g w) -> p g w", w=WH),
            in_=idxhi_dram.rearrange("(g p) w -> p g w", p=128),
        )
        e16_sb = cpool.tile([128, NT * BK * R], DT.float16, tag="e16")
        hbuf = cpool.tile([128, NT * F], DT.float16, tag="hbuf")

        # ---------------- x0 = emb[x_ids] ----------------
        nc.gpsimd.dma_gather(
            x_sb[:].rearrange("p (b e) -> p b e", e=F),
            emb_dram,
            xids_sb[:],
            SLAB,
            SLAB,
            F,
            single_packet=False,
        )

        # ---------------- helpers ----------------
        def h_chain(iv, i):
            """hbuf[:, tile] = fp16(x_tile @ W1[i] + b1[i])."""
            xcp = wk2.tile([128, F], DT.float32, tag="xcp")
            nc.scalar.copy(xcp[:], x_sb[:, bass.ts(iv, F)])
            xT_ps = ppx.tile([F, 128], DT.float32, tag="xps")
            nc.tensor.transpose(xT_ps[:], xcp[:], id32_sb[:])
            xT_sb = wk2.tile([F + 1, 128], DT.float32, tag="xT")
            nc.scalar.copy(xT_sb[0:F, :], xT_ps[:])
            nc.vector.memset(xT_sb[F:F + 1, :], 1.0)
            h_ps = ppx.tile([128, F], DT.float32, tag="xps2")
            nc.tensor.matmul(h_ps[:], xT_sb[:], w1b_sb[:, i * F:(i + 1) * F],
                             start=True, stop=True)
            nc.scalar.copy(hbuf[:, bass.ts(iv, F)], h_ps[:])

        # h-chunk boundaries (in tiles) for the overlapped AllGather: each
        # chunk is flushed + allgathered as soon as its h tiles are done, so
        # the collective overlaps the remaining edge compute of the conv.
        AGC = sorted({round(NT * k / 4) for k in range(5)})
        if not chunk_ag:
            AGC = [0, NT]

        def flush_chunk(t0, t1):
            """DMA: hbuf tiles [t0,t1) (SBUF) -> h_own rows [:, 0:64]."""
            nc.sync.dma_start(
                out=h_own[:][t0 * 128:t1 * 128, 0:F].rearrange(
                    "(t p) c -> p t c", p=128),
                in_=hbuf[:, t0 * F:t1 * F].rearrange("p (t c) -> p t c", c=F),
            )

        def ag_chunk(dst_t, t0, t1):
            flush_chunk(t0, t1)
            nc.gpsimd.collective_compute(
                "AllGather",
                OP.bypass,
                replica_groups=[list(range(CORES))],
                ins=[h_own[:][t0 * 128:t1 * 128, :].opt()],
                outs=[dst_t[:].rearrange("(c s) f -> c s f", s=SLAB)[
                    :, t0 * 128:t1 * 128, :].opt()],
            )

        def gather_group(gv, hs_lo, hs_hi, h_src):
            nc.gpsimd.dma_gather(
                hs_lo[:].rearrange("p (b e) -> p b e", e=128),
                h_src[:], ixlo_sb[:, bass.ts(gv, WL)],
                GRP * LO_SLOTS, GRP * LO_SLOTS, 128, single_packet=False,
            )
            nc.gpsimd.dma_gather(
                hs_hi[:].rearrange("p (b e) -> p b e", e=128),
                h_src[:][LOCUT:CS, :], ixhi_sb[:, bass.ts(gv, WH)],
                GRP * HI_SLOTS, GRP * HI_SLOTS, 128, single_packet=False,
            )

        def edge_phase(iv, i, tg, hs_lo, hs_hi):
            """Returns aggT psum tile [F, 128] accumulated over the tile."""
            hsl3 = hs_lo[:].rearrange("p (b e) -> p b e", e=128)
            hsh3 = hs_hi[:].rearrange("p (b e) -> p b e", e=128)
            tile_e16 = e16_sb[:, bass.ts(iv, BK * R)]
            # local static-offset copy of this tile's dst ids (cheap; lets
            # the broadcast APs below use raw strides)
            dst_sb = wk.tile([128, BK], DT.float16, tag="dst")
            nc.gpsimd.tensor_copy(dst_sb[:], dst16_sb[:, bass.ts(iv, BK)])

            aggT_ps = pp.tile([F, 128], DT.float32, tag="aggT")

            # f / m / S / scatter in groups of FG chunks; per group one
            # transpose of e16 cols -> eT [fn*11, 128], one block-diag matmul
            for g0 in range(0, BK, FG):
                fn = min(FG, BK - g0)
                eT_ps = ppf.tile([FG * R, 128], DT.float16, tag="eTp")
                nc.tensor.transpose(
                    eT_ps[0:fn * R, :],
                    tile_e16[:, g0 * R:(g0 + fn) * R],
                    id16_sb[:],
                )
                eT_sb = wk.tile([FG * R, 128], DT.float16, tag="eTs")
                nc.scalar.copy(eT_sb[0:fn * R, :], eT_ps[0:fn * R, :])
                f_ps = ppf.tile([128, FG * F], DT.float32, tag="fps")
                bd = webbd8_sb if fn == FG else webbdt_sb
                nc.tensor.matmul(
                    f_ps[:, 0:fn * F],
                    eT_sb[0:fn * R, :],
                    bd[:, i * fn * F:(i + 1) * fn * F],
                    start=True, stop=True,
                )
                if mixed_tt:
                    f3 = f_ps[:].rearrange("p (b e) -> p b e", e=F)
                else:
                    f16 = wk.tile([128, FG * F], DT.float16, tag="f16")
                    nc.scalar.copy(f16[0:128, 0:fn * F], f_ps[:, 0:fn * F])
                    f3 = f16[:].rearrange("p (b e) -> p b e", e=F)
                m_sb = wk.tile([128, FG * F], DT.float16, tag="m")
                segs = []
                c0, c1 = g0, g0 + fn
                if c0 < LO_BLKS:
                    segs.append((hsl3, tg * LO_BLKS + c0, c0,
                                 min(c1, LO_BLKS) - c0))
                if c1 > LO_BLKS:
                    cc0 = max(c0, LO_BLKS)
                    segs.append((hsh3, tg * HI_BLKS + (cc0 - LO_BLKS),
                                 cc0, c1 - cc0))
                for (src3, b0, coff, n) in segs:
                    nc.vector.tensor_tensor(
                        m_sb[:].rearrange("p (b e) -> p b e", e=F)[
                            :, coff - g0:coff - g0 + n, :],
                        src3[:, b0:b0 + n, 0:F],
                        f3[:, coff - g0:coff - g0 + n, :],
                        OP.mult,
                    )
                S_sb = wk.tile([128, FG * 128], DT.float16, tag="S")
                dst_b = bass.AP(
                    dst_sb.tensor,
                    dst_sb[:, g0:g0 + fn].offset,
                    [dst_sb[:].ap[0], [1, fn], [0, 128]],
                )
                iota_b = bass.AP(
                    iota_sb.tensor, iota_sb[:].offset,
                    [iota_sb[:].ap[0], [0, fn], [1, 128]],
                )
                nc.vector.tensor_tensor(
                    S_sb[:].rearrange("p (b e) -> p b e", e=128)[:, 0:fn, :],
                    dst_b, iota_b, OP.is_equal,
                )
                for c in range(fn):
                    cg = g0 + c
                    nc.tensor.matmul(
                        aggT_ps[:],
                        m_sb[:, c * F:(c + 1) * F],
                        S_sb[:, c * 128:(c + 1) * 128],
                        start=(cg == 0), stop=(cg == BK - 1),
                    )
            return aggT_ps

        def x_accum(iv, i, tg, aggT_ps, xs_g):
            """xs_g[:, tg*F:] = x + agg @ W2 + b2 (pre-softplus)."""
            aggT_sb = wk2.tile([F + 1, 128], DT.float32, tag="aggTs")
            nc.scalar.copy(aggT_sb[0:F, :], aggT_ps[:])
            nc.vector.memset(aggT_sb[F:F + 1, :], 1.0)
            xup_ps = ppx.tile([128, F], DT.float32, tag="xps2")
            nc.tensor.matmul(xup_ps[:], aggT_sb[:],
                             w2b_sb[:, i * F:(i + 1) * F],
                             start=True, stop=True)
            nc.vector.tensor_tensor(xs_g[:, tg * F:(tg + 1) * F], xup_ps[:],
                                    x_sb[:, bass.ts(iv, F)], OP.add)

        def softplus_group(gv, xs_g):
            """x_sb[group] = relu(xs) + ln1p(exp(-|xs|)), poly ln1p on Pool.

            Uses only Abs/Exp activations so one act-func table set serves
            the whole program (no per-tile table reloads).
            """
            n = GRP * F
            ab = wk2.tile([128, n], DT.float32, tag="ab")
            nc.scalar.activation(ab[:], xs_g[:], AF.Abs)
            nc.scalar.activation(ab[:], ab[:], AF.Exp, scale=-1.0)
            acc = wk2.tile([128, n], DT.float32, tag="acc")
            nc.vector.tensor_scalar(acc[:], ab[:], LN1P_C[4], LN1P_C[3],
                                    OP.mult, OP.add)
            for ck in (LN1P_C[2], LN1P_C[1], LN1P_C[0]):
                nc.vector.tensor_tensor(acc[:], acc[:], ab[:], OP.mult)
                nc.vector.tensor_scalar_add(acc[:], acc[:], ck)
            nc.vector.tensor_tensor(acc[:], acc[:], ab[:], OP.mult)
            nc.vector.scalar_tensor_tensor(
                x_sb[:, bass.ts(gv, GRP * F)], xs_g[:], 0.0, acc[:],
                OP.max, OP.add)

        # ---------------- prologue: e16 (RBF) + h0 ----------------
        nc.vector.memset(e16_sb[:], 1.0)  # aug ones cols; exp fills the rest

        def body_pro(iv):
            d_sb = wk.tile([128, BK], DT.float32, tag="d")
            nc.sync.dma_start(
                out=d_sb[:],
                in_=d_dram.rearrange("(t p) b -> t p b", p=128)[
                    bass.ds(iv, 1)][0],
            )
            e32 = wk.tile([128, BK * NRBF], DT.float32, tag="e32")
            d_b = bass.AP(d_sb.tensor, d_sb[:].offset,
                          [d_sb[:].ap[0], d_sb[:].ap[1], [0, NRBF]])
            offs_b = bass.AP(offs_sb.tensor, offs_sb[:].offset,
                             [offs_sb[:].ap[0], [0, BK], offs_sb[:].ap[1]])
            e32_3 = e32[:].rearrange("p (b r) -> p b r", r=NRBF)
            nc.vector.tensor_tensor(e32_3, d_b, offs_b, OP.subtract)
            nc.vector.tensor_tensor(e32[:], e32[:], e32[:], OP.mult)
            e16_t = e16_sb[:, bass.ts(iv, BK * R)].rearrange(
                "p (b r) -> p b r", r=R)
            nc.scalar.activation(e16_t[:, :, 0:NRBF], e32_3, AF.Exp,
                                 scale=coeff)
            h_chain(iv, 0)

        # prologue tile loop with chunked h0 allgathers into h_bufs[0]
        ag_i = 0
        for t in range(NT):
            body_pro(t)
            while ag_i + 1 < len(AGC) and t + 1 == AGC[ag_i + 1]:
                ag_chunk(h_bufs[0], AGC[ag_i], AGC[ag_i + 1])
                ag_i += 1
        stopped = stop_after in ("h0", "ag0")
        if debug:
            nc.sync.dma_start(out=dbg["x0"], in_=x_sb[:])
            nc.sync.dma_start(out=dbg["hall0"], in_=h_all_t[:])

        conv_c = 0  # running conv index for h_all ping-pong
        for rep in range(repeat):
            if stopped:
                break
            for i in range(NCONV):
                if stopped or (stop_after is not None
                               and stop_after.startswith("conv")
                               and i > int(stop_after[4:])):
                    stopped = True
                    break
                last = (rep == repeat - 1) and (i == NCONV - 1)
                rbuf = h_bufs[conv_c % 2]
                wbuf = h_bufs[(conv_c + 1) % 2]

                def body_conv_group(gv, i=i, last=last, rbuf=rbuf):
                    hs_lo = gp.tile([128, GRP * LO_SLOTS], DT.float16,
                                    tag="hslo")
                    hs_hi = gp.tile([128, GRP * HI_SLOTS], DT.float16,
                                    tag="hshi")
                    gather_group(gv, hs_lo, hs_hi, rbuf)
                    xs_g = gp.tile([128, GRP * F], DT.float32, tag="xsg")
                    for tg in range(GRP):
                        iv = gv * GRP + tg
                        aggT_ps = edge_phase(iv, i, tg, hs_lo, hs_hi)
                        x_accum(iv, i, tg, aggT_ps, xs_g)
                    softplus_group(gv, xs_g)
                    if not last:
                        for tg in range(GRP):
                            h_chain(gv * GRP + tg, (i + 1) % NCONV)

                ag_i = 0
                for g in range(NGRP):
                    body_conv_group(g)
                    if not last:
                        # issue each h chunk's allgather as soon as its
                        # tiles' h_chain is done (overlaps edge compute)
                        while (ag_i + 1 < len(AGC)
                               and (g + 1) * GRP >= AGC[ag_i + 1]):
                            if not skip_conv_ag:  # timing-probe mode
                                ag_chunk(wbuf, AGC[ag_i], AGC[ag_i + 1])
                            ag_i += 1
                if stop_after == f"conv{i}" and not last:
                    stopped = True
                    break
                if debug and rep == repeat - 1:
                    nc.sync.dma_start(out=dbg[f"x{i+1}"], in_=x_sb[:])
                conv_c += 1

        # ---------------- pooling ----------------
        if stopped:
            zz = wk.tile([1, gpc], DT.float32, tag="zz")
            nc.vector.memset(zz[:], 0.0)
            nc.sync.dma_start(out=obg_dram.rearrange("g one -> one g")[0:1, :],
                              in_=zz[:])
            nc.sync.dma_start(out=oeh_dram.rearrange("g one -> one g")[0:1, :],
                              in_=zz[:])
            conv_stk.close()
            do_rest = False
        else:
            conv_stk.close()
            do_rest = True
        if do_rest:
            ppm = stk.enter_context(tc.tile_pool(name="ppm", bufs=1,
                                                 space="PSUM"))
            poolT_ps = ppm.tile([F, gpc], DT.float32, tag="poolT")

            def body_pool(iv):
                gid_sb = wk.tile([128, 1], DT.float16, tag="gid")
                nc.sync.dma_start(
                    out=gid_sb[:],
                    in_=gid_dram.rearrange("(t p) b -> t p b", p=128)[
                        bass.ds(iv, 1)][0],
                )
                gid32 = wk.tile([128, 1], DT.float32, tag="gid32")
                nc.scalar.copy(gid32[:], gid_sb[:])
                Sp = wk.tile([128, gpc], DT.float32, tag="Sp")
                gid_b = bass.AP(gid32.tensor, gid32[:].offset,
                                [gid32[:].ap[0], [0, gpc]])
                nc.vector.tensor_tensor(Sp[:], gid_b, iotag_sb[:], OP.is_equal)
                nc.tensor.matmul(poolT_ps[:], x_sb[:, iv * F:(iv + 1) * F],
                                 Sp[:], start=(iv == 0), stop=(iv == NT - 1))

            for t in range(NT):
                body_pool(t)

            # mean + MLP (feature-major: cT = relu(Ws.T @ pooled + bs))
            poolT_sb = wk.tile([F, gpc], DT.float32, tag="poolTs")
            nc.vector.tensor_tensor(poolT_sb[:], poolT_ps[:], invc_sb[:], OP.mult)
            cT_ps = ppm.tile([2 * F, gpc], DT.float32, tag="cT")
            nc.tensor.matmul(cT_ps[:], ws_sb[:], poolT_sb[:], start=True,
                             stop=True)
            cT_sb = wk.tile([2 * F, gpc], DT.float32, tag="cTs")
            nc.scalar.activation(cT_sb[:], cT_ps[:], AF.Relu, bias=bs_sb[:])
            if debug:
                nc.sync.dma_start(out=dbg["poolT"], in_=poolT_sb[:])
                nc.sync.dma_start(out=dbg["cT"], in_=cT_sb[:])

            for (w1s, b1s, w2s, b2s, out_dram, tg) in (
                (wbg1_sb, bbg1_sb, wbg2_sb, bbg2_sb, obg_dram, "bg"),
                (weh1_sb, beh1_sb, weh2_sb, beh2_sb, oeh_dram, "eh"),
            ):
                t1_ps = ppm.tile([F, gpc], DT.float32, tag="t1")
                nc.tensor.matmul(t1_ps[:], w1s[:], cT_sb[:], start=True, stop=True)
                t1_sb = wk.tile([F, gpc], DT.float32, tag="t1s" + tg)
                nc.scalar.activation(t1_sb[:], t1_ps[:], AF.Relu, bias=b1s[:])
                o_ps = ppm.tile([1, gpc], DT.float32, tag="o")
                nc.tensor.matmul(o_ps[:], w2s[:], t1_sb[:], start=True, stop=True)
                o_sb = wk.tile([1, gpc], DT.float32, tag="os" + tg)
                nc.scalar.activation(o_sb[:], o_ps[:], AF.Identity, bias=b2s[:])
                nc.sync.dma_start(out=out_dram.rearrange("g one -> one g")[0:1, :],
                                  in_=o_sb[:])

    nc.compile()
    return nc


def make_in_maps(prep):
    """Per-core input dicts for run_bass_kernel_spmd."""
    NT, BK, SLAB, gpc = prep.NT, prep.BK, prep.SLAB, prep.gpc
    LO_SLOTS, HI_SLOTS = prep.LO_BLKS * 128, prep.HI_BLKS * 128
    maps = []
    iota128 = np.tile(np.arange(128, dtype=np.float16)[None, :], (128, 1))
    iotag = np.tile(np.arange(gpc, dtype=np.float32)[None, :], (128, 1))
    id16 = np.eye(128, dtype=np.float16)
    id32 = np.eye(128, dtype=np.float32)
    offs = np.tile(prep.offs[None, :], (128, 1)).astype(np.float32)

    def rep16(a):  # [16, S] -> [128, S] replicated
        return np.tile(a, (8, 1))

    R = NRBF + 1
    FG = 8
    TFG = BK % FG if BK % FG else FG

    def make_bd(fn):
        bd = np.zeros((fn * R, NCONV * fn * F), np.float16)
        for i in range(NCONV):
            for c in range(fn):
                bd[c * R:(c + 1) * R, i * fn * F + c * F:i * fn * F +
                   (c + 1) * F] = prep.Web[i]
        return bd

    webbd8 = make_bd(FG)
    webbdt = make_bd(TFG)

    for k in range(CORES):
        m = dict(
            d_edges=prep.d_arr[k].reshape(NT * 128, BK),
            dst_edges=prep.dst_arr[k].reshape(NT * 128, BK),
            idxlo=np.ascontiguousarray(
                np.tile(prep.idxlo_g[k], (1, 8, 1)).reshape(
                    prep.NGRP * 128, prep.GRP * LO_SLOTS // 16)),
            idxhi=np.ascontiguousarray(
                np.tile(prep.idxhi_g[k], (1, 8, 1)).reshape(
                    prep.NGRP * 128, prep.GRP * HI_SLOTS // 16)),
            gid=prep.gid_slab[k].reshape(NT * 128, 1).astype(np.float16),
            xids=rep16(prep.xids_w[k]),
            emb95=prep.emb,
            w1b=np.ascontiguousarray(
                prep.W1b.transpose(1, 0, 2).reshape(F + 1, NCONV * F)),
            w2b=np.ascontiguousarray(
                prep.W2b.transpose(1, 0, 2).reshape(F + 1, NCONV * F)),
            webbd8=webbd8,
            webbdt=webbdt,
            offs=offs,
            iota128=iota128,
            iotag=iotag,
            ident16=id16,
            ident32=id32,
            invc=np.tile(prep.inv_cnt[k][None, :], (F, 1)).astype(np.float32),
            ws=prep.Ws,
            bs=prep.bs.reshape(2 * F, 1),
            wbg1=prep.Wbg1,
            bbg1=prep.bbg1.reshape(F, 1),
            wbg2=prep.Wbg2,
            bbg2=prep.bbg2.reshape(1, 1),
            weh1=prep.Weh1,
            beh1=prep.beh1.reshape(F, 1),
            weh2=prep.Weh2,
            beh2=prep.beh2.reshape(1, 1),
        )
        maps.append({k2: np.ascontiguousarray(v) for k2, v in m.items()})
    return maps


def kernel(**inputs):
    import numpy as np
    from concourse.bass_utils import run_bass_kernel_spmd

    wkeys = ("emb blk_W1 blk_b1 blk_We blk_be blk_W2 blk_b2 Ws bs Wbg1 bbg1 "
             "Wbg2 bbg2 Weh1 beh1 Weh2 beh2").split()
    weights = {k: np.asarray(inputs[k]) for k in wkeys}
    p = Prep(np.asarray(inputs["x_ids"]), np.asarray(inputs["edge_index"]),
             np.asarray(inputs["edge_attr"]), np.asarray(inputs["batch"]),
             weights, n_graphs=512)
    nc = build_bass(p, unroll=25)
    maps = make_in_maps(p)
    res = run_bass_kernel_spmd(nc, maps, list(range(CORES)))
    bg = np.concatenate([np.asarray(res.results[k]["obg"], dtype=np.float32)
                         for k in range(CORES)])
    eh = np.concatenate([np.asarray(res.results[k]["oeh"], dtype=np.float32)
                         for k in range(CORES)])
    return bg, eh



# revision 40
# speedup vs baseline: 1.3150x; 1.0218x over previous
"""CrystalGNN (SchNet-style) Trainium2 Bass kernel — self-contained.

Sharding: nodes/graphs block-partitioned across 8 NeuronCores (graph-aligned
slabs); edges partitioned by owner(dst) and grouped by 128-node dst tile;
small weights replicated. Per conv: h = x@W1+b1 (fp16, slab-local) ->
AllGather -> per-tile edge pipeline (dma_gather h[src], RBF+block-diag Web
matmul for f, m = h*f, one-hot scatter matmul into PSUM) -> x update +
softplus. Mean-pool via one-hot matmul + small MLP heads on-device.
"""
"""Host-side sharding/preprocessing + numpy device-model for the CrystalGNN kernel.

Everything is parameterized by the problem dims so the same code paths can be
exercised at a small scale in CoreSim and at full scale on hardware.
"""
import numpy as np

F = 64          # atom feats
NRBF = 10
NCONV = 3
H = 64
CORES = 8


def ceil_div(a, b):
    return (a + b - 1) // b


def round_up(a, b):
    return ceil_div(a, b) * b


class Prep:
    """Per-problem host preprocessing. All outputs are numpy arrays keyed for
    the bass kernel's DRAM tensors (one dict per core)."""

    def __init__(self, x_ids, edge_index, edge_attr, batch, weights, n_graphs,
                 locut=None):
        N = x_ids.shape[0]
        E = edge_index.shape[1]
        G = n_graphs
        assert G % CORES == 0
        gpc = G // CORES  # graphs per core
        self.N, self.E, self.G, self.gpc = N, E, G, gpc

        batch = np.asarray(batch).astype(np.int64)
        x_ids = np.asarray(x_ids).astype(np.int64)
        src = np.asarray(edge_index[0]).astype(np.int64)
        dst = np.asarray(edge_index[1]).astype(np.int64)
        d = np.asarray(edge_attr).astype(np.float32)

        # graph -> node range (batch is sorted)
        gstart = np.searchsorted(batch, np.arange(G), side="left")
        gend = np.searchsorted(batch, np.arange(G), side="right")
        # core k owns graphs [k*gpc, (k+1)*gpc) -> nodes [cstart[k], cend[k])
        cstart = gstart[np.arange(CORES) * gpc]
        cend = np.append(cstart[1:], N)
        own = cend - cstart
        max_own = int(own.max())
        # slab size: per-core node capacity, multiple of 128
        SLAB = round_up(max_own, 128)
        NT = SLAB // 128  # node tiles per core
        self.SLAB, self.NT = SLAB, NT
        self.cstart, self.cend = cstart, cend

        # slab row of each global node
        owner = np.searchsorted(cstart, np.arange(N), side="right") - 1

        # int16 split point for gather indices (slab rows)
        self.LOCUT = min(32768, CORES * SLAB) if locut is None else locut

        # ---- balance dst-degree across each core's 128-node tiles: greedy
        # LPT assignment of nodes to tiles minimizing per-tile
        # ceil(lo/128)+ceil(hi/128), which sets BK (the padded edge-slot
        # budget every edge-phase cost scales with).
        srow_old = SLAB * owner + (np.arange(N) - cstart[owner])
        est_lo = srow_old[src] < self.LOCUT
        deg_lo = np.zeros(N, np.int64)
        deg_hi = np.zeros(N, np.int64)
        np.add.at(deg_lo, dst[est_lo], 1)
        np.add.at(deg_hi, dst[~est_lo], 1)
        new_local = np.empty(N, np.int64)
        for k in range(CORES):
            nodes = np.arange(cstart[k], cend[k])
            # balance hi tightly (its block budget has usable slack; lo's
            # does not), keep lo under its existing block budget
            order_n = nodes[np.argsort(
                -(deg_hi[nodes] * 10000 + deg_lo[nodes]), kind="stable")]
            tl = np.zeros(NT, np.int64)
            th = np.zeros(NT, np.int64)
            tc = np.zeros(NT, np.int64)
            lo_cap = 128 * (-(-(deg_lo[nodes].sum()) // (128 * NT)) + 1)
            for n in order_n:
                dl, dh = deg_lo[n], deg_hi[n]
                cost = (th + dh) * 10**6 + (tl + dl)
                cost[(tc >= 128) | (tl + dl > lo_cap)] = 2**62
                t = int(np.argmin(cost))
                new_local[n] = t * 128 + tc[t]
                tl[t] += dl
                th[t] += dh
                tc[t] += 1
        srow = SLAB * owner + new_local
        self.owner, self.srow = owner, srow

        # ---- edge partition: edge belongs to owner[dst], tile = local dst block
        e_owner = owner[dst]
        e_tile = new_local[dst] // 128
        e_dstloc = new_local[dst] % 128                    # local id within tile
        e_srow = srow[src]
        e_lo = e_srow < self.LOCUT

        # per (core, tile): count lo/hi edges
        # order edges by (core, tile, hi?, arbitrary)
        key = ((e_owner * NT + e_tile) * 2 + (~e_lo).astype(np.int64))
        order = np.argsort(key, kind="stable")
        s_core = e_owner[order]
        s_tile = e_tile[order]
        s_lo = e_lo[order]
        s_d = d[order]
        s_dstloc = e_dstloc[order]
        s_srow = e_srow[order]

        # counts
        n_lo = np.zeros((CORES, NT), np.int64)
        n_hi = np.zeros((CORES, NT), np.int64)
        np.add.at(n_lo, (e_owner[e_lo], e_tile[e_lo]), 1)
        np.add.at(n_hi, (e_owner[~e_lo], e_tile[~e_lo]), 1)
        LO_BLKS = int(ceil_div(n_lo.max(), 128))
        HI_BLKS = int(ceil_div(n_hi.max(), 128))
        BK = LO_BLKS + HI_BLKS
        self.LO_BLKS, self.HI_BLKS, self.BK = LO_BLKS, HI_BLKS, BK
        self.n_lo, self.n_hi = n_lo, n_hi

        LO_SLOTS = LO_BLKS * 128
        HI_SLOTS = HI_BLKS * 128
        SLOTS = BK * 128

        # ---- per-core packed arrays
        # slot s of (core,tile): s in [0, LO_SLOTS) lo edges; [LO_SLOTS, SLOTS) hi
        d_arr = np.zeros((CORES, NT, 128, BK), np.float32)
        dst_arr = np.full((CORES, NT, 128, BK), -1.0, np.float16)
        idxlo = np.zeros((CORES, NT, LO_SLOTS), np.int16)
        idxhi = np.zeros((CORES, NT, HI_SLOTS), np.int16)

        # fill using the sorted stream
        # positions within each (core,tile,lo/hi) group
        grp_key = (s_core * NT + s_tile) * 2 + (~s_lo).astype(np.int64)
        # index within group
        uniq, first_idx = np.unique(grp_key, return_index=True)
        pos_in_grp = np.arange(len(grp_key)) - np.repeat(
            first_idx, np.diff(np.append(first_idx, len(grp_key)))
        )
        slot = np.where(s_lo, pos_in_grp, LO_SLOTS + pos_in_grp)
        p = slot % 128
        b = slot // 128
        d_arr[s_core, s_tile, p, b] = s_d
        dst_arr[s_core, s_tile, p, b] = s_dstloc.astype(np.float16)
        lo_m = s_lo
        idxlo[s_core[lo_m], s_tile[lo_m], pos_in_grp[lo_m]] = s_srow[lo_m].astype(
            np.int16
        )
        hi_m = ~s_lo
        idxhi[s_core[hi_m], s_tile[hi_m], pos_in_grp[hi_m]] = (
            s_srow[hi_m] - self.LOCUT
        ).astype(np.int16)

        # pad gather idx: everything stays -1 after the real edges (trailing skip).
        # counts per tile (rounded: the dma consumes them via num_idxs_reg)
        cnts = np.stack([n_lo, n_hi], axis=-1).astype(np.int32)  # [CORES, NT, 2]

        # wrap idx arrays to the [16, n/16] layout: position i -> [i % 16, i // 16]
        def wrap16(a):  # [..., S] -> [..., 16, S//16]
            S = a.shape[-1]
            return np.ascontiguousarray(
                a.reshape(*a.shape[:-1], S // 16, 16).swapaxes(-1, -2)
            )

        GRP = 2 if NT % 2 == 0 else 1
        self.GRP, self.NGRP = GRP, NT // GRP
        self.d_arr = d_arr
        self.dst_arr = dst_arr
        self.idxlo_w = wrap16(idxlo)
        self.idxhi_w = wrap16(idxhi)
        self.idxlo_g = wrap16(idxlo.reshape(CORES, self.NGRP, GRP * LO_SLOTS))
        self.idxhi_g = wrap16(idxhi.reshape(CORES, self.NGRP, GRP * HI_SLOTS))
        self.cnts = cnts

        # ---- node-side per-core tables
        # x_ids slab: [CORES, SLAB] (pad -> 0)
        xids_slab = np.zeros((CORES, SLAB), np.int64)
        gid_slab = np.full((CORES, SLAB), -1.0, np.float16)  # local graph id
        for k in range(CORES):
            nodes = np.arange(cstart[k], cend[k])
            xids_slab[k, new_local[nodes]] = x_ids[nodes]
            gid_slab[k, new_local[nodes]] = (
                batch[nodes] - k * gpc).astype(np.float16)
        self.xids_w = wrap16(xids_slab.astype(np.int16))  # emb table < 32768 rows
        self.gid_slab = gid_slab.reshape(CORES, NT, 128)

        # inverse counts per graph (local)
        cnt_g = np.zeros((CORES, gpc), np.float32)
        for k in range(CORES):
            ids, c = np.unique(
                (batch[cstart[k]:cend[k]] - k * gpc), return_counts=True
            )
            cnt_g[k, ids] = c
        self.inv_cnt = (1.0 / np.maximum(cnt_g, 1.0)).astype(np.float32)  # [CORES,gpc]

        # ---- weights (augmented)
        w = weights
        self.W1b = np.concatenate(
            [w["blk_W1"], w["blk_b1"][:, None, :]], axis=1
        ).astype(np.float32)  # [NCONV, F+1, F]
        self.W2b = np.concatenate(
            [w["blk_W2"], w["blk_b2"][:, None, :]], axis=1
        ).astype(np.float32)
        self.Web = np.concatenate(
            [w["blk_We"], w["blk_be"][:, None, :]], axis=1
        ).astype(np.float16)  # [NCONV, NRBF+1, F]
        self.emb = np.asarray(w["emb"]).astype(np.float32)
        for nm in ("Ws", "bs", "Wbg1", "bbg1", "Wbg2", "bbg2",
                   "Weh1", "beh1", "Weh2", "beh2"):
            setattr(self, nm, np.asarray(w[nm]).astype(np.float32))

        # RBF offsets
        offs = np.linspace(0.0, 6.0, NRBF).astype(np.float32)
        self.offs = offs
        self.coeff = np.float32(-0.5 / (offs[1] - offs[0]) ** 2)



"""Bass/Tile kernel builder for the CrystalGNN (SchNet-style) message-passing net.

Data layout (per core, SPMD identical program):
  - nodes sharded by graph: core k owns graphs [k*gpc,(k+1)*gpc) -> a slab of
    SLAB node rows (NT = SLAB/128 tiles of 128 nodes).
  - x state [128, NT*F] f32 lives in SBUF for the whole kernel.
  - h table (x @ W1 + b1, fp16, padded to 128 cols) is written per-slab to DRAM
    and AllGather'd so every core can dma_gather rows of any node.
  - edges partitioned by owner(dst), grouped by dst tile; per tile a fixed
    budget of BK*128 edge slots (LO_BLKS lo-src + HI_BLKS hi-src blocks,
    src slab-row < / >= LOCUT for int16 gather indices).
  - per tile: gather h[src]; RBF e from distances; f = e_aug @ Web (PE);
    m = h*f (DVE, fp16); one-hot S from dst ids (DVE is_equal);
    aggT[64,128] += m.T @ S (PE, PSUM f32); x += aggT.T@W2+b2; softplus.
  - pool: pooledT[F,gpc] += x_tile.T @ onehot(graph); * 1/cnt; 3-layer MLP.
"""
import numpy as np
from contextlib import ExitStack

import concourse.bass as bass
import concourse.bacc as bacc
import concourse.mybir as mybir
from concourse import tile

F = 64
NRBF = 10
NCONV = 3
CORES = 8
AF = mybir.ActivationFunctionType
OP = mybir.AluOpType
DT = mybir.dt


# ln(1+t) ~= t*(P0 + P1 t + P2 t^2 + P3 t^3 + P4 t^4), t in [0,1]
# (max abs err 8.1e-5; exact 0 at t=0). Lets softplus avoid the Ln
# activation so a single act-func table serves the whole program.
LN1P_C = (0.99988793, -0.49636828, 0.30467236, -0.15602843, 0.04106451)


def build_bass(prep, unroll=10, debug=False, stop_after=None, py_loops=False,
               repeat=1, ag_strided=True, mixed_tt=True, chunk_ag=False,
               skip_conv_ag=False):
    """Returns (nc, input_names) — the SPMD program for all cores."""
    NT, BK = prep.NT, prep.BK
    LO_BLKS, HI_BLKS = prep.LO_BLKS, prep.HI_BLKS
    LO_SLOTS, HI_SLOTS = LO_BLKS * 128, HI_BLKS * 128
    SLAB, gpc, LOCUT = prep.SLAB, prep.gpc, prep.LOCUT
    GRP, NGRP = prep.GRP, prep.NGRP
    CS = CORES * SLAB
    coeff = float(prep.coeff)
    R = NRBF + 1
    WL = GRP * LO_SLOTS // 16
    WH = GRP * HI_SLOTS // 16

    nc = bacc.Bacc("TRN2", target_bir_lowering=False, debug=False,
                   num_devices=CORES)

    # ---------------- DRAM inputs ----------------
    def din(name, shape, dt):
        return nc.dram_tensor(name, list(shape), dt, kind="ExternalInput").ap()

    d_dram = din("d_edges", (NT * 128, BK), DT.float32)
    dst_dram = din("dst_edges", (NT * 128, BK), DT.float16)
    idxlo_dram = din("idxlo", (NGRP * 128, GRP * LO_SLOTS // 16), DT.int16)
    idxhi_dram = din("idxhi", (NGRP * 128, GRP * HI_SLOTS // 16), DT.int16)
    gid_dram = din("gid", (NT * 128, 1), DT.float16)
    xids_dram = din("xids", (128, SLAB // 16), DT.int16)
    emb_dram = din("emb95", (95, F), DT.float32)
    w1b_dram = din("w1b", (F + 1, NCONV * F), DT.float32)
    w2b_dram = din("w2b", (F + 1, NCONV * F), DT.float32)
    # block-diagonal Web for grouped f-matmuls: groups of FG=8 chunks (and a
    # tail group of BK%8 chunks). webbd8[(c,k),(c',j)] = Web[k,j] * (c==c')
    FG = 8
    TFG = BK % FG if BK % FG else FG  # tail group size
    webbd8_dram = din("webbd8", (FG * (NRBF + 1), NCONV * FG * F), DT.float16)
    webbdt_dram = din("webbdt", (TFG * (NRBF + 1), NCONV * TFG * F),
                      DT.float16)
    offs_dram = din("offs", (128, NRBF), DT.float32)
    iota_dram = din("iota128", (128, 128), DT.float16)
    iotag_dram = din("iotag", (128, gpc), DT.float32)
    ident16_dram = din("ident16", (128, 128), DT.float16)
    ident32_dram = din("ident32", (128, 128), DT.float32)
    invc_dram = din("invc", (F, gpc), DT.float32)
    ws_dram = din("ws", (F, 2 * F), DT.float32)
    bs_dram = din("bs", (2 * F, 1), DT.float32)
    wbg1_dram = din("wbg1", (2 * F, F), DT.float32)
    bbg1_dram = din("bbg1", (F, 1), DT.float32)
    wbg2_dram = din("wbg2", (F, 1), DT.float32)
    bbg2_dram = din("bbg2", (1, 1), DT.float32)
    weh1_dram = din("weh1", (2 * F, F), DT.float32)
    beh1_dram = din("beh1", (F, 1), DT.float32)
    weh2_dram = din("weh2", (F, 1), DT.float32)
    beh2_dram = din("beh2", (1, 1), DT.float32)

    dbg = {}
    if debug:
        dbg["x0"] = nc.dram_tensor("dbg_x0", [128, NT * F], DT.float32,
                                   kind="ExternalOutput").ap()
        dbg["hall0"] = nc.dram_tensor("dbg_hall0", [CS, 128], DT.float16,
                                      kind="ExternalOutput").ap()
        for i in range(NCONV):
            dbg[f"x{i+1}"] = nc.dram_tensor(f"dbg_x{i+1}", [128, NT * F],
                                            DT.float32,
                                            kind="ExternalOutput").ap()
        dbg["poolT"] = nc.dram_tensor("dbg_poolT", [F, gpc], DT.float32,
                                      kind="ExternalOutput").ap()
        dbg["cT"] = nc.dram_tensor("dbg_cT", [2 * F, gpc], DT.float32,
                                   kind="ExternalOutput").ap()
    h_all_t = nc.dram_tensor("h_all", [CS, 128], DT.float16,
                             addr_space="Shared")
    h_all2_t = nc.dram_tensor("h_all2", [CS, 128], DT.float16,
                              addr_space="Shared")
    h_bufs = (h_all_t, h_all2_t)
    obg_dram = nc.dram_tensor("obg", [gpc, 1], DT.float32,
                              kind="ExternalOutput").ap()
    oeh_dram = nc.dram_tensor("oeh", [gpc, 1], DT.float32,
                              kind="ExternalOutput").ap()

    with tile.TileContext(nc) as tc, ExitStack() as stk:
        cpool = stk.enter_context(tc.tile_pool(name="const", bufs=1))
        dpool = stk.enter_context(tc.tile_pool(name="dram", bufs=1,
                                               space="DRAM"))
        wk = stk.enter_context(tc.tile_pool(name="wk", bufs=4))
        wk2 = stk.enter_context(tc.tile_pool(name="wk2", bufs=2))
        conv_stk = ExitStack()
        gp = conv_stk.enter_context(tc.tile_pool(name="gp", bufs=3))
        pp = conv_stk.enter_context(tc.tile_pool(name="pp", bufs=2,
                                                 space="PSUM"))
        ppf = conv_stk.enter_context(tc.tile_pool(name="ppf", bufs=2,
                                                  space="PSUM"))
        ppx = conv_stk.enter_context(tc.tile_pool(name="ppx", bufs=1,
                                                  space="PSUM"))

        h_own = dpool.tile([SLAB, 128], DT.float16)

        # ---------------- persistent SBUF ----------------
        def load_const(name, ap_dram, shape, dt):
            t = cpool.tile(list(shape), dt, tag=name)
            nc.sync.dma_start(out=t[:], in_=ap_dram)
            return t

        w1b_sb = load_const("w1b", w1b_dram, (F + 1, NCONV * F), DT.float32)
        w2b_sb = load_const("w2b", w2b_dram, (F + 1, NCONV * F), DT.float32)
        webbd8_sb = load_const("webbd8", webbd8_dram,
                               (FG * (NRBF + 1), NCONV * FG * F), DT.float16)
        webbdt_sb = load_const("webbdt", webbdt_dram,
                               (TFG * (NRBF + 1), NCONV * TFG * F), DT.float16)
        offs_sb = load_const("offs", offs_dram, (128, NRBF), DT.float32)
        iota_sb = load_const("iota", iota_dram, (128, 128), DT.float16)
        iotag_sb = load_const("iotag", iotag_dram, (128, gpc), DT.float32)
        id16_sb = load_const("id16", ident16_dram, (128, 128), DT.float16)
        id32_sb = load_const("id32", ident32_dram, (128, 128), DT.float32)
        invc_sb = load_const("invc", invc_dram, (F, gpc), DT.float32)
        ws_sb = load_const("ws", ws_dram, (F, 2 * F), DT.float32)
        bs_sb = load_const("bs", bs_dram, (2 * F, 1), DT.float32)
        wbg1_sb = load_const("wbg1", wbg1_dram, (2 * F, F), DT.float32)
        bbg1_sb = load_const("bbg1", bbg1_dram, (F, 1), DT.float32)
        wbg2_sb = load_const("wbg2", wbg2_dram, (F, 1), DT.float32)
        bbg2_sb = load_const("bbg2", bbg2_dram, (1, 1), DT.float32)
        weh1_sb = load_const("weh1", weh1_dram, (2 * F, F), DT.float32)
        beh1_sb = load_const("beh1", beh1_dram, (F, 1), DT.float32)
        weh2_sb = load_const("weh2", weh2_dram, (F, 1), DT.float32)
        beh2_sb = load_const("beh2", beh2_dram, (1, 1), DT.float32)
        xids_sb = load_const("xids", xids_dram, (128, SLAB // 16), DT.int16)

        x_sb = cpool.tile([128, NT * F], DT.float32, tag="x")

        # persistent static edge-side state (loaded/computed once, reused
        # across all convs)
        dst16_sb = cpool.tile([128, NT * BK], DT.float16, tag="dst16")
        nc.sync.dma_start(
            out=dst16_sb[:].rearrange("p (t b) -> p t b", b=BK),
            in_=dst_dram.rearrange("(t p) b -> p t b", p=128),
        )
        ixlo_sb = cpool.tile([128, NGRP * WL], DT.int16, tag="ixlo")
        nc.sync.dma_start(
            out=ixlo_sb[:].rearrange("p (g w) -> p g w", w=WL),
            in_=idxlo_dram.rearrange("(g p) w -> p g w", p=128),
        )
        ixhi_sb = cpool.tile([128, NGRP * WH], DT.int16, tag="ixhi")
        nc.sync.dma_start(
            out=ixhi_sb[:].rearrange("p (g w) -> p g w", w=WH),
            in_=idxhi_dram.rearrange("(g p) w -> p g w", p=128),
        )
        e16_sb = cpool.tile([128, NT * BK * R], DT.float16, tag="e16")
        hbuf = cpool.tile([128, NT * F], DT.float16, tag="hbuf")
        # persistent transposed RBF: eT for every (tile, chunk-group),
        # computed once in the prologue (conv-independent)
        NGF = ceil_div(BK, FG)
        eTall = cpool.tile([FG * R, NT * NGF * 128], DT.float16, tag="eTall")

        # ---------------- x0 = emb[x_ids] ----------------
        nc.gpsimd.dma_gather(
            x_sb[:].rearrange("p (b e) -> p b e", e=F),
            emb_dram,
            xids_sb[:],
            SLAB,
            SLAB,
            F,
            single_packet=False,
        )

        # ---------------- helpers ----------------
        def h_chain(iv, i):
            """hbuf[:, tile] = fp16(x_tile @ W1[i] + b1[i])."""
            xcp = wk2.tile([128, F], DT.float32, tag="xcp")
            nc.scalar.copy(xcp[:], x_sb[:, bass.ts(iv, F)])
            xT_ps = ppx.tile([F, 128], DT.float32, tag="xps")
            nc.tensor.transpose(xT_ps[:], xcp[:], id32_sb[:])
            xT_sb = wk2.tile([F + 1, 128], DT.float32, tag="xT")
            nc.scalar.copy(xT_sb[0:F, :], xT_ps[:])
            nc.vector.memset(xT_sb[F:F + 1, :], 1.0)
            h_ps = ppx.tile([128, F], DT.float32, tag="xps2")
            nc.tensor.matmul(h_ps[:], xT_sb[:], w1b_sb[:, i * F:(i + 1) * F],
                             start=True, stop=True)
            nc.scalar.copy(hbuf[:, bass.ts(iv, F)], h_ps[:])

        # h-chunk boundaries (in tiles) for the overlapped AllGather: each
        # chunk is flushed + allgathered as soon as its h tiles are done, so
        # the collective overlaps the remaining edge compute of the conv.
        AGC = sorted({round(NT * k / 4) for k in range(5)})
        if not chunk_ag:
            AGC = [0, NT]

        def flush_chunk(t0, t1):
            """DMA: hbuf tiles [t0,t1) (SBUF) -> h_own rows [:, 0:64]."""
            nc.sync.dma_start(
                out=h_own[:][t0 * 128:t1 * 128, 0:F].rearrange(
                    "(t p) c -> p t c", p=128),
                in_=hbuf[:, t0 * F:t1 * F].rearrange("p (t c) -> p t c", c=F),
            )

        def ag_chunk(dst_t, t0, t1):
            flush_chunk(t0, t1)
            nc.gpsimd.collective_compute(
                "AllGather",
                OP.bypass,
                replica_groups=[list(range(CORES))],
                ins=[h_own[:][t0 * 128:t1 * 128, :].opt()],
                outs=[dst_t[:].rearrange("(c s) f -> c s f", s=SLAB)[
                    :, t0 * 128:t1 * 128, :].opt()],
            )

        def gather_group(gv, hs_lo, hs_hi, h_src):
            nc.gpsimd.dma_gather(
                hs_lo[:].rearrange("p (b e) -> p b e", e=128),
                h_src[:], ixlo_sb[:, bass.ts(gv, WL)],
                GRP * LO_SLOTS, GRP * LO_SLOTS, 128, single_packet=False,
            )
            nc.gpsimd.dma_gather(
                hs_hi[:].rearrange("p (b e) -> p b e", e=128),
                h_src[:][LOCUT:CS, :], ixhi_sb[:, bass.ts(gv, WH)],
                GRP * HI_SLOTS, GRP * HI_SLOTS, 128, single_packet=False,
            )

        def edge_phase(iv, i, tg, hs_lo, hs_hi):
            """Returns aggT psum tile [F, 128] accumulated over the tile."""
            hsl3 = hs_lo[:].rearrange("p (b e) -> p b e", e=128)
            hsh3 = hs_hi[:].rearrange("p (b e) -> p b e", e=128)
            tile_e16 = e16_sb[:, bass.ts(iv, BK * R)]
            # local static-offset copy of this tile's dst ids (cheap; lets
            # the broadcast APs below use raw strides)
            dst_sb = wk.tile([128, BK], DT.float16, tag="dst")
            nc.gpsimd.tensor_copy(dst_sb[:], dst16_sb[:, bass.ts(iv, BK)])

            aggT_ps = pp.tile([F, 128], DT.float32, tag="aggT")

            # f / m / S / scatter in groups of FG chunks; per group one
            # transpose of e16 cols -> eT [fn*11, 128], one block-diag matmul
            eT_tile = eTall[:, bass.ts(iv, NGF * 128)]
            for g0 in range(0, BK, FG):
                fn = min(FG, BK - g0)
                gi = g0 // FG
                f_ps = ppf.tile([128, FG * F], DT.float32, tag="fps")
                bd = webbd8_sb if fn == FG else webbdt_sb
                nc.tensor.matmul(
                    f_ps[:, 0:fn * F],
                    eT_tile[0:fn * R, gi * 128:(gi + 1) * 128],
                    bd[:, i * fn * F:(i + 1) * fn * F],
                    start=True, stop=True,
                )
                if mixed_tt:
                    f3 = f_ps[:].rearrange("p (b e) -> p b e", e=F)
                else:
                    f16 = wk.tile([128, FG * F], DT.float16, tag="f16")
                    nc.scalar.copy(f16[0:128, 0:fn * F], f_ps[:, 0:fn * F])
                    f3 = f16[:].rearrange("p (b e) -> p b e", e=F)
                m_sb = wk.tile([128, FG * F], DT.float16, tag="m")
                segs = []
                c0, c1 = g0, g0 + fn
                if c0 < LO_BLKS:
                    segs.append((hsl3, tg * LO_BLKS + c0, c0,
                                 min(c1, LO_BLKS) - c0))
                if c1 > LO_BLKS:
                    cc0 = max(c0, LO_BLKS)
                    segs.append((hsh3, tg * HI_BLKS + (cc0 - LO_BLKS),
                                 cc0, c1 - cc0))
                for (src3, b0, coff, n) in segs:
                    nc.vector.tensor_tensor(
                        m_sb[:].rearrange("p (b e) -> p b e", e=F)[
                            :, coff - g0:coff - g0 + n, :],
                        src3[:, b0:b0 + n, 0:F],
                        f3[:, coff - g0:coff - g0 + n, :],
                        OP.mult,
                    )
                S_sb = wk.tile([128, FG * 128], DT.float16, tag="S")
                dst_b = bass.AP(
                    dst_sb.tensor,
                    dst_sb[:, g0:g0 + fn].offset,
                    [dst_sb[:].ap[0], [1, fn], [0, 128]],
                )
                iota_b = bass.AP(
                    iota_sb.tensor, iota_sb[:].offset,
                    [iota_sb[:].ap[0], [0, fn], [1, 128]],
                )
                nc.vector.tensor_tensor(
                    S_sb[:].rearrange("p (b e) -> p b e", e=128)[:, 0:fn, :],
                    dst_b, iota_b, OP.is_equal,
                )
                for c in range(fn):
                    cg = g0 + c
                    nc.tensor.matmul(
                        aggT_ps[:],
                        m_sb[:, c * F:(c + 1) * F],
                        S_sb[:, c * 128:(c + 1) * 128],
                        start=(cg == 0), stop=(cg == BK - 1),
                    )
            return aggT_ps

        def x_accum(iv, i, tg, aggT_ps, xs_g):
            """xs_g[:, tg*F:] = x + agg @ W2 + b2 (pre-softplus)."""
            aggT_sb = wk2.tile([F + 1, 128], DT.float32, tag="aggTs")
            nc.scalar.copy(aggT_sb[0:F, :], aggT_ps[:])
            nc.vector.memset(aggT_sb[F:F + 1, :], 1.0)
            xup_ps = ppx.tile([128, F], DT.float32, tag="xps2")
            nc.tensor.matmul(xup_ps[:], aggT_sb[:],
                             w2b_sb[:, i * F:(i + 1) * F],
                             start=True, stop=True)
            nc.vector.tensor_tensor(xs_g[:, tg * F:(tg + 1) * F], xup_ps[:],
                                    x_sb[:, bass.ts(iv, F)], OP.add)

        def softplus_group(gv, xs_g):
            """x_sb[group] = relu(xs) + ln1p(exp(-|xs|)), poly ln1p on Pool.

            Uses only Abs/Exp activations so one act-func table set serves
            the whole program (no per-tile table reloads).
            """
            n = GRP * F
            ab = wk2.tile([128, n], DT.float32, tag="ab")
            nc.scalar.activation(ab[:], xs_g[:], AF.Abs)
            nc.scalar.activation(ab[:], ab[:], AF.Exp, scale=-1.0)
            acc = wk2.tile([128, n], DT.float32, tag="acc")
            nc.vector.tensor_scalar(acc[:], ab[:], LN1P_C[4], LN1P_C[3],
                                    OP.mult, OP.add)
            for ck in (LN1P_C[2], LN1P_C[1], LN1P_C[0]):
                nc.vector.tensor_tensor(acc[:], acc[:], ab[:], OP.mult)
                nc.vector.tensor_scalar_add(acc[:], acc[:], ck)
            nc.vector.tensor_tensor(acc[:], acc[:], ab[:], OP.mult)
            nc.vector.scalar_tensor_tensor(
                x_sb[:, bass.ts(gv, GRP * F)], xs_g[:], 0.0, acc[:],
                OP.max, OP.add)

        # ---------------- prologue: e16 (RBF) + h0 ----------------
        nc.vector.memset(e16_sb[:], 1.0)  # aug ones cols; exp fills the rest

        def body_pro(iv):
            d_sb = wk.tile([128, BK], DT.float32, tag="d")
            nc.sync.dma_start(
                out=d_sb[:],
                in_=d_dram.rearrange("(t p) b -> t p b", p=128)[
                    bass.ds(iv, 1)][0],
            )
            e32 = wk.tile([128, BK * NRBF], DT.float32, tag="e32")
            d_b = bass.AP(d_sb.tensor, d_sb[:].offset,
                          [d_sb[:].ap[0], d_sb[:].ap[1], [0, NRBF]])
            offs_b = bass.AP(offs_sb.tensor, offs_sb[:].offset,
                             [offs_sb[:].ap[0], [0, BK], offs_sb[:].ap[1]])
            e32_3 = e32[:].rearrange("p (b r) -> p b r", r=NRBF)
            nc.vector.tensor_tensor(e32_3, d_b, offs_b, OP.subtract)
            nc.vector.tensor_tensor(e32[:], e32[:], e32[:], OP.mult)
            tile_e16 = e16_sb[:, bass.ts(iv, BK * R)]
            e16_t = tile_e16.rearrange("p (b r) -> p b r", r=R)
            nc.scalar.activation(e16_t[:, :, 0:NRBF], e32_3, AF.Exp,
                                 scale=coeff)
            # transpose each chunk-group's eT once (reused by all convs)
            eT_tile = eTall[:, bass.ts(iv, NGF * 128)]
            for g0 in range(0, BK, FG):
                fn = min(FG, BK - g0)
                gi = g0 // FG
                eT_ps = ppf.tile([FG * R, 128], DT.float16, tag="eTp")
                nc.tensor.transpose(
                    eT_ps[0:fn * R, :],
                    tile_e16[:, g0 * R:(g0 + fn) * R],
                    id16_sb[:],
                )
                nc.scalar.copy(eT_tile[0:fn * R, gi * 128:(gi + 1) * 128],
                               eT_ps[0:fn * R, :])
            h_chain(iv, 0)

        # prologue tile loop with chunked h0 allgathers into h_bufs[0]
        ag_i = 0
        for t in range(NT):
            body_pro(t)
            while ag_i + 1 < len(AGC) and t + 1 == AGC[ag_i + 1]:
                ag_chunk(h_bufs[0], AGC[ag_i], AGC[ag_i + 1])
                ag_i += 1
        stopped = stop_after in ("h0", "ag0")
        if debug:
            nc.sync.dma_start(out=dbg["x0"], in_=x_sb[:])
            nc.sync.dma_start(out=dbg["hall0"], in_=h_all_t[:])

        conv_c = 0  # running conv index for h_all ping-pong
        for rep in range(repeat):
            if stopped:
                break
            for i in range(NCONV):
                if stopped or (stop_after is not None
                               and stop_after.startswith("conv")
                               and i > int(stop_after[4:])):
                    stopped = True
                    break
                last = (rep == repeat - 1) and (i == NCONV - 1)
                rbuf = h_bufs[conv_c % 2]
                wbuf = h_bufs[(conv_c + 1) % 2]

                def body_conv_group(gv, i=i, last=last, rbuf=rbuf):
                    hs_lo = gp.tile([128, GRP * LO_SLOTS], DT.float16,
                                    tag="hslo")
                    hs_hi = gp.tile([128, GRP * HI_SLOTS], DT.float16,
                                    tag="hshi")
                    gather_group(gv, hs_lo, hs_hi, rbuf)
                    xs_g = gp.tile([128, GRP * F], DT.float32, tag="xsg")
                    for tg in range(GRP):
                        iv = gv * GRP + tg
                        aggT_ps = edge_phase(iv, i, tg, hs_lo, hs_hi)
                        x_accum(iv, i, tg, aggT_ps, xs_g)
                    softplus_group(gv, xs_g)
                    if not last:
                        for tg in range(GRP):
                            h_chain(gv * GRP + tg, (i + 1) % NCONV)

                ag_i = 0
                for g in range(NGRP):
                    body_conv_group(g)
                    if not last:
                        # issue each h chunk's allgather as soon as its
                        # tiles' h_chain is done (overlaps edge compute)
                        while (ag_i + 1 < len(AGC)
                               and (g + 1) * GRP >= AGC[ag_i + 1]):
                            if not skip_conv_ag:  # timing-probe mode
                                ag_chunk(wbuf, AGC[ag_i], AGC[ag_i + 1])
                            ag_i += 1
                if stop_after == f"conv{i}" and not last:
                    stopped = True
                    break
                if debug and rep == repeat - 1:
                    nc.sync.dma_start(out=dbg[f"x{i+1}"], in_=x_sb[:])
                conv_c += 1

        # ---------------- pooling ----------------
        if stopped:
            zz = wk.tile([1, gpc], DT.float32, tag="zz")
            nc.vector.memset(zz[:], 0.0)
            nc.sync.dma_start(out=obg_dram.rearrange("g one -> one g")[0:1, :],
                              in_=zz[:])
            nc.sync.dma_start(out=oeh_dram.rearrange("g one -> one g")[0:1, :],
                              in_=zz[:])
            conv_stk.close()
            do_rest = False
        else:
            conv_stk.close()
            do_rest = True
        if do_rest:
            ppm = stk.enter_context(tc.tile_pool(name="ppm", bufs=1,
                                                 space="PSUM"))
            poolT_ps = ppm.tile([F, gpc], DT.float32, tag="poolT")

            def body_pool(iv):
                gid_sb = wk.tile([128, 1], DT.float16, tag="gid")
                nc.sync.dma_start(
                    out=gid_sb[:],
                    in_=gid_dram.rearrange("(t p) b -> t p b", p=128)[
                        bass.ds(iv, 1)][0],
                )
                gid32 = wk.tile([128, 1], DT.float32, tag="gid32")
                nc.scalar.copy(gid32[:], gid_sb[:])
                Sp = wk.tile([128, gpc], DT.float32, tag="Sp")
                gid_b = bass.AP(gid32.tensor, gid32[:].offset,
                                [gid32[:].ap[0], [0, gpc]])
                nc.vector.tensor_tensor(Sp[:], gid_b, iotag_sb[:], OP.is_equal)
                nc.tensor.matmul(poolT_ps[:], x_sb[:, iv * F:(iv + 1) * F],
                                 Sp[:], start=(iv == 0), stop=(iv == NT - 1))

            for t in range(NT):
                body_pool(t)

            # mean + MLP (feature-major: cT = relu(Ws.T @ pooled + bs))
            poolT_sb = wk.tile([F, gpc], DT.float32, tag="poolTs")
            nc.vector.tensor_tensor(poolT_sb[:], poolT_ps[:], invc_sb[:], OP.mult)
            cT_ps = ppm.tile([2 * F, gpc], DT.float32, tag="cT")
            nc.tensor.matmul(cT_ps[:], ws_sb[:], poolT_sb[:], start=True,
                             stop=True)
            cT_sb = wk.tile([2 * F, gpc], DT.float32, tag="cTs")
            nc.scalar.activation(cT_sb[:], cT_ps[:], AF.Relu, bias=bs_sb[:])
            if debug:
                nc.sync.dma_start(out=dbg["poolT"], in_=poolT_sb[:])
                nc.sync.dma_start(out=dbg["cT"], in_=cT_sb[:])

            for (w1s, b1s, w2s, b2s, out_dram, tg) in (
                (wbg1_sb, bbg1_sb, wbg2_sb, bbg2_sb, obg_dram, "bg"),
                (weh1_sb, beh1_sb, weh2_sb, beh2_sb, oeh_dram, "eh"),
            ):
                t1_ps = ppm.tile([F, gpc], DT.float32, tag="t1")
                nc.tensor.matmul(t1_ps[:], w1s[:], cT_sb[:], start=True, stop=True)
                t1_sb = wk.tile([F, gpc], DT.float32, tag="t1s" + tg)
                nc.scalar.activation(t1_sb[:], t1_ps[:], AF.Relu, bias=b1s[:])
                o_ps = ppm.tile([1, gpc], DT.float32, tag="o")
                nc.tensor.matmul(o_ps[:], w2s[:], t1_sb[:], start=True, stop=True)
                o_sb = wk.tile([1, gpc], DT.float32, tag="os" + tg)
                nc.scalar.activation(o_sb[:], o_ps[:], AF.Identity, bias=b2s[:])
                nc.sync.dma_start(out=out_dram.rearrange("g one -> one g")[0:1, :],
                                  in_=o_sb[:])

    nc.compile()
    return nc


def make_in_maps(prep):
    """Per-core input dicts for run_bass_kernel_spmd."""
    NT, BK, SLAB, gpc = prep.NT, prep.BK, prep.SLAB, prep.gpc
    LO_SLOTS, HI_SLOTS = prep.LO_BLKS * 128, prep.HI_BLKS * 128
    maps = []
    iota128 = np.tile(np.arange(128, dtype=np.float16)[None, :], (128, 1))
    iotag = np.tile(np.arange(gpc, dtype=np.float32)[None, :], (128, 1))
    id16 = np.eye(128, dtype=np.float16)
    id32 = np.eye(128, dtype=np.float32)
    offs = np.tile(prep.offs[None, :], (128, 1)).astype(np.float32)

    def rep16(a):  # [16, S] -> [128, S] replicated
        return np.tile(a, (8, 1))

    R = NRBF + 1
    FG = 8
    TFG = BK % FG if BK % FG else FG

    def make_bd(fn):
        bd = np.zeros((fn * R, NCONV * fn * F), np.float16)
        for i in range(NCONV):
            for c in range(fn):
                bd[c * R:(c + 1) * R, i * fn * F + c * F:i * fn * F +
                   (c + 1) * F] = prep.Web[i]
        return bd

    webbd8 = make_bd(FG)
    webbdt = make_bd(TFG)

    for k in range(CORES):
        m = dict(
            d_edges=prep.d_arr[k].reshape(NT * 128, BK),
            dst_edges=prep.dst_arr[k].reshape(NT * 128, BK),
            idxlo=np.ascontiguousarray(
                np.tile(prep.idxlo_g[k], (1, 8, 1)).reshape(
                    prep.NGRP * 128, prep.GRP * LO_SLOTS // 16)),
            idxhi=np.ascontiguousarray(
                np.tile(prep.idxhi_g[k], (1, 8, 1)).reshape(
                    prep.NGRP * 128, prep.GRP * HI_SLOTS // 16)),
            gid=prep.gid_slab[k].reshape(NT * 128, 1).astype(np.float16),
            xids=rep16(prep.xids_w[k]),
            emb95=prep.emb,
            w1b=np.ascontiguousarray(
                prep.W1b.transpose(1, 0, 2).reshape(F + 1, NCONV * F)),
            w2b=np.ascontiguousarray(
                prep.W2b.transpose(1, 0, 2).reshape(F + 1, NCONV * F)),
            webbd8=webbd8,
            webbdt=webbdt,
            offs=offs,
            iota128=iota128,
            iotag=iotag,
            ident16=id16,
            ident32=id32,
            invc=np.tile(prep.inv_cnt[k][None, :], (F, 1)).astype(np.float32),
            ws=prep.Ws,
            bs=prep.bs.reshape(2 * F, 1),
            wbg1=prep.Wbg1,
            bbg1=prep.bbg1.reshape(F, 1),
            wbg2=prep.Wbg2,
            bbg2=prep.bbg2.reshape(1, 1),
            weh1=prep.Weh1,
            beh1=prep.beh1.reshape(F, 1),
            weh2=prep.Weh2,
            beh2=prep.beh2.reshape(1, 1),
        )
        maps.append({k2: np.ascontiguousarray(v) for k2, v in m.items()})
    return maps


def kernel(**inputs):
    import numpy as np
    from concourse.bass_utils import run_bass_kernel_spmd

    wkeys = ("emb blk_W1 blk_b1 blk_We blk_be blk_W2 blk_b2 Ws bs Wbg1 bbg1 "
             "Wbg2 bbg2 Weh1 beh1 Weh2 beh2").split()
    weights = {k: np.asarray(inputs[k]) for k in wkeys}
    p = Prep(np.asarray(inputs["x_ids"]), np.asarray(inputs["edge_index"]),
             np.asarray(inputs["edge_attr"]), np.asarray(inputs["batch"]),
             weights, n_graphs=512)
    nc = build_bass(p, unroll=25)
    maps = make_in_maps(p)
    res = run_bass_kernel_spmd(nc, maps, list(range(CORES)))
    bg = np.concatenate([np.asarray(res.results[k]["obg"], dtype=np.float32)
                         for k in range(CORES)])
    eh = np.concatenate([np.asarray(res.results[k]["oeh"], dtype=np.float32)
                         for k in range(CORES)])
    return bg, eh



# revision 45
# speedup vs baseline: 1.3438x; 1.0219x over previous
"""CrystalGNN (SchNet-style) Trainium2 Bass kernel — self-contained.

Sharding: nodes/graphs block-partitioned across 8 NeuronCores (graph-aligned
slabs); edges partitioned by owner(dst) and grouped by 128-node dst tile;
small weights replicated. Per conv: h = x@W1+b1 (fp16, slab-local) ->
AllGather -> per-tile edge pipeline (dma_gather h[src], RBF+block-diag Web
matmul for f, m = h*f, one-hot scatter matmul into PSUM) -> x update +
softplus. Mean-pool via one-hot matmul + small MLP heads on-device.
"""
"""Host-side sharding/preprocessing + numpy device-model for the CrystalGNN kernel.

Everything is parameterized by the problem dims so the same code paths can be
exercised at a small scale in CoreSim and at full scale on hardware.
"""
import numpy as np

F = 64          # atom feats
NRBF = 10
NCONV = 3
H = 64
CORES = 8


def ceil_div(a, b):
    return (a + b - 1) // b


def round_up(a, b):
    return ceil_div(a, b) * b


class Prep:
    """Per-problem host preprocessing. All outputs are numpy arrays keyed for
    the bass kernel's DRAM tensors (one dict per core)."""

    def __init__(self, x_ids, edge_index, edge_attr, batch, weights, n_graphs,
                 locut=None):
        N = x_ids.shape[0]
        E = edge_index.shape[1]
        G = n_graphs
        assert G % CORES == 0
        gpc = G // CORES  # graphs per core
        self.N, self.E, self.G, self.gpc = N, E, G, gpc

        batch = np.asarray(batch).astype(np.int64)
        x_ids = np.asarray(x_ids).astype(np.int64)
        src = np.asarray(edge_index[0]).astype(np.int64)
        dst = np.asarray(edge_index[1]).astype(np.int64)
        d = np.asarray(edge_attr).astype(np.float32)

        # graph -> node range (batch is sorted)
        gstart = np.searchsorted(batch, np.arange(G), side="left")
        gend = np.searchsorted(batch, np.arange(G), side="right")
        # core k owns graphs [k*gpc, (k+1)*gpc) -> nodes [cstart[k], cend[k])
        cstart = gstart[np.arange(CORES) * gpc]
        cend = np.append(cstart[1:], N)
        own = cend - cstart
        max_own = int(own.max())
        # slab size: per-core node capacity, multiple of 128
        SLAB = round_up(max_own, 128)
        NT = SLAB // 128  # node tiles per core
        self.SLAB, self.NT = SLAB, NT
        self.cstart, self.cend = cstart, cend

        # slab row of each global node
        owner = np.searchsorted(cstart, np.arange(N), side="right") - 1

        # int16 split point for gather indices (slab rows)
        self.LOCUT = min(32768, CORES * SLAB) if locut is None else locut

        # ---- balance dst-degree across each core's 128-node tiles: greedy
        # LPT assignment of nodes to tiles minimizing per-tile
        # ceil(lo/128)+ceil(hi/128), which sets BK (the padded edge-slot
        # budget every edge-phase cost scales with).
        srow_old = SLAB * owner + (np.arange(N) - cstart[owner])
        est_lo = srow_old[src] < self.LOCUT
        deg_lo = np.zeros(N, np.int64)
        deg_hi = np.zeros(N, np.int64)
        np.add.at(deg_lo, dst[est_lo], 1)
        np.add.at(deg_hi, dst[~est_lo], 1)
        new_local = np.empty(N, np.int64)
        for k in range(CORES):
            nodes = np.arange(cstart[k], cend[k])
            # balance hi tightly (its block budget has usable slack; lo's
            # does not), keep lo under its existing block budget
            order_n = nodes[np.argsort(
                -(deg_hi[nodes] * 10000 + deg_lo[nodes]), kind="stable")]
            tl = np.zeros(NT, np.int64)
            th = np.zeros(NT, np.int64)
            tc = np.zeros(NT, np.int64)
            lo_cap = 128 * (-(-(deg_lo[nodes].sum()) // (128 * NT)) + 1)
            for n in order_n:
                dl, dh = deg_lo[n], deg_hi[n]
                cost = (th + dh) * 10**6 + (tl + dl)
                cost[(tc >= 128) | (tl + dl > lo_cap)] = 2**62
                t = int(np.argmin(cost))
                new_local[n] = t * 128 + tc[t]
                tl[t] += dl
                th[t] += dh
                tc[t] += 1
        srow = SLAB * owner + new_local
        self.owner, self.srow = owner, srow

        # ---- edge partition: edge belongs to owner[dst], tile = local dst block
        e_owner = owner[dst]
        e_tile = new_local[dst] // 128
        e_dstloc = new_local[dst] % 128                    # local id within tile
        e_srow = srow[src]
        e_lo = e_srow < self.LOCUT

        # per (core, tile): count lo/hi edges
        # order edges by (core, tile, hi?, arbitrary)
        key = ((e_owner * NT + e_tile) * 2 + (~e_lo).astype(np.int64))
        order = np.argsort(key, kind="stable")
        s_core = e_owner[order]
        s_tile = e_tile[order]
        s_lo = e_lo[order]
        s_d = d[order]
        s_dstloc = e_dstloc[order]
        s_srow = e_srow[order]

        # counts
        n_lo = np.zeros((CORES, NT), np.int64)
        n_hi = np.zeros((CORES, NT), np.int64)
        np.add.at(n_lo, (e_owner[e_lo], e_tile[e_lo]), 1)
        np.add.at(n_hi, (e_owner[~e_lo], e_tile[~e_lo]), 1)
        LO_BLKS = int(ceil_div(n_lo.max(), 128))
        HI_BLKS = int(ceil_div(n_hi.max(), 128))
        BK = LO_BLKS + HI_BLKS
        self.LO_BLKS, self.HI_BLKS, self.BK = LO_BLKS, HI_BLKS, BK
        self.n_lo, self.n_hi = n_lo, n_hi

        LO_SLOTS = LO_BLKS * 128
        HI_SLOTS = HI_BLKS * 128
        SLOTS = BK * 128

        # ---- per-core packed arrays
        # slot s of (core,tile): s in [0, LO_SLOTS) lo edges; [LO_SLOTS, SLOTS) hi
        d_arr = np.zeros((CORES, NT, 128, BK), np.float32)
        dst_arr = np.full((CORES, NT, 128, BK), -1.0, np.float16)
        idxlo = np.zeros((CORES, NT, LO_SLOTS), np.int16)
        idxhi = np.zeros((CORES, NT, HI_SLOTS), np.int16)

        # fill using the sorted stream
        # positions within each (core,tile,lo/hi) group
        grp_key = (s_core * NT + s_tile) * 2 + (~s_lo).astype(np.int64)
        # index within group
        uniq, first_idx = np.unique(grp_key, return_index=True)
        pos_in_grp = np.arange(len(grp_key)) - np.repeat(
            first_idx, np.diff(np.append(first_idx, len(grp_key)))
        )
        slot = np.where(s_lo, pos_in_grp, LO_SLOTS + pos_in_grp)
        p = slot % 128
        b = slot // 128
        d_arr[s_core, s_tile, p, b] = s_d
        dst_arr[s_core, s_tile, p, b] = s_dstloc.astype(np.float16)
        lo_m = s_lo
        idxlo[s_core[lo_m], s_tile[lo_m], pos_in_grp[lo_m]] = s_srow[lo_m].astype(
            np.int16
        )
        hi_m = ~s_lo
        idxhi[s_core[hi_m], s_tile[hi_m], pos_in_grp[hi_m]] = (
            s_srow[hi_m] - self.LOCUT
        ).astype(np.int16)

        # pad gather idx: everything stays -1 after the real edges (trailing skip).
        # counts per tile (rounded: the dma consumes them via num_idxs_reg)
        cnts = np.stack([n_lo, n_hi], axis=-1).astype(np.int32)  # [CORES, NT, 2]

        # wrap idx arrays to the [16, n/16] layout: position i -> [i % 16, i // 16]
        def wrap16(a):  # [..., S] -> [..., 16, S//16]
            S = a.shape[-1]
            return np.ascontiguousarray(
                a.reshape(*a.shape[:-1], S // 16, 16).swapaxes(-1, -2)
            )

        GRP = 2 if NT % 2 == 0 else 1
        self.GRP, self.NGRP = GRP, NT // GRP
        self.d_arr = d_arr
        self.dst_arr = dst_arr
        self.idxlo_w = wrap16(idxlo)
        self.idxhi_w = wrap16(idxhi)
        self.idxlo_g = wrap16(idxlo.reshape(CORES, self.NGRP, GRP * LO_SLOTS))
        self.idxhi_g = wrap16(idxhi.reshape(CORES, self.NGRP, GRP * HI_SLOTS))
        self.cnts = cnts

        # ---- node-side per-core tables
        # x_ids slab: [CORES, SLAB] (pad -> 0)
        xids_slab = np.zeros((CORES, SLAB), np.int64)
        gid_slab = np.full((CORES, SLAB), -1.0, np.float16)  # local graph id
        for k in range(CORES):
            nodes = np.arange(cstart[k], cend[k])
            xids_slab[k, new_local[nodes]] = x_ids[nodes]
            gid_slab[k, new_local[nodes]] = (
                batch[nodes] - k * gpc).astype(np.float16)
        self.xids_w = wrap16(xids_slab.astype(np.int16))  # emb table < 32768 rows
        self.gid_slab = gid_slab.reshape(CORES, NT, 128)

        # inverse counts per graph (local)
        cnt_g = np.zeros((CORES, gpc), np.float32)
        for k in range(CORES):
            ids, c = np.unique(
                (batch[cstart[k]:cend[k]] - k * gpc), return_counts=True
            )
            cnt_g[k, ids] = c
        self.inv_cnt = (1.0 / np.maximum(cnt_g, 1.0)).astype(np.float32)  # [CORES,gpc]

        # ---- weights (augmented)
        w = weights
        self.W1b = np.concatenate(
            [w["blk_W1"], w["blk_b1"][:, None, :]], axis=1
        ).astype(np.float32)  # [NCONV, F+1, F]
        self.W2b = np.concatenate(
            [w["blk_W2"], w["blk_b2"][:, None, :]], axis=1
        ).astype(np.float32)
        self.Web = np.concatenate(
            [w["blk_We"], w["blk_be"][:, None, :]], axis=1
        ).astype(np.float16)  # [NCONV, NRBF+1, F]
        self.emb = np.asarray(w["emb"]).astype(np.float32)
        for nm in ("Ws", "bs", "Wbg1", "bbg1", "Wbg2", "bbg2",
                   "Weh1", "beh1", "Weh2", "beh2"):
            setattr(self, nm, np.asarray(w[nm]).astype(np.float32))

        # RBF offsets
        offs = np.linspace(0.0, 6.0, NRBF).astype(np.float32)
        self.offs = offs
        self.coeff = np.float32(-0.5 / (offs[1] - offs[0]) ** 2)



"""Bass/Tile kernel builder for the CrystalGNN (SchNet-style) message-passing net.

Data layout (per core, SPMD identical program):
  - nodes sharded by graph: core k owns graphs [k*gpc,(k+1)*gpc) -> a slab of
    SLAB node rows (NT = SLAB/128 tiles of 128 nodes).
  - x state [128, NT*F] f32 lives in SBUF for the whole kernel.
  - h table (x @ W1 + b1, fp16, padded to 128 cols) is written per-slab to DRAM
    and AllGather'd so every core can dma_gather rows of any node.
  - edges partitioned by owner(dst), grouped by dst tile; per tile a fixed
    budget of BK*128 edge slots (LO_BLKS lo-src + HI_BLKS hi-src blocks,
    src slab-row < / >= LOCUT for int16 gather indices).
  - per tile: gather h[src]; RBF e from distances; f = e_aug @ Web (PE);
    m = h*f (DVE, fp16); one-hot S from dst ids (DVE is_equal);
    aggT[64,128] += m.T @ S (PE, PSUM f32); x += aggT.T@W2+b2; softplus.
  - pool: pooledT[F,gpc] += x_tile.T @ onehot(graph); * 1/cnt; 3-layer MLP.
"""
import numpy as np
from contextlib import ExitStack

import concourse.bass as bass
import concourse.bacc as bacc
import concourse.mybir as mybir
from concourse import tile

F = 64
NRBF = 10
NCONV = 3
CORES = 8
AF = mybir.ActivationFunctionType
OP = mybir.AluOpType
DT = mybir.dt


# ln(1+t) ~= t*(P0 + P1 t + P2 t^2 + P3 t^3 + P4 t^4), t in [0,1]
# (max abs err 8.1e-5; exact 0 at t=0). Lets softplus avoid the Ln
# activation so a single act-func table serves the whole program.
LN1P_C = (0.99988793, -0.49636828, 0.30467236, -0.15602843, 0.04106451)


def build_bass(prep, unroll=10, debug=False, stop_after=None, py_loops=False,
               repeat=1, ag_strided=True, mixed_tt=True, chunk_ag=False,
               skip_conv_ag=False):
    """Returns (nc, input_names) — the SPMD program for all cores."""
    NT, BK = prep.NT, prep.BK
    LO_BLKS, HI_BLKS = prep.LO_BLKS, prep.HI_BLKS
    LO_SLOTS, HI_SLOTS = LO_BLKS * 128, HI_BLKS * 128
    SLAB, gpc, LOCUT = prep.SLAB, prep.gpc, prep.LOCUT
    GRP, NGRP = prep.GRP, prep.NGRP
    CS = CORES * SLAB
    coeff = float(prep.coeff)
    R = NRBF + 1
    WL = GRP * LO_SLOTS // 16
    WH = GRP * HI_SLOTS // 16

    nc = bacc.Bacc("TRN2", target_bir_lowering=False, debug=False,
                   num_devices=CORES)

    # ---------------- DRAM inputs ----------------
    def din(name, shape, dt):
        return nc.dram_tensor(name, list(shape), dt, kind="ExternalInput").ap()

    d_dram = din("d_edges", (NT * 128, BK), DT.float32)
    dst_dram = din("dst_edges", (NT * 128, BK), DT.float16)
    idxlo_dram = din("idxlo", (NGRP * 128, GRP * LO_SLOTS // 16), DT.int16)
    idxhi_dram = din("idxhi", (NGRP * 128, GRP * HI_SLOTS // 16), DT.int16)
    gid_dram = din("gid", (NT * 128, 1), DT.float16)
    xids_dram = din("xids", (128, SLAB // 16), DT.int16)
    emb_dram = din("emb95", (95, F), DT.float32)
    w1b_dram = din("w1b", (F + 1, NCONV * F), DT.float32)
    w2b_dram = din("w2b", (F + 1, NCONV * F), DT.float32)
    # block-diagonal Web for grouped f-matmuls: groups of FG=8 chunks (and a
    # tail group of BK%8 chunks). webbd8[(c,k),(c',j)] = Web[k,j] * (c==c')
    FG = 8
    TFG = BK % FG if BK % FG else FG  # tail group size
    webbd8_dram = din("webbd8", (FG * (NRBF + 1), NCONV * FG * F), DT.float16)
    webbdt_dram = din("webbdt", (TFG * (NRBF + 1), NCONV * TFG * F),
                      DT.float16)
    offs_dram = din("offs", (128, NRBF), DT.float32)
    iota_dram = din("iota128", (128, 128), DT.float16)
    iotag_dram = din("iotag", (128, gpc), DT.float32)
    ident16_dram = din("ident16", (128, 128), DT.float16)
    ident32_dram = din("ident32", (128, 128), DT.float32)
    invc_dram = din("invc", (F, gpc), DT.float32)
    ws_dram = din("ws", (F, 2 * F), DT.float32)
    bs_dram = din("bs", (2 * F, 1), DT.float32)
    wbg1_dram = din("wbg1", (2 * F, F), DT.float32)
    bbg1_dram = din("bbg1", (F, 1), DT.float32)
    wbg2_dram = din("wbg2", (F, 1), DT.float32)
    bbg2_dram = din("bbg2", (1, 1), DT.float32)
    weh1_dram = din("weh1", (2 * F, F), DT.float32)
    beh1_dram = din("beh1", (F, 1), DT.float32)
    weh2_dram = din("weh2", (F, 1), DT.float32)
    beh2_dram = din("beh2", (1, 1), DT.float32)

    dbg = {}
    if debug:
        dbg["x0"] = nc.dram_tensor("dbg_x0", [128, NT * F], DT.float32,
                                   kind="ExternalOutput").ap()
        dbg["hall0"] = nc.dram_tensor("dbg_hall0", [CS, 128], DT.float16,
                                      kind="ExternalOutput").ap()
        for i in range(NCONV):
            dbg[f"x{i+1}"] = nc.dram_tensor(f"dbg_x{i+1}", [128, NT * F],
                                            DT.float32,
                                            kind="ExternalOutput").ap()
        dbg["poolT"] = nc.dram_tensor("dbg_poolT", [F, gpc], DT.float32,
                                      kind="ExternalOutput").ap()
        dbg["cT"] = nc.dram_tensor("dbg_cT", [2 * F, gpc], DT.float32,
                                   kind="ExternalOutput").ap()
    h_all_t = nc.dram_tensor("h_all", [CS, 128], DT.float16,
                             addr_space="Shared")
    h_all2_t = nc.dram_tensor("h_all2", [CS, 128], DT.float16,
                              addr_space="Shared")
    h_bufs = (h_all_t, h_all2_t)
    obg_dram = nc.dram_tensor("obg", [gpc, 1], DT.float32,
                              kind="ExternalOutput").ap()
    oeh_dram = nc.dram_tensor("oeh", [gpc, 1], DT.float32,
                              kind="ExternalOutput").ap()

    with tile.TileContext(nc) as tc, ExitStack() as stk:
        cpool = stk.enter_context(tc.tile_pool(name="const", bufs=1))
        dpool = stk.enter_context(tc.tile_pool(name="dram", bufs=1,
                                               space="DRAM"))
        wk = stk.enter_context(tc.tile_pool(name="wk", bufs=4))
        wk2 = stk.enter_context(tc.tile_pool(name="wk2", bufs=2))
        conv_stk = ExitStack()
        gp = conv_stk.enter_context(tc.tile_pool(name="gp", bufs=4))
        pp = conv_stk.enter_context(tc.tile_pool(name="pp", bufs=3,
                                                 space="PSUM"))
        ppf = conv_stk.enter_context(tc.tile_pool(name="ppf", bufs=2,
                                                  space="PSUM"))
        ppx = conv_stk.enter_context(tc.tile_pool(name="ppx", bufs=1,
                                                  space="PSUM"))

        h_own = dpool.tile([SLAB, 128], DT.float16)

        # ---------------- persistent SBUF ----------------
        def load_const(name, ap_dram, shape, dt):
            t = cpool.tile(list(shape), dt, tag=name)
            nc.sync.dma_start(out=t[:], in_=ap_dram)
            return t

        w1b_sb = load_const("w1b", w1b_dram, (F + 1, NCONV * F), DT.float32)
        w2b_sb = load_const("w2b", w2b_dram, (F + 1, NCONV * F), DT.float32)
        webbd8_sb = load_const("webbd8", webbd8_dram,
                               (FG * (NRBF + 1), NCONV * FG * F), DT.float16)
        webbdt_sb = load_const("webbdt", webbdt_dram,
                               (TFG * (NRBF + 1), NCONV * TFG * F), DT.float16)
        offs_sb = load_const("offs", offs_dram, (128, NRBF), DT.float32)
        iota_sb = load_const("iota", iota_dram, (128, 128), DT.float16)
        iotag_sb = load_const("iotag", iotag_dram, (128, gpc), DT.float32)
        id16_sb = load_const("id16", ident16_dram, (128, 128), DT.float16)
        id32_sb = load_const("id32", ident32_dram, (128, 128), DT.float32)
        invc_sb = load_const("invc", invc_dram, (F, gpc), DT.float32)
        ws_sb = load_const("ws", ws_dram, (F, 2 * F), DT.float32)
        bs_sb = load_const("bs", bs_dram, (2 * F, 1), DT.float32)
        wbg1_sb = load_const("wbg1", wbg1_dram, (2 * F, F), DT.float32)
        bbg1_sb = load_const("bbg1", bbg1_dram, (F, 1), DT.float32)
        wbg2_sb = load_const("wbg2", wbg2_dram, (F, 1), DT.float32)
        bbg2_sb = load_const("bbg2", bbg2_dram, (1, 1), DT.float32)
        weh1_sb = load_const("weh1", weh1_dram, (2 * F, F), DT.float32)
        beh1_sb = load_const("beh1", beh1_dram, (F, 1), DT.float32)
        weh2_sb = load_const("weh2", weh2_dram, (F, 1), DT.float32)
        beh2_sb = load_const("beh2", beh2_dram, (1, 1), DT.float32)
        xids_sb = load_const("xids", xids_dram, (128, SLAB // 16), DT.int16)

        x_sb = cpool.tile([128, NT * F], DT.float32, tag="x")

        # persistent static edge-side state (loaded/computed once, reused
        # across all convs)
        dst16_sb = cpool.tile([128, NT * BK], DT.float16, tag="dst16")
        nc.sync.dma_start(
            out=dst16_sb[:].rearrange("p (t b) -> p t b", b=BK),
            in_=dst_dram.rearrange("(t p) b -> p t b", p=128),
        )
        ixlo_sb = cpool.tile([128, NGRP * WL], DT.int16, tag="ixlo")
        nc.sync.dma_start(
            out=ixlo_sb[:].rearrange("p (g w) -> p g w", w=WL),
            in_=idxlo_dram.rearrange("(g p) w -> p g w", p=128),
        )
        ixhi_sb = cpool.tile([128, NGRP * WH], DT.int16, tag="ixhi")
        nc.sync.dma_start(
            out=ixhi_sb[:].rearrange("p (g w) -> p g w", w=WH),
            in_=idxhi_dram.rearrange("(g p) w -> p g w", p=128),
        )
        e16_sb = cpool.tile([128, NT * BK * R], DT.float16, tag="e16")
        hbuf = cpool.tile([128, NT * F], DT.float16, tag="hbuf")
        # persistent transposed RBF: eT for every (tile, chunk-group),
        # computed once in the prologue (conv-independent)
        NGF = ceil_div(BK, FG)
        eTall = cpool.tile([FG * R, NT * NGF * 128], DT.float16, tag="eTall")

        # ---------------- x0 = emb[x_ids] ----------------
        nc.gpsimd.dma_gather(
            x_sb[:].rearrange("p (b e) -> p b e", e=F),
            emb_dram,
            xids_sb[:],
            SLAB,
            SLAB,
            F,
            single_packet=False,
        )

        # ---------------- helpers ----------------
        def h_chain(iv, i):
            """hbuf[:, tile] = fp16(x_tile @ W1[i] + b1[i])."""
            xcp = wk2.tile([128, F], DT.float32, tag="xcp")
            nc.scalar.copy(xcp[:], x_sb[:, bass.ts(iv, F)])
            xT_ps = ppx.tile([F, 128], DT.float32, tag="xps")
            nc.tensor.transpose(xT_ps[:], xcp[:], id32_sb[:])
            xT_sb = wk2.tile([F + 1, 128], DT.float32, tag="xT")
            nc.scalar.copy(xT_sb[0:F, :], xT_ps[:])
            nc.vector.memset(xT_sb[F:F + 1, :], 1.0)
            h_ps = ppx.tile([128, F], DT.float32, tag="xps2")
            nc.tensor.matmul(h_ps[:], xT_sb[:], w1b_sb[:, i * F:(i + 1) * F],
                             start=True, stop=True)
            nc.scalar.copy(hbuf[:, bass.ts(iv, F)], h_ps[:])

        # h-chunk boundaries (in tiles) for the overlapped AllGather: each
        # chunk is flushed + allgathered as soon as its h tiles are done, so
        # the collective overlaps the remaining edge compute of the conv.
        AGC = sorted({round(NT * k / 4) for k in range(5)})
        if not chunk_ag:
            AGC = [0, NT]

        def flush_chunk(t0, t1):
            """DMA: hbuf tiles [t0,t1) (SBUF) -> h_own rows [:, 0:64]."""
            nc.sync.dma_start(
                out=h_own[:][t0 * 128:t1 * 128, 0:F].rearrange(
                    "(t p) c -> p t c", p=128),
                in_=hbuf[:, t0 * F:t1 * F].rearrange("p (t c) -> p t c", c=F),
            )

        def ag_chunk(dst_t, t0, t1):
            flush_chunk(t0, t1)
            nc.gpsimd.collective_compute(
                "AllGather",
                OP.bypass,
                replica_groups=[list(range(CORES))],
                ins=[h_own[:][t0 * 128:t1 * 128, :].opt()],
                outs=[dst_t[:].rearrange("(c s) f -> c s f", s=SLAB)[
                    :, t0 * 128:t1 * 128, :].opt()],
            )

        def gather_group(gv, hs_lo, hs_hi, h_src):
            nc.gpsimd.dma_gather(
                hs_lo[:].rearrange("p (b e) -> p b e", e=128),
                h_src[:], ixlo_sb[:, bass.ts(gv, WL)],
                GRP * LO_SLOTS, GRP * LO_SLOTS, 128, single_packet=False,
            )
            nc.gpsimd.dma_gather(
                hs_hi[:].rearrange("p (b e) -> p b e", e=128),
                h_src[:][LOCUT:CS, :], ixhi_sb[:, bass.ts(gv, WH)],
                GRP * HI_SLOTS, GRP * HI_SLOTS, 128, single_packet=False,
            )

        def edge_phase(iv, i, tg, hs_lo, hs_hi):
            """Returns aggT psum tile [F, 128] accumulated over the tile."""
            hsl3 = hs_lo[:].rearrange("p (b e) -> p b e", e=128)
            hsh3 = hs_hi[:].rearrange("p (b e) -> p b e", e=128)
            tile_e16 = e16_sb[:, bass.ts(iv, BK * R)]
            # local static-offset copy of this tile's dst ids (cheap; lets
            # the broadcast APs below use raw strides)
            dst_sb = wk.tile([128, BK], DT.float16, tag="dst")
            nc.gpsimd.tensor_copy(dst_sb[:], dst16_sb[:, bass.ts(iv, BK)])

            aggT_ps = pp.tile([F, 128], DT.float32, tag="aggT")

            # f / m / S / scatter in groups of FG chunks; per group one
            # transpose of e16 cols -> eT [fn*11, 128], one block-diag matmul
            eT_tile = eTall[:, bass.ts(iv, NGF * 128)]
            for g0 in range(0, BK, FG):
                fn = min(FG, BK - g0)
                gi = g0 // FG
                f_ps = ppf.tile([128, FG * F], DT.float32, tag="fps")
                bd = webbd8_sb if fn == FG else webbdt_sb
                nc.tensor.matmul(
                    f_ps[:, 0:fn * F],
                    eT_tile[0:fn * R, gi * 128:(gi + 1) * 128],
                    bd[:, i * fn * F:(i + 1) * fn * F],
                    start=True, stop=True,
                )
                if mixed_tt:
                    f3 = f_ps[:].rearrange("p (b e) -> p b e", e=F)
                else:
                    f16 = wk.tile([128, FG * F], DT.float16, tag="f16")
                    nc.scalar.copy(f16[0:128, 0:fn * F], f_ps[:, 0:fn * F])
                    f3 = f16[:].rearrange("p (b e) -> p b e", e=F)
                m_sb = wk.tile([128, FG * F], DT.float8e4, tag="m")
                segs = []
                c0, c1 = g0, g0 + fn
                if c0 < LO_BLKS:
                    segs.append((hsl3, tg * LO_BLKS + c0, c0,
                                 min(c1, LO_BLKS) - c0))
                if c1 > LO_BLKS:
                    cc0 = max(c0, LO_BLKS)
                    segs.append((hsh3, tg * HI_BLKS + (cc0 - LO_BLKS),
                                 cc0, c1 - cc0))
                for (src3, b0, coff, n) in segs:
                    nc.vector.tensor_tensor(
                        m_sb[:].rearrange("p (b e) -> p b e", e=F)[
                            :, coff - g0:coff - g0 + n, :],
                        src3[:, b0:b0 + n, 0:F],
                        f3[:, coff - g0:coff - g0 + n, :],
                        OP.mult,
                    )
                S_sb = wk.tile([128, FG * 128], DT.float8e4, tag="S")
                dst_b = bass.AP(
                    dst_sb.tensor,
                    dst_sb[:, g0:g0 + fn].offset,
                    [dst_sb[:].ap[0], [1, fn], [0, 128]],
                )
                iota_b = bass.AP(
                    iota_sb.tensor, iota_sb[:].offset,
                    [iota_sb[:].ap[0], [0, fn], [1, 128]],
                )
                nc.vector.tensor_tensor(
                    S_sb[:].rearrange("p (b e) -> p b e", e=128)[:, 0:fn, :],
                    dst_b, iota_b, OP.is_equal,
                )
                for c in range(fn):
                    cg = g0 + c
                    nc.tensor.matmul(
                        aggT_ps[:],
                        m_sb[:, c * F:(c + 1) * F],
                        S_sb[:, c * 128:(c + 1) * 128],
                        start=(cg == 0), stop=(cg == BK - 1),
                    )
            return aggT_ps

        def x_accum(iv, i, tg, aggT_ps, xs_g):
            """xs_g[:, tg*F:] = x + agg @ W2 + b2 (pre-softplus)."""
            aggT_sb = wk2.tile([F + 1, 128], DT.float32, tag="aggTs")
            nc.scalar.copy(aggT_sb[0:F, :], aggT_ps[:])
            nc.vector.memset(aggT_sb[F:F + 1, :], 1.0)
            xup_ps = ppx.tile([128, F], DT.float32, tag="xps2")
            nc.tensor.matmul(xup_ps[:], aggT_sb[:],
                             w2b_sb[:, i * F:(i + 1) * F],
                             start=True, stop=True)
            nc.vector.tensor_tensor(xs_g[:, tg * F:(tg + 1) * F], xup_ps[:],
                                    x_sb[:, bass.ts(iv, F)], OP.add)

        def softplus_group(gv, xs_g):
            """x_sb[group] = relu(xs) + ln1p(exp(-|xs|)), poly ln1p on Pool.

            Uses only Abs/Exp activations so one act-func table set serves
            the whole program (no per-tile table reloads).
            """
            n = GRP * F
            ab = wk2.tile([128, n], DT.float32, tag="ab")
            nc.scalar.activation(ab[:], xs_g[:], AF.Abs)
            nc.scalar.activation(ab[:], ab[:], AF.Exp, scale=-1.0)
            acc = wk2.tile([128, n], DT.float32, tag="acc")
            nc.vector.tensor_scalar(acc[:], ab[:], LN1P_C[4], LN1P_C[3],
                                    OP.mult, OP.add)
            for ck in (LN1P_C[2], LN1P_C[1], LN1P_C[0]):
                nc.vector.tensor_tensor(acc[:], acc[:], ab[:], OP.mult)
                nc.vector.tensor_scalar_add(acc[:], acc[:], ck)
            nc.vector.tensor_tensor(acc[:], acc[:], ab[:], OP.mult)
            nc.vector.scalar_tensor_tensor(
                x_sb[:, bass.ts(gv, GRP * F)], xs_g[:], 0.0, acc[:],
                OP.max, OP.add)

        # ---------------- prologue: e16 (RBF) + h0 ----------------
        nc.vector.memset(e16_sb[:], 1.0)  # aug ones cols; exp fills the rest

        def body_pro(iv):
            d_sb = wk.tile([128, BK], DT.float32, tag="d")
            nc.sync.dma_start(
                out=d_sb[:],
                in_=d_dram.rearrange("(t p) b -> t p b", p=128)[
                    bass.ds(iv, 1)][0],
            )
            e32 = wk.tile([128, BK * NRBF], DT.float32, tag="e32")
            d_b = bass.AP(d_sb.tensor, d_sb[:].offset,
                          [d_sb[:].ap[0], d_sb[:].ap[1], [0, NRBF]])
            offs_b = bass.AP(offs_sb.tensor, offs_sb[:].offset,
                             [offs_sb[:].ap[0], [0, BK], offs_sb[:].ap[1]])
            e32_3 = e32[:].rearrange("p (b r) -> p b r", r=NRBF)
            nc.vector.tensor_tensor(e32_3, d_b, offs_b, OP.subtract)
            nc.vector.tensor_tensor(e32[:], e32[:], e32[:], OP.mult)
            tile_e16 = e16_sb[:, bass.ts(iv, BK * R)]
            e16_t = tile_e16.rearrange("p (b r) -> p b r", r=R)
            nc.scalar.activation(e16_t[:, :, 0:NRBF], e32_3, AF.Exp,
                                 scale=coeff)
            # transpose each chunk-group's eT once (reused by all convs)
            eT_tile = eTall[:, bass.ts(iv, NGF * 128)]
            for g0 in range(0, BK, FG):
                fn = min(FG, BK - g0)
                gi = g0 // FG
                eT_ps = ppx.tile([FG * R, 128], DT.float16, tag="eTp")
                nc.tensor.transpose(
                    eT_ps[0:fn * R, :],
                    tile_e16[:, g0 * R:(g0 + fn) * R],
                    id16_sb[:],
                )
                nc.scalar.copy(eT_tile[0:fn * R, gi * 128:(gi + 1) * 128],
                               eT_ps[0:fn * R, :])
            h_chain(iv, 0)

        # prologue tile loop with chunked h0 allgathers into h_bufs[0]
        ag_i = 0
        for t in range(NT):
            body_pro(t)
            while ag_i + 1 < len(AGC) and t + 1 == AGC[ag_i + 1]:
                ag_chunk(h_bufs[0], AGC[ag_i], AGC[ag_i + 1])
                ag_i += 1
        stopped = stop_after in ("h0", "ag0")
        if debug:
            nc.sync.dma_start(out=dbg["x0"], in_=x_sb[:])
            nc.sync.dma_start(out=dbg["hall0"], in_=h_all_t[:])

        conv_c = 0  # running conv index for h_all ping-pong
        for rep in range(repeat):
            if stopped:
                break
            for i in range(NCONV):
                if stopped or (stop_after is not None
                               and stop_after.startswith("conv")
                               and i > int(stop_after[4:])):
                    stopped = True
                    break
                last = (rep == repeat - 1) and (i == NCONV - 1)
                rbuf = h_bufs[conv_c % 2]
                wbuf = h_bufs[(conv_c + 1) % 2]

                def body_conv_group(gv, i=i, last=last, rbuf=rbuf):
                    hs_lo = gp.tile([128, GRP * LO_SLOTS], DT.float16,
                                    tag="hslo")
                    hs_hi = gp.tile([128, GRP * HI_SLOTS], DT.float16,
                                    tag="hshi")
                    gather_group(gv, hs_lo, hs_hi, rbuf)
                    xs_g = gp.tile([128, GRP * F], DT.float32, tag="xsg")
                    for tg in range(GRP):
                        iv = gv * GRP + tg
                        aggT_ps = edge_phase(iv, i, tg, hs_lo, hs_hi)
                        x_accum(iv, i, tg, aggT_ps, xs_g)
                    softplus_group(gv, xs_g)
                    if not last:
                        for tg in range(GRP):
                            h_chain(gv * GRP + tg, (i + 1) % NCONV)

                ag_i = 0
                for g in range(NGRP):
                    body_conv_group(g)
                    if not last:
                        # issue each h chunk's allgather as soon as its
                        # tiles' h_chain is done (overlaps edge compute)
                        while (ag_i + 1 < len(AGC)
                               and (g + 1) * GRP >= AGC[ag_i + 1]):
                            if not skip_conv_ag:  # timing-probe mode
                                ag_chunk(wbuf, AGC[ag_i], AGC[ag_i + 1])
                            ag_i += 1
                if stop_after == f"conv{i}" and not last:
                    stopped = True
                    break
                if debug and rep == repeat - 1:
                    nc.sync.dma_start(out=dbg[f"x{i+1}"], in_=x_sb[:])
                conv_c += 1

        # ---------------- pooling ----------------
        if stopped:
            zz = wk.tile([1, gpc], DT.float32, tag="zz")
            nc.vector.memset(zz[:], 0.0)
            nc.sync.dma_start(out=obg_dram.rearrange("g one -> one g")[0:1, :],
                              in_=zz[:])
            nc.sync.dma_start(out=oeh_dram.rearrange("g one -> one g")[0:1, :],
                              in_=zz[:])
            conv_stk.close()
            do_rest = False
        else:
            conv_stk.close()
            do_rest = True
        if do_rest:
            ppm = stk.enter_context(tc.tile_pool(name="ppm", bufs=1,
                                                 space="PSUM"))
            poolT_ps = ppm.tile([F, gpc], DT.float32, tag="poolT")

            def body_pool(iv):
                gid_sb = wk.tile([128, 1], DT.float16, tag="gid")
                nc.sync.dma_start(
                    out=gid_sb[:],
                    in_=gid_dram.rearrange("(t p) b -> t p b", p=128)[
                        bass.ds(iv, 1)][0],
                )
                gid32 = wk.tile([128, 1], DT.float32, tag="gid32")
                nc.scalar.copy(gid32[:], gid_sb[:])
                Sp = wk.tile([128, gpc], DT.float32, tag="Sp")
                gid_b = bass.AP(gid32.tensor, gid32[:].offset,
                                [gid32[:].ap[0], [0, gpc]])
                nc.vector.tensor_tensor(Sp[:], gid_b, iotag_sb[:], OP.is_equal)
                nc.tensor.matmul(poolT_ps[:], x_sb[:, iv * F:(iv + 1) * F],
                                 Sp[:], start=(iv == 0), stop=(iv == NT - 1))

            for t in range(NT):
                body_pool(t)

            # mean + MLP (feature-major: cT = relu(Ws.T @ pooled + bs))
            poolT_sb = wk.tile([F, gpc], DT.float32, tag="poolTs")
            nc.vector.tensor_tensor(poolT_sb[:], poolT_ps[:], invc_sb[:], OP.mult)
            cT_ps = ppm.tile([2 * F, gpc], DT.float32, tag="cT")
            nc.tensor.matmul(cT_ps[:], ws_sb[:], poolT_sb[:], start=True,
                             stop=True)
            cT_sb = wk.tile([2 * F, gpc], DT.float32, tag="cTs")
            nc.scalar.activation(cT_sb[:], cT_ps[:], AF.Relu, bias=bs_sb[:])
            if debug:
                nc.sync.dma_start(out=dbg["poolT"], in_=poolT_sb[:])
                nc.sync.dma_start(out=dbg["cT"], in_=cT_sb[:])

            for (w1s, b1s, w2s, b2s, out_dram, tg) in (
                (wbg1_sb, bbg1_sb, wbg2_sb, bbg2_sb, obg_dram, "bg"),
                (weh1_sb, beh1_sb, weh2_sb, beh2_sb, oeh_dram, "eh"),
            ):
                t1_ps = ppm.tile([F, gpc], DT.float32, tag="t1")
                nc.tensor.matmul(t1_ps[:], w1s[:], cT_sb[:], start=True, stop=True)
                t1_sb = wk.tile([F, gpc], DT.float32, tag="t1s" + tg)
                nc.scalar.activation(t1_sb[:], t1_ps[:], AF.Relu, bias=b1s[:])
                o_ps = ppm.tile([1, gpc], DT.float32, tag="o")
                nc.tensor.matmul(o_ps[:], w2s[:], t1_sb[:], start=True, stop=True)
                o_sb = wk.tile([1, gpc], DT.float32, tag="os" + tg)
                nc.scalar.activation(o_sb[:], o_ps[:], AF.Identity, bias=b2s[:])
                nc.sync.dma_start(out=out_dram.rearrange("g one -> one g")[0:1, :],
                                  in_=o_sb[:])

    nc.compile()
    return nc


def make_in_maps(prep):
    """Per-core input dicts for run_bass_kernel_spmd."""
    NT, BK, SLAB, gpc = prep.NT, prep.BK, prep.SLAB, prep.gpc
    LO_SLOTS, HI_SLOTS = prep.LO_BLKS * 128, prep.HI_BLKS * 128
    maps = []
    iota128 = np.tile(np.arange(128, dtype=np.float16)[None, :], (128, 1))
    iotag = np.tile(np.arange(gpc, dtype=np.float32)[None, :], (128, 1))
    id16 = np.eye(128, dtype=np.float16)
    id32 = np.eye(128, dtype=np.float32)
    offs = np.tile(prep.offs[None, :], (128, 1)).astype(np.float32)

    def rep16(a):  # [16, S] -> [128, S] replicated
        return np.tile(a, (8, 1))

    R = NRBF + 1
    FG = 8
    TFG = BK % FG if BK % FG else FG

    def make_bd(fn):
        bd = np.zeros((fn * R, NCONV * fn * F), np.float16)
        for i in range(NCONV):
            for c in range(fn):
                bd[c * R:(c + 1) * R, i * fn * F + c * F:i * fn * F +
                   (c + 1) * F] = prep.Web[i]
        return bd

    webbd8 = make_bd(FG)
    webbdt = make_bd(TFG)

    for k in range(CORES):
        m = dict(
            d_edges=prep.d_arr[k].reshape(NT * 128, BK),
            dst_edges=prep.dst_arr[k].reshape(NT * 128, BK),
            idxlo=np.ascontiguousarray(
                np.tile(prep.idxlo_g[k], (1, 8, 1)).reshape(
                    prep.NGRP * 128, prep.GRP * LO_SLOTS // 16)),
            idxhi=np.ascontiguousarray(
                np.tile(prep.idxhi_g[k], (1, 8, 1)).reshape(
                    prep.NGRP * 128, prep.GRP * HI_SLOTS // 16)),
            gid=prep.gid_slab[k].reshape(NT * 128, 1).astype(np.float16),
            xids=rep16(prep.xids_w[k]),
            emb95=prep.emb,
            w1b=np.ascontiguousarray(
                prep.W1b.transpose(1, 0, 2).reshape(F + 1, NCONV * F)),
            w2b=np.ascontiguousarray(
                prep.W2b.transpose(1, 0, 2).reshape(F + 1, NCONV * F)),
            webbd8=webbd8,
            webbdt=webbdt,
            offs=offs,
            iota128=iota128,
            iotag=iotag,
            ident16=id16,
            ident32=id32,
            invc=np.tile(prep.inv_cnt[k][None, :], (F, 1)).astype(np.float32),
            ws=prep.Ws,
            bs=prep.bs.reshape(2 * F, 1),
            wbg1=prep.Wbg1,
            bbg1=prep.bbg1.reshape(F, 1),
            wbg2=prep.Wbg2,
            bbg2=prep.bbg2.reshape(1, 1),
            weh1=prep.Weh1,
            beh1=prep.beh1.reshape(F, 1),
            weh2=prep.Weh2,
            beh2=prep.beh2.reshape(1, 1),
        )
        maps.append({k2: np.ascontiguousarray(v) for k2, v in m.items()})
    return maps


def kernel(**inputs):
    import numpy as np
    from concourse.bass_utils import run_bass_kernel_spmd

    wkeys = ("emb blk_W1 blk_b1 blk_We blk_be blk_W2 blk_b2 Ws bs Wbg1 bbg1 "
             "Wbg2 bbg2 Weh1 beh1 Weh2 beh2").split()
    weights = {k: np.asarray(inputs[k]) for k in wkeys}
    p = Prep(np.asarray(inputs["x_ids"]), np.asarray(inputs["edge_index"]),
             np.asarray(inputs["edge_attr"]), np.asarray(inputs["batch"]),
             weights, n_graphs=512)
    nc = build_bass(p, unroll=25)
    maps = make_in_maps(p)
    res = run_bass_kernel_spmd(nc, maps, list(range(CORES)))
    bg = np.concatenate([np.asarray(res.results[k]["obg"], dtype=np.float32)
                         for k in range(CORES)])
    eh = np.concatenate([np.asarray(res.results[k]["oeh"], dtype=np.float32)
                         for k in range(CORES)])
    return bg, eh

